# revision 1
# baseline (speedup 1.0000x reference)
# Self-contained 8-core Trainium2 Bass kernel for the 2-layer GAT + mean-pool
# problem (nn_GAT_83820581749190).
#
# Sharding: destination nodes (and all their incident edges) are partitioned
# across the 8 cores, so each layer's attention softmax and aggregation
# complete locally per core. Each core builds a replicated layer-1 feature
# table [h1 | al_src] (bf16, 512-byte rows) in HBM with a replicated
# x @ W1ext matmul, edge-gathers rows with the GPSIMD dma_gather custom op
# (int16 indices force a lo/hi table-half split), computes the edge softmax
# without segment-max (logits are small enough that exp cannot overflow), and
# aggregates per-destination with identity-matmul PSUM accumulation
# (destinations sit on partitions via degree-bucketed groups of 128).
# Layer-2 features are exchanged with an AllGather; mean-pool is a matmul
# against a host-built one-hot graph matrix plus a tiny AllReduce.
import numpy as np
import ml_dtypes

N = 50000
E = 800000
IN = 128
HID = 32
HEADS = 4
OUT = 10
GPOOL = 64
NEG = 0.2
NCORES = 8
S = N // NCORES
LO_MAX = 32767          # max usable int16 gather index
SPECIAL_ALS = -100.0    # al_src of pad rows: exp(0.2*(-100+ald)) ~ 2e-9
SB_BLOCK_BUDGET = 48    # max gather blocks per superblock
XCHUNK = 512
PHASES = 99  # debug: 1=X only, 2=+L1, 3=+exchange, 4=+L2, 5=full
L1STEP = 99  # debug: 1=gathers 2=+softmax 3=+exh 4=+agg 5=+epilogue 6=+scatter

bf16 = ml_dtypes.bfloat16


def _ceil_to(v, m):
    return (v + m - 1) // m * m


# ======================= host prep =========================================

def _boundary_aware_order(deg_lo, deg_hi):
    """Sort ids by (lo desc, hi desc), but fill 128-groups that straddle a
    lo-run boundary from the *small-hi tail* of the next run, keeping
    per-group max_lo + max_hi tight."""
    Sn = len(deg_lo)
    base = np.lexsort((-deg_hi, -deg_lo))
    glo = deg_lo[base]
    runs = []
    i = 0
    while i < Sn:
        j = i
        while j < Sn and glo[j] == glo[i]:
            j += 1
        runs.append(list(base[i:j]))
        i = j
    order = []
    ri = 0
    fronts = [0] * len(runs)
    backs = [len(r) for r in runs]
    while len(order) < Sn:
        while ri < len(runs) and fronts[ri] >= backs[ri]:
            ri += 1
        if ri >= len(runs):
            break
        need = 128 - (len(order) % 128)
        avail = backs[ri] - fronts[ri]
        if avail >= need:
            order.extend(runs[ri][fronts[ri]:fronts[ri] + need])
            fronts[ri] += need
        else:
            order.extend(runs[ri][fronts[ri]:backs[ri]])
            fronts[ri] = backs[ri]
            need -= avail
            rj = ri + 1
            while need > 0 and rj < len(runs):
                a = backs[rj] - fronts[rj]
                t = min(a, need)
                order.extend(reversed(runs[rj][backs[rj] - t:backs[rj]]))
                backs[rj] -= t
                need -= t
                rj += 1
    P = np.asarray(order, np.int64)
    Ppos = np.empty(Sn, np.int64)
    Ppos[P] = np.arange(Sn)
    return P, Ppos


def _run_groups(glo, ghi, max_rows=128):
    Sn = len(glo)
    ng = (Sn + max_rows - 1) // max_rows
    dlo = np.zeros(ng, np.int64)
    dhi = np.zeros(ng, np.int64)
    for g in range(ng):
        s, e = g * max_rows, min((g + 1) * max_rows, Sn)
        dlo[g] = glo[s:e].max()
        dhi[g] = ghi[s:e].max()
    return dlo, dhi


def _build_layer(src, dstl, is_lo):
    deg_lo = np.bincount(dstl[is_lo], minlength=S)
    deg_hi = np.bincount(dstl[~is_lo], minlength=S)
    P, Ppos = _boundary_aware_order(deg_lo, deg_hi)
    dlo, dhi = _run_groups(deg_lo[P], deg_hi[P])
    return dict(src=src, dstl=dstl, is_lo=is_lo, deg_lo=deg_lo, deg_hi=deg_hi,
                P=P, Ppos=Ppos, dlo=dlo, dhi=dhi)


def _emit_slots(l, DLO, DHI, idx_lo_of, idx_hi_of, special_lo, special_hi):
    NG = len(DLO)
    src, is_lo = l["src"], l["is_lo"]
    Ppos = l["Ppos"]
    nreal = len(l["P"])
    slot2cmp = np.full(NG * 128, -1, np.int64)
    slot2cmp[:nreal] = np.arange(nreal)
    idx_lo = [np.full((int(DLO[g]), 128), special_lo, np.int64)
              for g in range(NG)]
    idx_hi = [np.full((int(DHI[g]), 128), special_hi, np.int64)
              for g in range(NG)]
    slot_of_edge = Ppos[l["dstl"]]
    order = np.argsort(slot_of_edge, kind="stable")
    for mask, arrs, idx_fn in ((is_lo, idx_lo, idx_lo_of),
                               (~is_lo, idx_hi, idx_hi_of)):
        m = mask[order]
        so = slot_of_edge[order][m]
        sr = src[order][m]
        jj = np.arange(len(so)) - np.searchsorted(so, so, side="left")
        gg, kk = so // 128, so % 128
        vals = idx_fn(sr)
        for g in range(NG):
            sel = gg == g
            if sel.any():
                arrs[g][jj[sel], kk[sel]] = vals[sel]
    return idx_lo, idx_hi, slot2cmp


def _wrap16(idx):
    """[n] -> [128, n//16] int16: idx i at [i%16, i//16], replicated x8."""
    n = len(idx)
    assert n % 16 == 0
    w = np.ascontiguousarray(np.asarray(idx).reshape(n // 16, 16).T)
    w = w.astype(np.int16)
    return np.tile(w, (8, 1))


def _wrap_groups(arrs):
    segs = [_wrap16(a.reshape(-1)) if a.size else np.zeros((128, 0), np.int16)
            for a in arrs]
    return np.concatenate(segs, axis=1) if segs else np.zeros((128, 0), np.int16)


def host_prep(x, edge_index, batch, W1, a1_src, a1_dst, b1, W2, a2_src, a2_dst,
              b2, Wl, bl):
    x = np.asarray(x, np.float32)
    edge_index = np.asarray(edge_index, np.int64)
    batch = np.asarray(batch, np.int64)
    src_all = np.concatenate([edge_index[0], np.arange(N, dtype=np.int64)])
    dst_all = np.concatenate([edge_index[1], np.arange(N, dtype=np.int64)])
    owner = dst_all // S

    a1_src = np.asarray(a1_src, np.float32)
    a1_dst = np.asarray(a1_dst, np.float32)
    W1 = np.asarray(W1, np.float32)
    W2 = np.asarray(W2, np.float32)
    As1 = np.zeros((HEADS * HID, HEADS), np.float32)
    Ad1 = np.zeros((HEADS * HID, HEADS), np.float32)
    for h in range(HEADS):
        As1[h * HID:(h + 1) * HID, h] = a1_src[h]
        Ad1[h * HID:(h + 1) * HID, h] = a1_dst[h]
    W1ext = np.concatenate([W1, W1 @ As1, W1 @ Ad1], axis=1)   # [128,136]
    W2ext = np.concatenate(
        [W2, W2 @ np.asarray(a2_src, np.float32)[0][:, None],
         W2 @ np.asarray(a2_dst, np.float32)[0][:, None]], axis=1)  # [128,34]

    cores = [dict(c=c) for c in range(NCORES)]
    for cd in cores:
        c = cd["c"]
        m = owner == c
        cd["src"] = src_all[m]
        cd["dstl"] = dst_all[m] - c * S

    # ---------- layer 1 ----------
    for cd in cores:
        c = cd["c"]
        pos_of = np.empty(N, np.int64)
        own = np.arange(c * S, (c + 1) * S)
        oth = np.concatenate([np.arange(0, c * S), np.arange((c + 1) * S, N)])
        pos_of[oth] = S + np.arange(N - S)
        pos_of[own] = 0
        is_lo1 = pos_of[cd["src"]] < LO_MAX
        l1 = _build_layer(cd["src"], cd["dstl"], is_lo1)
        pos_of[own] = l1["Ppos"]
        row_of = np.where(pos_of < LO_MAX, pos_of, pos_of + 1)
        cd["l1"] = l1
        cd["row_of"] = row_of
    NG1 = max(len(cd["l1"]["dlo"]) for cd in cores)
    DLO1 = np.zeros(NG1, np.int64)
    DHI1 = np.zeros(NG1, np.int64)
    for cd in cores:
        d = cd["l1"]
        DLO1[:len(d["dlo"])] = np.maximum(DLO1[:len(d["dlo"])], d["dlo"])
        DHI1[:len(d["dhi"])] = np.maximum(DHI1[:len(d["dhi"])], d["dhi"])
    for cd in cores:
        r = cd["row_of"]
        cd["idx1_lo"], cd["idx1_hi"], cd["slot2cmp1"] = _emit_slots(
            cd["l1"], DLO1, DHI1,
            lambda s, r=r: r[s], lambda s, r=r: r[s] - (LO_MAX + 1),
            LO_MAX, N + 1 - (LO_MAX + 1))

    # ---------- layer 2 ----------
    # core-5's own positions straddle LO_MAX; freeze its lo membership first
    cd5 = cores[5]
    alo = cd5["src"] // S <= 5
    da = np.bincount(cd5["dstl"][alo], minlength=S)
    db = np.bincount(cd5["dstl"][~alo], minlength=S)
    P5a, _ = _boundary_aware_order(da, db)
    n_lo5 = LO_MAX - 5 * S
    lo5_set = np.zeros(S, bool)
    if n_lo5 > 0:
        lo5_set[P5a[:n_lo5]] = True

    def lo2_mask_of(src):
        ow = src // S
        lo = (ow <= 4).copy()
        m5 = ow == 5
        lo[m5] = lo5_set[src[m5] - 5 * S]
        return lo

    for cd in cores:
        cd["l2"] = _build_layer(cd["src"], cd["dstl"], lo2_mask_of(cd["src"]))
    l25 = cores[5]["l2"]
    idsA = np.where(lo5_set)[0]
    idsB = np.where(~lo5_set)[0]
    PA, _ = _boundary_aware_order(l25["deg_lo"][idsA], l25["deg_hi"][idsA])
    PB, _ = _boundary_aware_order(l25["deg_lo"][idsB], l25["deg_hi"][idsB])
    P5 = np.concatenate([idsA[PA], idsB[PB]])
    P5pos = np.empty(S, np.int64)
    P5pos[P5] = np.arange(S)
    l25["P"] = P5
    l25["Ppos"] = P5pos
    l25["dlo"], l25["dhi"] = _run_groups(l25["deg_lo"][P5], l25["deg_hi"][P5])

    pos2_of = np.empty(N, np.int64)
    for cd in cores:
        c = cd["c"]
        pos2_of[c * S:(c + 1) * S] = c * S + cd["l2"]["Ppos"]
    row2_of = pos2_of + 1
    NG2 = max(len(cd["l2"]["dlo"]) for cd in cores)
    DLO2 = np.zeros(NG2, np.int64)
    DHI2 = np.zeros(NG2, np.int64)
    for cd in cores:
        d = cd["l2"]
        DLO2[:len(d["dlo"])] = np.maximum(DLO2[:len(d["dlo"])], d["dlo"])
        DHI2[:len(d["dhi"])] = np.maximum(DHI2[:len(d["dhi"])], d["dhi"])
    for cd in cores:
        l2 = cd["l2"]
        assert (row2_of[l2["src"][l2["is_lo"]]] <= LO_MAX).all()
        assert (row2_of[l2["src"][~l2["is_lo"]]] > LO_MAX).all()
        cd["idx2_lo"], cd["idx2_hi"], cd["slot2cmp2"] = _emit_slots(
            l2, DLO2, DHI2,
            lambda s: row2_of[s], lambda s: row2_of[s] - (LO_MAX + 1),
            0, N + 1 - (LO_MAX + 1))

    # ---------- aux ----------
    cnt = np.bincount(batch, minlength=GPOOL).astype(np.float32)
    recip_cnt = (1.0 / np.maximum(cnt, 1.0)).astype(np.float32)

    for cd in cores:
        c = cd["c"]
        gids = batch[c * S:(c + 1) * S]
        Mp = np.zeros((NG2 * 128, GPOOL), np.float32)
        s2c = cd["slot2cmp2"]
        real = s2c >= 0
        Mp[np.where(real)[0], gids[cd["l2"]["P"][s2c[real]]]] = 1.0
        cd["mpool"] = Mp.astype(bf16)

        s2c1 = cd["slot2cmp1"]
        tgt = np.full(len(s2c1), S, np.int64)  # trash row for dummy slots
        r1 = s2c1 >= 0
        tgt[r1] = cd["l2"]["Ppos"][cd["l1"]["P"][s2c1[r1]]]
        cd["aldidx1"] = np.where(s2c1 >= 0, s2c1, 0)
        cd["aldidx2"] = np.where(cd["slot2cmp2"] >= 0, cd["slot2cmp2"], 0)

        xt = np.zeros((IN, _ceil_to(N + 2, XCHUNK)), np.float32)
        xt[:, cd["row_of"]] = x.T
        cd["xT"] = xt.astype(bf16)

        cd["w_idx1lo"] = _wrap_groups(cd["idx1_lo"])
        cd["w_idx1hi"] = _wrap_groups(cd["idx1_hi"])
        cd["w_idx2lo"] = _wrap_groups(cd["idx2_lo"])
        cd["w_idx2hi"] = _wrap_groups(cd["idx2_hi"])
        cd["w_ald1"] = _wrap16(cd["aldidx1"])
        cd["w_ald2"] = _wrap16(cd["aldidx2"])
        cd["w_scat1"] = _wrap16(tgt)

    patch1 = np.zeros((2, 256), np.float32)
    patch1[:, 128:132] = SPECIAL_ALS
    patch2 = np.zeros((2, 64), np.float32)
    patch2[:, 32] = SPECIAL_ALS

    return dict(cores=cores,
                DLO1=[int(v) for v in DLO1], DHI1=[int(v) for v in DHI1],
                DLO2=[int(v) for v in DLO2], DHI2=[int(v) for v in DHI2],
                W1ext=W1ext.astype(bf16), W2ext=W2ext.astype(bf16),
                Wl=np.asarray(Wl, np.float32),
                b1=np.tile(np.asarray(b1, np.float32).reshape(1, -1),
                           (128, 1)),
                b2=np.tile(np.asarray(b2, np.float32).reshape(1, -1),
                           (128, 1)),
                bl=np.tile(np.asarray(bl, np.float32).reshape(1, -1),
                           (GPOOL, 1)),
                rcnt=np.tile(recip_cnt.reshape(1, -1), (HID, 1)),
                patch1=patch1.astype(bf16), patch2=patch2,
                ident=np.eye(128, dtype=bf16))


def _pack_superblocks(DLO, DHI, budget=SB_BLOCK_BUDGET):
    sbs, cur, tot = [], [], 0
    for g in range(len(DLO)):
        d = int(DLO[g] + DHI[g])
        if cur and tot + d > budget:
            sbs.append(cur)
            cur, tot = [], 0
        cur.append(g)
        tot += d
    if cur:
        sbs.append(cur)
    return sbs


def make_sched(prep):
    DLO1, DHI1 = prep["DLO1"], prep["DHI1"]
    DLO2, DHI2 = prep["DLO2"], prep["DHI2"]
    assert all(a + b > 0 for a, b in zip(DLO1, DHI1))
    assert all(a + b > 0 for a, b in zip(DLO2, DHI2))
    return dict(DLO1=DLO1, DHI1=DHI1, DLO2=DLO2, DHI2=DHI2,
                SB1=_pack_superblocks(DLO1, DHI1),
                SB2=_pack_superblocks(DLO2, DHI2),
                HASB1=bool(np.any(prep["b1"])), HASB2=bool(np.any(prep["b2"])),
                HASBL=bool(np.any(prep["bl"])))


# ======================= bass kernel =======================================

def _emit_dummy_out(nc, tc, t_out, dt):
    with tc.tile_pool(name='dummy', bufs=1) as dp:
        d = dp.tile([GPOOL, OUT], dt.float32)
        nc.vector.memset(d[:], 0.0)
        nc.sync.dma_start(t_out[:, :], d[:])


def build_bass(sc):
    import concourse.bacc as bacc
    import concourse.tile as tile
    import concourse.mybir as mybir
    from concourse.library_config import mlp

    dt = mybir.dt
    Alu = mybir.AluOpType
    Act = mybir.ActivationFunctionType
    Axis = mybir.AxisListType

    DLO1, DHI1 = sc["DLO1"], sc["DHI1"]
    DLO2, DHI2 = sc["DLO2"], sc["DHI2"]
    SB1, SB2 = sc["SB1"], sc["SB2"]
    HASB1 = sc.get("HASB1", True)
    HASB2 = sc.get("HASB2", True)
    HASBL = sc.get("HASBL", True)
    NG1, NG2 = len(DLO1), len(DLO2)
    XT_COLS = _ceil_to(N + 2, XCHUNK)
    NCHUNK = XT_COLS // XCHUNK
    SH2_ROWS = _ceil_to(S + 2, 128)

    nc = bacc.Bacc("TRN2", target_bir_lowering=False, debug=False,
                   num_devices=NCORES, num_swdge_queues=4)

    t_xT = nc.dram_tensor("xT", [IN, XT_COLS], dt.bfloat16, kind="ExternalInput")
    t_w1 = nc.dram_tensor("w1ext", [IN, 136], dt.bfloat16, kind="ExternalInput")
    t_w2 = nc.dram_tensor("w2ext", [IN, 34], dt.bfloat16, kind="ExternalInput")
    t_wl = nc.dram_tensor("wl", [HID, OUT], dt.float32, kind="ExternalInput")
    t_b1 = nc.dram_tensor("b1", [128, HEADS * HID], dt.float32,
                          kind="ExternalInput")
    t_b2 = nc.dram_tensor("b2", [128, HID], dt.float32, kind="ExternalInput")
    t_bl = nc.dram_tensor("bl", [GPOOL, OUT], dt.float32, kind="ExternalInput")
    t_rcnt = nc.dram_tensor("rcnt", [HID, GPOOL], dt.float32,
                            kind="ExternalInput")
    t_patch1 = nc.dram_tensor("patch1", [2, 256], dt.bfloat16,
                              kind="ExternalInput")
    t_patch2 = nc.dram_tensor("patch2", [2, 64], dt.float32,
                              kind="ExternalInput")
    t_ident = nc.dram_tensor("ident", [128, 128], dt.bfloat16,
                             kind="ExternalInput")
    t_mpool = nc.dram_tensor("mpool", [NG2 * 128, GPOOL], dt.bfloat16,
                             kind="ExternalInput")
    n1lo = max(8 * sum(DLO1), 8)
    n1hi = max(8 * sum(DHI1), 8)
    n2lo = max(8 * sum(DLO2), 8)
    n2hi = max(8 * sum(DHI2), 8)
    t_i1lo = nc.dram_tensor("idx1lo", [128, n1lo], dt.int16, kind="ExternalInput")
    t_i1hi = nc.dram_tensor("idx1hi", [128, n1hi], dt.int16, kind="ExternalInput")
    t_i2lo = nc.dram_tensor("idx2lo", [128, n2lo], dt.int16, kind="ExternalInput")
    t_i2hi = nc.dram_tensor("idx2hi", [128, n2hi], dt.int16, kind="ExternalInput")
    t_ald1 = nc.dram_tensor("ald1", [128, 8 * NG1], dt.int16,
                            kind="ExternalInput")
    t_ald2 = nc.dram_tensor("ald2", [128, 8 * NG2], dt.int16,
                            kind="ExternalInput")
    t_scat1 = nc.dram_tensor("scat1", [128, 8 * NG1], dt.int16,
                             kind="ExternalInput")
    t_out = nc.dram_tensor("out", [GPOOL, OUT], dt.float32,
                           kind="ExternalOutput")

    rg = [list(range(NCORES))]
    _qc = [0]

    def nextq():
        _qc[0] = (_qc[0] + 1) % 4
        return _qc[0]

    with tile.TileContext(nc) as tc:
        with (
            tc.tile_pool(name="const", bufs=1) as constp,
            tc.tile_pool(name="dram", bufs=1, space="DRAM") as dramp,
        ):
            nc.gpsimd.load_library(mlp)

            table1 = dramp.tile([XT_COLS, 256], dt.bfloat16, tag="table1")
            table2 = dramp.tile([_ceil_to(N + 2, 4), 64], dt.float32,
                                tag="table2")
            h2sh = dramp.tile([SH2_ROWS, 64], dt.float32, tag="h2sh")
            cc_in = dramp.tile([HID, GPOOL], dt.float32, tag="ccin")
            cc_out = dramp.tile([HID, GPOOL], dt.float32, tag="ccout")

            w1_t = constp.tile([IN, 136], dt.bfloat16)
            nc.sync.dma_start(w1_t[:], t_w1[:])
            w2_t = constp.tile([IN, 34], dt.bfloat16)
            nc.sync.dma_start(w2_t[:], t_w2[:])
            wl_t = constp.tile([HID, OUT], dt.float32)
            nc.sync.dma_start(wl_t[:], t_wl[:])
            b1_t = constp.tile([128, HEADS * HID], dt.float32)
            nc.sync.dma_start(b1_t[:], t_b1[:])
            b2_t = constp.tile([128, HID], dt.float32)
            nc.sync.dma_start(b2_t[:], t_b2[:])
            bl_t = constp.tile([GPOOL, OUT], dt.float32)
            nc.sync.dma_start(bl_t[:], t_bl[:])
            rc_t = constp.tile([HID, GPOOL], dt.float32)
            nc.sync.dma_start(rc_t[:], t_rcnt[:])
            id_t = constp.tile([128, 128], dt.bfloat16)
            nc.sync.dma_start(id_t[:], t_ident[:])

            # zero the scatter_add target
            with tc.tile_pool(name="zp", bufs=1) as zp:
                z_t = zp.tile([128, SH2_ROWS // 128 * 64], dt.float32)
                nc.vector.memset(z_t[:], 0.0)
                nc.sync.dma_start(
                    h2sh[:, :].rearrange("(p k) e -> p (k e)", p=128), z_t[:])

            # ---------------- phase X: build table1 ----------------
            with (
                tc.tile_pool(name="xload", bufs=3) as xlp,
                tc.tile_pool(name="xout", bufs=3) as xop,
                tc.tile_pool(name="xpsum", bufs=4, space="PSUM") as xpp,
            ):
                for t in range(NCHUNK):
                    xt_t = xlp.tile([IN, XCHUNK], dt.bfloat16, tag="xt")
                    nc.sync.dma_start(xt_t[:],
                                      t_xT[:, t * XCHUNK:(t + 1) * XCHUNK])
                    o_t = xop.tile([128, 4, 256], dt.bfloat16, tag="xo")
                    nc.vector.memset(o_t[:, :, 136:256], 0.0)
                    for k in range(4):
                        ps = xpp.tile([128, 136], dt.float32, tag="xp")
                        nc.tensor.matmul(ps[:], xt_t[:, k * 128:(k + 1) * 128],
                                         w1_t[:], start=True, stop=True)
                        if k % 2 == 0:
                            nc.vector.tensor_copy(o_t[:, k, 0:136], ps[:])
                        else:
                            nc.scalar.activation(o_t[:, k, 0:136], ps[:],
                                                 Act.Copy)
                    nc.sync.dma_start(
                        table1[t * XCHUNK:(t + 1) * XCHUNK, :].rearrange(
                            "(k p) e -> p k e", p=128), o_t[:])
            with tc.tile_pool(name="patchp", bufs=1) as pp:
                p1_t = pp.tile([2, 256], dt.bfloat16)
                nc.sync.dma_start(p1_t[:], t_patch1[:])
                nc.sync.dma_start(table1[LO_MAX:LO_MAX + 1, :], p1_t[0:1, :])
                nc.sync.dma_start(table1[N + 1:N + 2, :], p1_t[1:2, :])
                p2_t = pp.tile([2, 64], dt.float32)
                nc.sync.dma_start(p2_t[:], t_patch2[:])
                nc.sync.dma_start(table2[0:1, :], p2_t[0:1, :])
                nc.sync.dma_start(table2[N + 1:N + 2, :], p2_t[1:2, :])

            if PHASES >= 2:
                # ---------------- phase L1: edges ----------------
                tab1_lo = table1[0:LO_MAX + 1, :]
                tab1_hi = table1[LO_MAX + 1:N + 2, :]
                Dmax1 = max(DLO1[g] + DHI1[g] for g in range(NG1))
                NBLO1 = max(sum(DLO1[g] for g in sb) for sb in SB1)
                NBHI1 = max(max(sum(DHI1[g] for g in sb) for sb in SB1), 1)
                NGSB1 = max(len(sb) for sb in SB1)
                olo = np.concatenate([[0], np.cumsum(DLO1)]).astype(int)
                ohi = np.concatenate([[0], np.cumsum(DHI1)]).astype(int)
                with (
                    tc.tile_pool(name="idx1", bufs=4) as idxp,
                    tc.tile_pool(name="gath1", bufs=3) as gathp,
                    tc.tile_pool(name="small1", bufs=3) as smallp,
                    tc.tile_pool(name="epi1", bufs=3) as epip,
                    tc.tile_pool(name="scatp", bufs=1) as scatp,
                    tc.tile_pool(name="agg1", bufs=2, space="PSUM") as aggp,
                    tc.tile_pool(name="psT1", bufs=2, space="PSUM") as psTp,
                    tc.tile_pool(name="ps21", bufs=2, space="PSUM") as ps2p,
                ):
                    scat_t = scatp.tile([128, NG1, 64], dt.float32, tag="sc")
                    nc.vector.memset(scat_t[:], 0.0)
                    elu_all = scatp.tile([128, NG1, 128], dt.bfloat16,
                                         tag="eluall")
                    for sb in SB1:
                        g0 = sb[0]
                        nlo = sum(DLO1[g] for g in sb)
                        nhi = sum(DHI1[g] for g in sb)
                        ng = len(sb)
                        ilo_t = idxp.tile([128, 8 * NBLO1], dt.int16,
                                          tag="ilo")
                        nc.sync.dma_start(
                            ilo_t[:, :8 * nlo],
                            t_i1lo[:, 8 * olo[g0]:8 * (olo[g0] + nlo)])
                        glo_t = gathp.tile([128, NBLO1, 256],
                                           dt.bfloat16, tag="glo")
                        nc.gpsimd.dma_gather(
                            glo_t[:, :nlo, :], tab1_lo,
                            ilo_t[:, :8 * nlo],
                            128 * nlo, 128 * nlo, 256,
                            single_packet=False, queue_num=nextq())
                        ald_t = gathp.tile([128, NGSB1, 256], dt.bfloat16,
                                           tag="ald")
                        nc.sync.dma_start(
                            ald_t[:, :ng, :],
                            table1[128 * g0:128 * (g0 + ng), :].rearrange(
                                "(b p) e -> p b e", p=128))
                        ghi_t = gathp.tile([128, NBHI1, 256], dt.bfloat16,
                                           tag="ghi")
                        if nhi > 0:
                            ihi_t = idxp.tile([128, 8 * NBHI1], dt.int16,
                                              tag="ihi")
                            nc.sync.dma_start(
                                ihi_t[:, :8 * nhi],
                                t_i1hi[:, 8 * ohi[g0]:8 * (ohi[g0] + nhi)])
                            nc.gpsimd.dma_gather(
                                ghi_t[:, :nhi, :], tab1_hi, ihi_t[:, :8 * nhi],
                                128 * nhi, 128 * nhi, 256, single_packet=False,
                                queue_num=nextq())
                        lo_off = 0
                        hi_off = 0
                        for gi, g in enumerate(sb):
                            dlo, dhi = DLO1[g], DHI1[g]
                            D = dlo + dhi
                            if L1STEP < 2:
                                lo_off += dlo
                                hi_off += dhi
                                continue
                            logit_t = smallp.tile([128, Dmax1, 4], dt.float32,
                                                  tag="lg")
                            exb_t = smallp.tile([128, Dmax1, 4], dt.bfloat16,
                                                tag="exb")
                            den_t = smallp.tile([128, 4], dt.float32, tag="dn")
                            rec_t = smallp.tile([128, 4], dt.float32, tag="rc")
                            ald_ap = ald_t[:, gi, 132:136]
                            if dlo > 0:
                                nc.vector.scalar_tensor_tensor(
                                    logit_t[:, :dlo, :],
                                    glo_t[:, lo_off:lo_off + dlo, 128:132], 0.0,
                                    ald_ap.unsqueeze(1).broadcast_to(
                                        (128, dlo, 4)), Alu.add, Alu.add)
                            if dhi > 0:
                                nc.vector.scalar_tensor_tensor(
                                    logit_t[:, dlo:D, :],
                                    ghi_t[:, hi_off:hi_off + dhi, 128:132], 0.0,
                                    ald_ap.unsqueeze(1).broadcast_to(
                                        (128, dhi, 4)), Alu.add, Alu.add)
                            nc.vector.scalar_tensor_tensor(
                                logit_t[:, :D, :], logit_t[:, :D, :], NEG,
                                logit_t[:, :D, :], Alu.mult, Alu.max)
                            nc.scalar.activation(exb_t[:, :D, :],
                                                 logit_t[:, :D, :], Act.Exp)
                            nc.vector.tensor_reduce(
                                den_t[:], exb_t[:, :D, :].transpose([0, 2, 1]),
                                axis=Axis.X, op=Alu.add)
                            nc.vector.reciprocal(rec_t[:], den_t[:])
                            if L1STEP < 3:
                                lo_off += dlo
                                hi_off += dhi
                                continue
                            if dlo > 0:
                                h_lo = glo_t[:, lo_off:lo_off + dlo, 0:128]
                                h_lo = h_lo.rearrange("p b (h c) -> p b h c",
                                                      h=4)
                                nc.vector.tensor_tensor(
                                    h_lo, h_lo,
                                    exb_t[:, :dlo, :].unsqueeze(3).broadcast_to(
                                        (128, dlo, 4, HID)), Alu.mult)
                            if dhi > 0:
                                h_hi = ghi_t[:, hi_off:hi_off + dhi, 0:128]
                                h_hi = h_hi.rearrange("p b (h c) -> p b h c",
                                                      h=4)
                                nc.vector.tensor_tensor(
                                    h_hi, h_hi,
                                    exb_t[:, dlo:D, :].unsqueeze(3).broadcast_to(
                                        (128, dhi, 4, HID)), Alu.mult)
                            if L1STEP < 4:
                                lo_off += dlo
                                hi_off += dhi
                                continue
                            agg = aggp.tile([128, 128], dt.float32, tag="agg")
                            rhss = ([glo_t[:, lo_off + b, 0:128]
                                     for b in range(dlo)]
                                    + [ghi_t[:, hi_off + b, 0:128]
                                       for b in range(dhi)])
                            for bi, rhs in enumerate(rhss):
                                nc.tensor.matmul(agg[:], id_t[:], rhs,
                                                 start=(bi == 0),
                                                 stop=(bi == len(rhss) - 1))
                            if L1STEP < 5:
                                lo_off += dlo
                                hi_off += dhi
                                continue
                            scaled_t = epip.tile([128, 128], dt.float32,
                                                 tag="sd")
                            nc.vector.tensor_tensor(
                                scaled_t[:].rearrange("p (h c) -> p h c", h=4),
                                agg[:].rearrange("p (h c) -> p h c", h=4),
                                rec_t[:].unsqueeze(2).broadcast_to(
                                    (128, 4, HID)), Alu.mult)
                            if HASB1:
                                nc.vector.tensor_tensor(
                                    scaled_t[:], scaled_t[:], b1_t[:], Alu.add)
                            tmp_t = epip.tile([128, 128], dt.float32, tag="tm")
                            nc.scalar.activation(tmp_t[:], scaled_t[:], Act.Relu,
                                                 scale=-1.0)
                            nc.scalar.activation(tmp_t[:], tmp_t[:], Act.Exp,
                                                 scale=-1.0)
                            nc.vector.scalar_tensor_tensor(
                                elu_all[:, g, :], tmp_t[:], -1.0, scaled_t[:],
                                Alu.add, Alu.max)
                            lo_off += dlo
                            hi_off += dhi
                    # ---- pass 2: transpose + W2ext per group, then one scatter
                    if L1STEP >= 5:
                        for g in range(NG1):
                            psT = psTp.tile([128, 128], dt.bfloat16, tag="pt")
                            nc.tensor.transpose(psT[:], elu_all[:, g, :],
                                                id_t[:])
                            eluT_t = epip.tile([128, 128], dt.bfloat16,
                                               tag="et")
                            nc.scalar.activation(eluT_t[:], psT[:], Act.Copy)
                            ps2 = ps2p.tile([128, 34], dt.float32, tag="p2")
                            nc.tensor.matmul(ps2[:], eluT_t[:], w2_t[:],
                                             start=True, stop=True)
                            if g % 2 == 0:
                                nc.scalar.activation(scat_t[:, g, 0:34],
                                                     ps2[:], Act.Copy)
                            else:
                                nc.vector.tensor_copy(scat_t[:, g, 0:34],
                                                      ps2[:])
                    if L1STEP >= 6:
                        si_t = idxp.tile([128, 8 * NG1], dt.int16, tag="si")
                        nc.sync.dma_start(si_t[:], t_scat1[:])
                        nc.gpsimd.dma_scatter_add(
                            h2sh[0:S + 1, :], scat_t[:], si_t[:],
                            128 * NG1, 128 * NG1, 64,
                            single_packet=False, queue_num=nextq())
            if PHASES >= 3:
                # ---------------- exchange ----------------
                nc.gpsimd.collective_compute(
                    "AllGather", mybir.AluOpType.bypass, replica_groups=rg,
                    ins=[h2sh[0:S, :]], outs=[table2[1:N + 1, :]])

            if PHASES >= 4:
                # ---------------- phase L2: edges + pool ----------------
                tab2_lo = table2[0:LO_MAX + 1, :]
                tab2_hi = table2[LO_MAX + 1:N + 2, :]
                Dmax2 = max(DLO2[g] + DHI2[g] for g in range(NG2))
                NBLO2 = max(sum(DLO2[g] for g in sb) for sb in SB2)
                NBHI2 = max(max(sum(DHI2[g] for g in sb) for sb in SB2), 1)
                NGSB2 = max(len(sb) for sb in SB2)
                olo2 = np.concatenate([[0], np.cumsum(DLO2)]).astype(int)
                ohi2 = np.concatenate([[0], np.cumsum(DHI2)]).astype(int)
                with (
                    tc.tile_pool(name="idx2", bufs=4) as idxp,
                    tc.tile_pool(name="gath2", bufs=3) as gathp,
                    tc.tile_pool(name="small2", bufs=3) as smallp,
                    tc.tile_pool(name="epi2", bufs=3) as epip,
                    tc.tile_pool(name="agg2", bufs=2, space="PSUM") as aggp,
                    tc.tile_pool(name="poolps", bufs=1, space="PSUM") as poolpp,
                    tc.tile_pool(name="mp2", bufs=3) as mpp,
                ):
                    poolps = poolpp.tile([HID, GPOOL], dt.float32)
                    h2p_all = mpp.tile([128, NG2, HID], dt.bfloat16,
                                       tag="h2pall", bufs=1)
                    for sb in SB2:
                        g0 = sb[0]
                        nlo = sum(DLO2[g] for g in sb)
                        nhi = sum(DHI2[g] for g in sb)
                        ng = len(sb)
                        ilo_t = idxp.tile([128, 8 * NBLO2], dt.int16, tag="ilo")
                        nc.sync.dma_start(
                            ilo_t[:, :8 * nlo],
                            t_i2lo[:, 8 * olo2[g0]:8 * (olo2[g0] + nlo)])
                        glo_t = gathp.tile([128, NBLO2, 64], dt.float32, tag="glo")
                        if nlo > 0:
                            nc.gpsimd.dma_gather(
                                glo_t[:, :nlo, :], tab2_lo, ilo_t[:, :8 * nlo],
                                128 * nlo, 128 * nlo, 64, single_packet=False,
                                queue_num=nextq())
                        ald_t = gathp.tile([128, NGSB2, 64], dt.float32, tag="ald")
                        nc.sync.dma_start(
                            ald_t[:, :ng, :],
                            h2sh[128 * g0:128 * (g0 + ng), :].rearrange(
                                "(b p) e -> p b e", p=128))
                        ghi_t = gathp.tile([128, NBHI2, 64], dt.float32, tag="ghi")
                        if nhi > 0:
                            ihi_t = idxp.tile([128, 8 * NBHI2], dt.int16, tag="ihi")
                            nc.sync.dma_start(
                                ihi_t[:, :8 * nhi],
                                t_i2hi[:, 8 * ohi2[g0]:8 * (ohi2[g0] + nhi)])
                            nc.gpsimd.dma_gather(
                                ghi_t[:, :nhi, :], tab2_hi, ihi_t[:, :8 * nhi],
                                128 * nhi, 128 * nhi, 64, single_packet=False,
                                queue_num=nextq())
                        lo_off = 0
                        hi_off = 0
                        for gi, g in enumerate(sb):
                            dlo, dhi = DLO2[g], DHI2[g]
                            D = dlo + dhi
                            logit_t = smallp.tile([128, Dmax2, 1], dt.float32,
                                                  tag="lg")
                            exf_t = smallp.tile([128, Dmax2, 1], dt.float32,
                                                tag="exf")
                            den_t = smallp.tile([128, 1], dt.float32, tag="dn")
                            rec_t = smallp.tile([128, 1], dt.float32, tag="rc")
                            ald_ap = ald_t[:, gi, 33:34]
                            if dlo > 0:
                                nc.vector.tensor_scalar(
                                    logit_t[:, :dlo, :],
                                    glo_t[:, lo_off:lo_off + dlo, 32:33],
                                    ald_ap, None, Alu.add)
                            if dhi > 0:
                                nc.vector.tensor_scalar(
                                    logit_t[:, dlo:D, :],
                                    ghi_t[:, hi_off:hi_off + dhi, 32:33],
                                    ald_ap, None, Alu.add)
                            nc.vector.scalar_tensor_tensor(
                                logit_t[:, :D, :], logit_t[:, :D, :], NEG,
                                logit_t[:, :D, :], Alu.mult, Alu.max)
                            nc.scalar.activation(exf_t[:, :D, :], logit_t[:, :D, :],
                                                 Act.Exp)
                            nc.vector.tensor_reduce(
                                den_t[:], exf_t[:, :D, :].transpose([0, 2, 1]),
                                axis=Axis.X, op=Alu.add)
                            nc.vector.reciprocal(rec_t[:], den_t[:])
                            exh_t = smallp.tile([128, Dmax2, HID], dt.bfloat16,
                                                tag="exh")
                            if dlo > 0:
                                nc.vector.tensor_tensor(
                                    exh_t[:, :dlo, :],
                                    glo_t[:, lo_off:lo_off + dlo, 0:HID],
                                    exf_t[:, :dlo, :].broadcast_to(
                                        (128, dlo, HID)), Alu.mult)
                            if dhi > 0:
                                nc.vector.tensor_tensor(
                                    exh_t[:, dlo:D, :],
                                    ghi_t[:, hi_off:hi_off + dhi, 0:HID],
                                    exf_t[:, dlo:D, :].broadcast_to(
                                        (128, dhi, HID)), Alu.mult)
                            agg = aggp.tile([128, HID], dt.float32, tag="agg")
                            for bi in range(D):
                                nc.tensor.matmul(agg[:], id_t[:], exh_t[:, bi, :],
                                                 start=(bi == 0),
                                                 stop=(bi == D - 1))
                            scaled_t = epip.tile([128, HID], dt.float32, tag="sd")
                            nc.vector.tensor_scalar(scaled_t[:], agg[:], rec_t[:],
                                                    None, Alu.mult)
                            if HASB2:
                                nc.vector.tensor_tensor(
                                    scaled_t[:], scaled_t[:], b2_t[:], Alu.add)
                            tmp_t = epip.tile([128, HID], dt.float32, tag="tm")
                            nc.scalar.activation(tmp_t[:], scaled_t[:], Act.Relu,
                                                 scale=-1.0)
                            nc.scalar.activation(tmp_t[:], tmp_t[:], Act.Exp,
                                                 scale=-1.0)
                            nc.vector.scalar_tensor_tensor(
                                h2p_all[:, g, :], tmp_t[:], -1.0, scaled_t[:],
                                Alu.add, Alu.max)
                            lo_off += dlo
                            hi_off += dhi

                    for g in range(NG2):
                        mp_t = mpp.tile([128, GPOOL], dt.bfloat16, tag="mp")
                        nc.sync.dma_start(mp_t[:],
                                          t_mpool[g * 128:(g + 1) * 128, :])
                        nc.tensor.matmul(poolps[:], h2p_all[:, g, :], mp_t[:],
                                         start=(g == 0), stop=(g == NG2 - 1))
                    # ------------- pool + final linear -------------
                    with tc.tile_pool(name="fin", bufs=1) as finp, \
                            tc.tile_pool(name="finps", bufs=1, space="PSUM") as fpp:
                        poolsb = finp.tile([HID, GPOOL], dt.float32)
                        nc.vector.tensor_copy(poolsb[:], poolps[:])
                        nc.sync.dma_start(cc_in[:, :], poolsb[:])
                        nc.gpsimd.collective_compute(
                            "AllReduce", Alu.add, replica_groups=rg,
                            ins=[cc_in[:, :]], outs=[cc_out[:, :]])
                        psum_t = finp.tile([HID, GPOOL], dt.float32)
                        nc.sync.dma_start(psum_t[:], cc_out[:, :])
                        mean_t = finp.tile([HID, GPOOL], dt.float32)
                        nc.vector.tensor_tensor(
                            mean_t[:], psum_t[:],
                            rc_t[:], Alu.mult)
                        psO = fpp.tile([GPOOL, OUT], dt.float32)
                        nc.tensor.matmul(psO[:], mean_t[:], wl_t[:], start=True,
                                         stop=True)
                        out_t = finp.tile([GPOOL, OUT], dt.float32)
                        if HASBL:
                            nc.vector.tensor_tensor(out_t[:], psO[:], bl_t[:],
                                                    Alu.add)
                        else:
                            nc.vector.tensor_copy(out_t[:], psO[:])
                        nc.sync.dma_start(t_out[:, :], out_t[:])
            if PHASES < 4:
                _emit_dummy_out(nc, tc, t_out, dt)


    nc.compile()
    return nc


def core_inputs(prep, c):
    cd = prep["cores"][c]

    def padcols(a, cols):
        if a.shape[1] == cols:
            return a
        out = np.zeros((a.shape[0], cols), a.dtype)
        out[:, :a.shape[1]] = a
        return out

    n1lo = max(8 * sum(prep["DLO1"]), 8)
    n1hi = max(8 * sum(prep["DHI1"]), 8)
    n2lo = max(8 * sum(prep["DLO2"]), 8)
    n2hi = max(8 * sum(prep["DHI2"]), 8)
    return dict(
        xT=np.ascontiguousarray(cd["xT"]),
        w1ext=prep["W1ext"], w2ext=prep["W2ext"], wl=prep["Wl"],
        b1=prep["b1"], b2=prep["b2"], bl=prep["bl"], rcnt=prep["rcnt"],
        patch1=prep["patch1"], patch2=prep["patch2"], ident=prep["ident"],
        mpool=np.ascontiguousarray(cd["mpool"]),
        idx1lo=padcols(cd["w_idx1lo"], n1lo),
        idx1hi=padcols(cd["w_idx1hi"], n1hi),
        idx2lo=padcols(cd["w_idx2lo"], n2lo),
        idx2hi=padcols(cd["w_idx2hi"], n2hi),
        ald1=cd["w_ald1"], ald2=cd["w_ald2"], scat1=cd["w_scat1"],
    )


_CACHE = {}


def kernel(**inputs):
    from concourse.bass_utils import run_bass_kernel_spmd

    inputs = {k: np.asarray(v) for k, v in inputs.items()}
    prep = host_prep(**inputs)
    sc = make_sched(prep)
    key = str(sc)
    if key not in _CACHE:
        _CACHE[key] = build_bass(sc)
    nc = _CACHE[key]
    in_maps = [core_inputs(prep, c) for c in range(NCORES)]
    res = run_bass_kernel_spmd(nc, in_maps, list(range(NCORES)))
    return np.asarray(res.results[0]["out"], np.float32)



# revision 8
# speedup vs baseline: 1.0203x; 1.0203x over previous
# Self-contained 8-core Trainium2 Bass kernel for the 2-layer GAT + mean-pool
# problem (nn_GAT_83820581749190).
#
# Sharding: destination nodes (and all their incident edges) are partitioned
# across the 8 cores, so each layer's attention softmax and aggregation
# complete locally per core. Each core builds a replicated layer-1 feature
# table [h1 | al_src] (bf16, 512-byte rows) in HBM with a replicated
# x @ W1ext matmul, edge-gathers rows with the GPSIMD dma_gather custom op
# (int16 indices force a lo/hi table-half split), computes the edge softmax
# without segment-max (logits are small enough that exp cannot overflow), and
# aggregates per-destination with identity-matmul PSUM accumulation
# (destinations sit on partitions via degree-bucketed groups of 128).
# Layer-2 features are exchanged with an AllGather; mean-pool is a matmul
# against a host-built one-hot graph matrix plus a tiny AllReduce.
import numpy as np
import ml_dtypes

N = 50000
E = 800000
IN = 128
HID = 32
HEADS = 4
OUT = 10
GPOOL = 64
NEG = 0.2
NCORES = 8
S = N // NCORES
LO_MAX = 32767          # max usable int16 gather index
SPECIAL_ALS = -100.0    # al_src of pad rows: exp(0.2*(-100+ald)) ~ 2e-9
SB_BLOCK_BUDGET = 24    # max gather blocks per superblock
XCHUNK = 512
PHASES = 99  # debug: 1=X only, 2=+L1, 3=+exchange, 4=+L2, 5=full
L1STEP = 99  # debug: 1=gathers 2=+softmax 3=+exh 4=+agg 5=+epilogue 6=+scatter

bf16 = ml_dtypes.bfloat16


def _ceil_to(v, m):
    return (v + m - 1) // m * m


# ======================= host prep =========================================

def _boundary_aware_order(deg_lo, deg_hi):
    """Sort ids by (lo desc, hi desc), but fill 128-groups that straddle a
    lo-run boundary from the *small-hi tail* of the next run, keeping
    per-group max_lo + max_hi tight."""
    Sn = len(deg_lo)
    base = np.lexsort((-deg_hi, -deg_lo))
    glo = deg_lo[base]
    runs = []
    i = 0
    while i < Sn:
        j = i
        while j < Sn and glo[j] == glo[i]:
            j += 1
        runs.append(list(base[i:j]))
        i = j
    order = []
    ri = 0
    fronts = [0] * len(runs)
    backs = [len(r) for r in runs]
    while len(order) < Sn:
        while ri < len(runs) and fronts[ri] >= backs[ri]:
            ri += 1
        if ri >= len(runs):
            break
        need = 128 - (len(order) % 128)
        avail = backs[ri] - fronts[ri]
        if avail >= need:
            order.extend(runs[ri][fronts[ri]:fronts[ri] + need])
            fronts[ri] += need
        else:
            order.extend(runs[ri][fronts[ri]:backs[ri]])
            fronts[ri] = backs[ri]
            need -= avail
            rj = ri + 1
            while need > 0 and rj < len(runs):
                a = backs[rj] - fronts[rj]
                t = min(a, need)
                order.extend(reversed(runs[rj][backs[rj] - t:backs[rj]]))
                backs[rj] -= t
                need -= t
                rj += 1
    P = np.asarray(order, np.int64)
    Ppos = np.empty(Sn, np.int64)
    Ppos[P] = np.arange(Sn)
    return P, Ppos


def _run_groups(glo, ghi, max_rows=128):
    Sn = len(glo)
    ng = (Sn + max_rows - 1) // max_rows
    dlo = np.zeros(ng, np.int64)
    dhi = np.zeros(ng, np.int64)
    for g in range(ng):
        s, e = g * max_rows, min((g + 1) * max_rows, Sn)
        dlo[g] = glo[s:e].max()
        dhi[g] = ghi[s:e].max()
    return dlo, dhi


def _build_layer(src, dstl, is_lo):
    deg_lo = np.bincount(dstl[is_lo], minlength=S)
    deg_hi = np.bincount(dstl[~is_lo], minlength=S)
    P, Ppos = _boundary_aware_order(deg_lo, deg_hi)
    dlo, dhi = _run_groups(deg_lo[P], deg_hi[P])
    return dict(src=src, dstl=dstl, is_lo=is_lo, deg_lo=deg_lo, deg_hi=deg_hi,
                P=P, Ppos=Ppos, dlo=dlo, dhi=dhi)


def _emit_slots(l, DLO, DHI, idx_lo_of, idx_hi_of, special_lo, special_hi):
    NG = len(DLO)
    src, is_lo = l["src"], l["is_lo"]
    Ppos = l["Ppos"]
    nreal = len(l["P"])
    slot2cmp = np.full(NG * 128, -1, np.int64)
    slot2cmp[:nreal] = np.arange(nreal)
    idx_lo = [np.full((int(DLO[g]), 128), special_lo, np.int64)
              for g in range(NG)]
    idx_hi = [np.full((int(DHI[g]), 128), special_hi, np.int64)
              for g in range(NG)]
    slot_of_edge = Ppos[l["dstl"]]
    order = np.argsort(slot_of_edge, kind="stable")
    for mask, arrs, idx_fn in ((is_lo, idx_lo, idx_lo_of),
                               (~is_lo, idx_hi, idx_hi_of)):
        m = mask[order]
        so = slot_of_edge[order][m]
        sr = src[order][m]
        jj = np.arange(len(so)) - np.searchsorted(so, so, side="left")
        gg, kk = so // 128, so % 128
        vals = idx_fn(sr)
        for g in range(NG):
            sel = gg == g
            if sel.any():
                arrs[g][jj[sel], kk[sel]] = vals[sel]
    return idx_lo, idx_hi, slot2cmp


def _wrap16(idx):
    """[n] -> [128, n//16] int16: idx i at [i%16, i//16], replicated x8."""
    n = len(idx)
    assert n % 16 == 0
    w = np.ascontiguousarray(np.asarray(idx).reshape(n // 16, 16).T)
    w = w.astype(np.int16)
    return np.tile(w, (8, 1))


def _wrap_groups(arrs):
    segs = [_wrap16(a.reshape(-1)) if a.size else np.zeros((128, 0), np.int16)
            for a in arrs]
    return np.concatenate(segs, axis=1) if segs else np.zeros((128, 0), np.int16)


def host_prep(x, edge_index, batch, W1, a1_src, a1_dst, b1, W2, a2_src, a2_dst,
              b2, Wl, bl):
    x = np.asarray(x, np.float32)
    edge_index = np.asarray(edge_index, np.int64)
    batch = np.asarray(batch, np.int64)
    src_all = np.concatenate([edge_index[0], np.arange(N, dtype=np.int64)])
    dst_all = np.concatenate([edge_index[1], np.arange(N, dtype=np.int64)])
    owner = dst_all // S

    a1_src = np.asarray(a1_src, np.float32)
    a1_dst = np.asarray(a1_dst, np.float32)
    W1 = np.asarray(W1, np.float32)
    W2 = np.asarray(W2, np.float32)
    As1 = np.zeros((HEADS * HID, HEADS), np.float32)
    Ad1 = np.zeros((HEADS * HID, HEADS), np.float32)
    for h in range(HEADS):
        As1[h * HID:(h + 1) * HID, h] = a1_src[h]
        Ad1[h * HID:(h + 1) * HID, h] = a1_dst[h]
    W1ext = np.concatenate([W1, W1 @ As1, W1 @ Ad1], axis=1)   # [128,136]
    W2ext = np.concatenate(
        [W2, W2 @ np.asarray(a2_src, np.float32)[0][:, None],
         W2 @ np.asarray(a2_dst, np.float32)[0][:, None]], axis=1)  # [128,34]

    cores = [dict(c=c) for c in range(NCORES)]
    for cd in cores:
        c = cd["c"]
        m = owner == c
        cd["src"] = src_all[m]
        cd["dstl"] = dst_all[m] - c * S

    # ---------- layer 1 ----------
    for cd in cores:
        c = cd["c"]
        pos_of = np.empty(N, np.int64)
        own = np.arange(c * S, (c + 1) * S)
        oth = np.concatenate([np.arange(0, c * S), np.arange((c + 1) * S, N)])
        pos_of[oth] = S + np.arange(N - S)
        pos_of[own] = 0
        is_lo1 = pos_of[cd["src"]] < LO_MAX
        l1 = _build_layer(cd["src"], cd["dstl"], is_lo1)
        pos_of[own] = l1["Ppos"]
        row_of = np.where(pos_of < LO_MAX, pos_of, pos_of + 1)
        cd["l1"] = l1
        cd["row_of"] = row_of
    NG1 = max(len(cd["l1"]["dlo"]) for cd in cores)
    DLO1 = np.zeros(NG1, np.int64)
    DHI1 = np.zeros(NG1, np.int64)
    for cd in cores:
        d = cd["l1"]
        DLO1[:len(d["dlo"])] = np.maximum(DLO1[:len(d["dlo"])], d["dlo"])
        DHI1[:len(d["dhi"])] = np.maximum(DHI1[:len(d["dhi"])], d["dhi"])
    for cd in cores:
        r = cd["row_of"]
        cd["idx1_lo"], cd["idx1_hi"], cd["slot2cmp1"] = _emit_slots(
            cd["l1"], DLO1, DHI1,
            lambda s, r=r: r[s], lambda s, r=r: r[s] - (LO_MAX + 1),
            LO_MAX, N + 1 - (LO_MAX + 1))

    # ---------- layer 2 ----------
    # core-5's own positions straddle LO_MAX; freeze its lo membership first
    cd5 = cores[5]
    alo = cd5["src"] // S <= 5
    da = np.bincount(cd5["dstl"][alo], minlength=S)
    db = np.bincount(cd5["dstl"][~alo], minlength=S)
    P5a, _ = _boundary_aware_order(da, db)
    n_lo5 = LO_MAX - 5 * S
    lo5_set = np.zeros(S, bool)
    if n_lo5 > 0:
        lo5_set[P5a[:n_lo5]] = True

    def lo2_mask_of(src):
        ow = src // S
        lo = (ow <= 4).copy()
        m5 = ow == 5
        lo[m5] = lo5_set[src[m5] - 5 * S]
        return lo

    for cd in cores:
        cd["l2"] = _build_layer(cd["src"], cd["dstl"], lo2_mask_of(cd["src"]))
    l25 = cores[5]["l2"]
    idsA = np.where(lo5_set)[0]
    idsB = np.where(~lo5_set)[0]
    PA, _ = _boundary_aware_order(l25["deg_lo"][idsA], l25["deg_hi"][idsA])
    PB, _ = _boundary_aware_order(l25["deg_lo"][idsB], l25["deg_hi"][idsB])
    P5 = np.concatenate([idsA[PA], idsB[PB]])
    P5pos = np.empty(S, np.int64)
    P5pos[P5] = np.arange(S)
    l25["P"] = P5
    l25["Ppos"] = P5pos
    l25["dlo"], l25["dhi"] = _run_groups(l25["deg_lo"][P5], l25["deg_hi"][P5])

    pos2_of = np.empty(N, np.int64)
    for cd in cores:
        c = cd["c"]
        pos2_of[c * S:(c + 1) * S] = c * S + cd["l2"]["Ppos"]
    row2_of = pos2_of + 1
    NG2 = max(len(cd["l2"]["dlo"]) for cd in cores)
    DLO2 = np.zeros(NG2, np.int64)
    DHI2 = np.zeros(NG2, np.int64)
    for cd in cores:
        d = cd["l2"]
        DLO2[:len(d["dlo"])] = np.maximum(DLO2[:len(d["dlo"])], d["dlo"])
        DHI2[:len(d["dhi"])] = np.maximum(DHI2[:len(d["dhi"])], d["dhi"])
    for cd in cores:
        l2 = cd["l2"]
        assert (row2_of[l2["src"][l2["is_lo"]]] <= LO_MAX).all()
        assert (row2_of[l2["src"][~l2["is_lo"]]] > LO_MAX).all()
        cd["idx2_lo"], cd["idx2_hi"], cd["slot2cmp2"] = _emit_slots(
            l2, DLO2, DHI2,
            lambda s: row2_of[s], lambda s: row2_of[s] - (LO_MAX + 1),
            0, N + 1 - (LO_MAX + 1))

    # ---------- aux ----------
    cnt = np.bincount(batch, minlength=GPOOL).astype(np.float32)
    recip_cnt = (1.0 / np.maximum(cnt, 1.0)).astype(np.float32)

    for cd in cores:
        c = cd["c"]
        gids = batch[c * S:(c + 1) * S]
        Mp = np.zeros((NG2 * 128, GPOOL), np.float32)
        s2c = cd["slot2cmp2"]
        real = s2c >= 0
        Mp[np.where(real)[0], gids[cd["l2"]["P"][s2c[real]]]] = 1.0
        cd["mpool"] = Mp.astype(bf16)

        s2c1 = cd["slot2cmp1"]
        tgt = np.full(len(s2c1), S, np.int64)  # trash row for dummy slots
        r1 = s2c1 >= 0
        tgt[r1] = cd["l2"]["Ppos"][cd["l1"]["P"][s2c1[r1]]]
        cd["aldidx1"] = np.where(s2c1 >= 0, s2c1, 0)
        cd["aldidx2"] = np.where(cd["slot2cmp2"] >= 0, cd["slot2cmp2"], 0)

        xt = np.zeros((IN, _ceil_to(N + 2, XCHUNK)), np.float32)
        xt[:, cd["row_of"]] = x.T
        cd["xT"] = xt.astype(bf16)

        cd["w_idx1lo"] = _wrap_groups(cd["idx1_lo"])
        cd["w_idx1hi"] = _wrap_groups(cd["idx1_hi"])
        cd["w_idx2lo"] = _wrap_groups(cd["idx2_lo"])
        cd["w_idx2hi"] = _wrap_groups(cd["idx2_hi"])
        cd["w_ald1"] = _wrap16(cd["aldidx1"])
        cd["w_ald2"] = _wrap16(cd["aldidx2"])
        cd["w_scat1"] = _wrap16(tgt)

    patch1 = np.zeros((2, 256), np.float32)
    patch1[:, 128:132] = SPECIAL_ALS
    patch2 = np.zeros((2, 64), np.float32)
    patch2[:, 32] = SPECIAL_ALS

    return dict(cores=cores,
                DLO1=[int(v) for v in DLO1], DHI1=[int(v) for v in DHI1],
                DLO2=[int(v) for v in DLO2], DHI2=[int(v) for v in DHI2],
                W1ext=W1ext.astype(bf16), W2ext=W2ext.astype(bf16),
                Wl=np.asarray(Wl, np.float32),
                b1=np.tile(np.asarray(b1, np.float32).reshape(1, -1),
                           (128, 1)),
                b2=np.tile(np.asarray(b2, np.float32).reshape(1, -1),
                           (128, 1)),
                bl=np.tile(np.asarray(bl, np.float32).reshape(1, -1),
                           (GPOOL, 1)),
                rcnt=np.tile(recip_cnt.reshape(1, -1), (HID, 1)),
                patch1=patch1.astype(bf16), patch2=patch2,
                ident=np.eye(128, dtype=bf16))


def _pack_superblocks(DLO, DHI, budget=SB_BLOCK_BUDGET):
    sbs, cur, tot = [], [], 0
    for g in range(len(DLO)):
        d = int(DLO[g] + DHI[g])
        if cur and tot + d > budget:
            sbs.append(cur)
            cur, tot = [], 0
        cur.append(g)
        tot += d
    if cur:
        sbs.append(cur)
    return sbs


def make_sched(prep):
    DLO1, DHI1 = prep["DLO1"], prep["DHI1"]
    DLO2, DHI2 = prep["DLO2"], prep["DHI2"]
    assert all(a + b > 0 for a, b in zip(DLO1, DHI1))
    assert all(a + b > 0 for a, b in zip(DLO2, DHI2))
    return dict(DLO1=DLO1, DHI1=DHI1, DLO2=DLO2, DHI2=DHI2,
                SB1=_pack_superblocks(DLO1, DHI1),
                SB2=_pack_superblocks(DLO2, DHI2),
                HASB1=bool(np.any(prep["b1"])), HASB2=bool(np.any(prep["b2"])),
                HASBL=bool(np.any(prep["bl"])))


# ======================= bass kernel =======================================

def _emit_dummy_out(nc, tc, t_out, dt):
    with tc.tile_pool(name='dummy', bufs=1) as dp:
        d = dp.tile([GPOOL, OUT], dt.float32)
        nc.vector.memset(d[:], 0.0)
        nc.sync.dma_start(t_out[:, :], d[:])


def build_bass(sc):
    import concourse.bacc as bacc
    import concourse.tile as tile
    import concourse.mybir as mybir
    from concourse.library_config import mlp

    dt = mybir.dt
    Alu = mybir.AluOpType
    Act = mybir.ActivationFunctionType
    Axis = mybir.AxisListType

    DLO1, DHI1 = sc["DLO1"], sc["DHI1"]
    DLO2, DHI2 = sc["DLO2"], sc["DHI2"]
    SB1, SB2 = sc["SB1"], sc["SB2"]
    HASB1 = sc.get("HASB1", True)
    HASB2 = sc.get("HASB2", True)
    HASBL = sc.get("HASBL", True)
    NG1, NG2 = len(DLO1), len(DLO2)
    XT_COLS = _ceil_to(N + 2, XCHUNK)
    NCHUNK = XT_COLS // XCHUNK
    SH2_ROWS = _ceil_to(S + 2, 128)

    nc = bacc.Bacc("TRN2", target_bir_lowering=False, debug=False,
                   num_devices=NCORES, num_swdge_queues=4)

    t_xT = nc.dram_tensor("xT", [IN, XT_COLS], dt.bfloat16, kind="ExternalInput")
    t_w1 = nc.dram_tensor("w1ext", [IN, 136], dt.bfloat16, kind="ExternalInput")
    t_w2 = nc.dram_tensor("w2ext", [IN, 34], dt.bfloat16, kind="ExternalInput")
    t_wl = nc.dram_tensor("wl", [HID, OUT], dt.float32, kind="ExternalInput")
    t_b1 = nc.dram_tensor("b1", [128, HEADS * HID], dt.float32,
                          kind="ExternalInput")
    t_b2 = nc.dram_tensor("b2", [128, HID], dt.float32, kind="ExternalInput")
    t_bl = nc.dram_tensor("bl", [GPOOL, OUT], dt.float32, kind="ExternalInput")
    t_rcnt = nc.dram_tensor("rcnt", [HID, GPOOL], dt.float32,
                            kind="ExternalInput")
    t_patch1 = nc.dram_tensor("patch1", [2, 256], dt.bfloat16,
                              kind="ExternalInput")
    t_patch2 = nc.dram_tensor("patch2", [2, 64], dt.float32,
                              kind="ExternalInput")
    t_ident = nc.dram_tensor("ident", [128, 128], dt.bfloat16,
                             kind="ExternalInput")
    t_mpool = nc.dram_tensor("mpool", [NG2 * 128, GPOOL], dt.bfloat16,
                             kind="ExternalInput")
    n1lo = max(8 * sum(DLO1), 8)
    n1hi = max(8 * sum(DHI1), 8)
    n2lo = max(8 * sum(DLO2), 8)
    n2hi = max(8 * sum(DHI2), 8)
    t_i1lo = nc.dram_tensor("idx1lo", [128, n1lo], dt.int16, kind="ExternalInput")
    t_i1hi = nc.dram_tensor("idx1hi", [128, n1hi], dt.int16, kind="ExternalInput")
    t_i2lo = nc.dram_tensor("idx2lo", [128, n2lo], dt.int16, kind="ExternalInput")
    t_i2hi = nc.dram_tensor("idx2hi", [128, n2hi], dt.int16, kind="ExternalInput")
    t_ald1 = nc.dram_tensor("ald1", [128, 8 * NG1], dt.int16,
                            kind="ExternalInput")
    t_ald2 = nc.dram_tensor("ald2", [128, 8 * NG2], dt.int16,
                            kind="ExternalInput")
    t_scat1 = nc.dram_tensor("scat1", [128, 8 * NG1], dt.int16,
                             kind="ExternalInput")
    t_out = nc.dram_tensor("out", [GPOOL, OUT], dt.float32,
                           kind="ExternalOutput")

    rg = [list(range(NCORES))]
    _qc = [0]

    def nextq():
        _qc[0] = (_qc[0] + 1) % 4
        return _qc[0]

    with tile.TileContext(nc) as tc:
        with (
            tc.tile_pool(name="const", bufs=1) as constp,
            tc.tile_pool(name="pre", bufs=1) as prep_pool,
            tc.tile_pool(name="dram", bufs=1, space="DRAM") as dramp,
        ):
            nc.gpsimd.load_library(mlp)

            # preload all gather indices once (keeps the per-superblock
            # critical path free of HWDGE idx loads)
            n1lo = max(8 * sum(DLO1), 8)
            n1hi = max(8 * sum(DHI1), 8)
            n2lo = max(8 * sum(DLO2), 8)
            n2hi = max(8 * sum(DHI2), 8)
            i1lo_all = prep_pool.tile([128, n1lo], dt.int16)
            nc.sync.dma_start(i1lo_all[:], t_i1lo[:])
            i1hi_all = prep_pool.tile([128, n1hi], dt.int16)
            nc.sync.dma_start(i1hi_all[:], t_i1hi[:])
            i2lo_all = prep_pool.tile([128, n2lo], dt.int16)
            i2hi_all = prep_pool.tile([128, n2hi], dt.int16)
            scat1_t = prep_pool.tile([128, 8 * NG1], dt.int16)
            nc.sync.dma_start(scat1_t[:], t_scat1[:])
            ald1_t = prep_pool.tile([128, NG1, 4], dt.float32)
            ald2_t = prep_pool.tile([128, NG2, 1], dt.float32)

            table1 = dramp.tile([XT_COLS, 256], dt.bfloat16, tag="table1")
            table2 = dramp.tile([_ceil_to(N + 2, 4), 64], dt.float32,
                                tag="table2")
            h2sh = dramp.tile([SH2_ROWS, 64], dt.float32, tag="h2sh")
            cc_in = dramp.tile([HID, GPOOL], dt.float32, tag="ccin")
            cc_out = dramp.tile([HID, GPOOL], dt.float32, tag="ccout")

            w1_t = constp.tile([IN, 136], dt.bfloat16)
            nc.sync.dma_start(w1_t[:], t_w1[:])
            w2_t = constp.tile([IN, 34], dt.bfloat16)
            nc.sync.dma_start(w2_t[:], t_w2[:])
            wl_t = constp.tile([HID, OUT], dt.float32)
            nc.sync.dma_start(wl_t[:], t_wl[:])
            b1_t = constp.tile([128, HEADS * HID], dt.float32)
            nc.sync.dma_start(b1_t[:], t_b1[:])
            b2_t = constp.tile([128, HID], dt.float32)
            nc.sync.dma_start(b2_t[:], t_b2[:])
            bl_t = constp.tile([GPOOL, OUT], dt.float32)
            nc.sync.dma_start(bl_t[:], t_bl[:])
            rc_t = constp.tile([HID, GPOOL], dt.float32)
            nc.sync.dma_start(rc_t[:], t_rcnt[:])
            id_t = constp.tile([128, 128], dt.bfloat16)
            nc.sync.dma_start(id_t[:], t_ident[:])

            # zero the scatter_add target
            with tc.tile_pool(name="zp", bufs=1) as zp:
                z_t = zp.tile([128, SH2_ROWS // 128 * 64], dt.float32)
                nc.vector.memset(z_t[:], 0.0)
                nc.sync.dma_start(
                    h2sh[:, :].rearrange("(p k) e -> p (k e)", p=128), z_t[:])

            # ---------------- phase X: build table1 ----------------
            with (
                tc.tile_pool(name="xload", bufs=3) as xlp,
                tc.tile_pool(name="xout", bufs=3) as xop,
                tc.tile_pool(name="xpsum", bufs=4, space="PSUM") as xpp,
            ):
                for t in range(NCHUNK):
                    xt_t = xlp.tile([IN, XCHUNK], dt.bfloat16, tag="xt")
                    nc.sync.dma_start(xt_t[:],
                                      t_xT[:, t * XCHUNK:(t + 1) * XCHUNK])
                    o_t = xop.tile([128, 4, 256], dt.bfloat16, tag="xo")
                    nc.vector.memset(o_t[:, :, 136:256], 0.0)
                    for k in range(4):
                        ps = xpp.tile([128, 136], dt.float32, tag="xp")
                        nc.tensor.matmul(ps[:], xt_t[:, k * 128:(k + 1) * 128],
                                         w1_t[:], start=True, stop=True)
                        if k % 2 == 0:
                            nc.vector.tensor_copy(o_t[:, k, 0:136], ps[:])
                        else:
                            nc.scalar.activation(o_t[:, k, 0:136], ps[:],
                                                 Act.Copy)
                    nc.sync.dma_start(
                        table1[t * XCHUNK:(t + 1) * XCHUNK, :].rearrange(
                            "(k p) e -> p k e", p=128), o_t[:])
            with tc.tile_pool(name="patchp", bufs=1) as pp:
                p1_t = pp.tile([2, 256], dt.bfloat16)
                nc.sync.dma_start(p1_t[:], t_patch1[:])
                nc.sync.dma_start(table1[LO_MAX:LO_MAX + 1, :], p1_t[0:1, :])
                nc.sync.dma_start(table1[N + 1:N + 2, :], p1_t[1:2, :])
                p2_t = pp.tile([2, 64], dt.float32)
                nc.sync.dma_start(p2_t[:], t_patch2[:])
                nc.sync.dma_start(table2[0:1, :], p2_t[0:1, :])
                nc.sync.dma_start(table2[N + 1:N + 2, :], p2_t[1:2, :])

            if PHASES >= 2:
                # ---------------- phase L1: edges ----------------
                tab1_lo = table1[0:LO_MAX + 1, :]
                tab1_hi = table1[LO_MAX + 1:N + 2, :]
                Dmax1 = max(DLO1[g] + DHI1[g] for g in range(NG1))
                NBSB1 = max(sum(DLO1[g] + DHI1[g] for g in sb) for sb in SB1)
                olo = np.concatenate([[0], np.cumsum(DLO1)]).astype(int)
                ohi = np.concatenate([[0], np.cumsum(DHI1)]).astype(int)
                # own-destination ald values, extracted once from table1
                with tc.tile_pool(name="aldtmp", bufs=1) as atp:
                    atmp = atp.tile([128, NG1, 256], dt.bfloat16)
                    nc.sync.dma_start(
                        atmp[:],
                        table1[0:128 * NG1, :].rearrange("(b p) e -> p b e",
                                                         p=128))
                    nc.vector.tensor_copy(ald1_t[:], atmp[:, :, 132:136])
                with (
                    tc.tile_pool(name="gath1", bufs=4) as gathp,
                    tc.tile_pool(name="small1", bufs=3) as smallp,
                    tc.tile_pool(name="epi1", bufs=3) as epip,
                    tc.tile_pool(name="scatp", bufs=1) as scatp,
                    tc.tile_pool(name="agg1", bufs=2, space="PSUM") as aggp,
                    tc.tile_pool(name="psT1", bufs=2, space="PSUM") as psTp,
                    tc.tile_pool(name="ps21", bufs=2, space="PSUM") as ps2p,
                ):
                    scat_t = scatp.tile([128, NG1, 64], dt.float32, tag="sc")
                    nc.vector.memset(scat_t[:], 0.0)
                    elu_all = scatp.tile([128, NG1, 128], dt.bfloat16,
                                         tag="eluall")
                    for sb in SB1:
                        g0 = sb[0]
                        nlo = sum(DLO1[g] for g in sb)
                        nhi = sum(DHI1[g] for g in sb)
                        gb_t = gathp.tile([128, NBSB1, 256],
                                          dt.bfloat16, tag="gb")
                        if nlo > 0:
                            nc.gpsimd.dma_gather(
                                gb_t[:, :nlo, :], tab1_lo,
                                i1lo_all[:, 8 * olo[g0]:8 * (olo[g0] + nlo)],
                                128 * nlo, 128 * nlo, 256,
                                single_packet=False, queue_num=nextq())
                        if nhi > 0:
                            nc.gpsimd.dma_gather(
                                gb_t[:, nlo:nlo + nhi, :], tab1_hi,
                                i1hi_all[:, 8 * ohi[g0]:8 * (ohi[g0] + nhi)],
                                128 * nhi, 128 * nhi, 256, single_packet=False,
                                queue_num=nextq())
                        lo_off = 0
                        hi_off = 0
                        for gi, g in enumerate(sb):
                            dlo, dhi = DLO1[g], DHI1[g]
                            D = dlo + dhi
                            if L1STEP < 2:
                                lo_off += dlo
                                hi_off += dhi
                                continue
                            logit_t = smallp.tile([128, Dmax1, 4], dt.float32,
                                                  tag="lg")
                            exb_t = smallp.tile([128, Dmax1, 4], dt.bfloat16,
                                                tag="exb")
                            den_t = smallp.tile([128, 4], dt.float32, tag="dn")
                            rec_t = smallp.tile([128, 4], dt.float32, tag="rc")
                            ald_ap = ald1_t[:, g, :]
                            if dlo > 0:
                                nc.vector.scalar_tensor_tensor(
                                    logit_t[:, :dlo, :],
                                    gb_t[:, lo_off:lo_off + dlo, 128:132], 0.0,
                                    ald_ap.unsqueeze(1).broadcast_to(
                                        (128, dlo, 4)), Alu.add, Alu.add)
                            if dhi > 0:
                                nc.vector.scalar_tensor_tensor(
                                    logit_t[:, dlo:D, :],
                                    gb_t[:, nlo + hi_off:nlo + hi_off + dhi,
                                         128:132], 0.0,
                                    ald_ap.unsqueeze(1).broadcast_to(
                                        (128, dhi, 4)), Alu.add, Alu.add)
                            nc.vector.scalar_tensor_tensor(
                                logit_t[:, :D, :], logit_t[:, :D, :], NEG,
                                logit_t[:, :D, :], Alu.mult, Alu.max)
                            nc.scalar.activation(exb_t[:, :D, :],
                                                 logit_t[:, :D, :], Act.Exp)
                            nc.vector.tensor_reduce(
                                den_t[:], exb_t[:, :D, :].transpose([0, 2, 1]),
                                axis=Axis.X, op=Alu.add)
                            nc.vector.reciprocal(rec_t[:], den_t[:])
                            if L1STEP < 3:
                                lo_off += dlo
                                hi_off += dhi
                                continue
                            if dlo > 0:
                                h_lo = gb_t[:, lo_off:lo_off + dlo, 0:128]
                                h_lo = h_lo.rearrange("p b (h c) -> p b h c",
                                                      h=4)
                                nc.vector.tensor_tensor(
                                    h_lo, h_lo,
                                    exb_t[:, :dlo, :].unsqueeze(3).broadcast_to(
                                        (128, dlo, 4, HID)), Alu.mult)
                            if dhi > 0:
                                h_hi = gb_t[:, nlo + hi_off:nlo + hi_off + dhi,
                                            0:128]
                                h_hi = h_hi.rearrange("p b (h c) -> p b h c",
                                                      h=4)
                                nc.vector.tensor_tensor(
                                    h_hi, h_hi,
                                    exb_t[:, dlo:D, :].unsqueeze(3).broadcast_to(
                                        (128, dhi, 4, HID)), Alu.mult)
                            if L1STEP < 4:
                                lo_off += dlo
                                hi_off += dhi
                                continue
                            agg = aggp.tile([128, 128], dt.float32, tag="agg")
                            rhss = ([gb_t[:, lo_off + b, 0:128]
                                     for b in range(dlo)]
                                    + [gb_t[:, nlo + hi_off + b, 0:128]
                                       for b in range(dhi)])
                            for bi, rhs in enumerate(rhss):
                                nc.tensor.matmul(agg[:], id_t[:], rhs,
                                                 start=(bi == 0),
                                                 stop=(bi == len(rhss) - 1))
                            if L1STEP < 5:
                                lo_off += dlo
                                hi_off += dhi
                                continue
                            scaled_t = epip.tile([128, 128], dt.float32,
                                                 tag="sd")
                            nc.vector.tensor_tensor(
                                scaled_t[:].rearrange("p (h c) -> p h c", h=4),
                                agg[:].rearrange("p (h c) -> p h c", h=4),
                                rec_t[:].unsqueeze(2).broadcast_to(
                                    (128, 4, HID)), Alu.mult)
                            if HASB1:
                                nc.vector.tensor_tensor(
                                    scaled_t[:], scaled_t[:], b1_t[:], Alu.add)
                            tmp_t = epip.tile([128, 128], dt.float32, tag="tm")
                            nc.scalar.activation(tmp_t[:], scaled_t[:], Act.Relu,
                                                 scale=-1.0)
                            nc.scalar.activation(tmp_t[:], tmp_t[:], Act.Exp,
                                                 scale=-1.0)
                            nc.vector.scalar_tensor_tensor(
                                elu_all[:, g, :], tmp_t[:], -1.0, scaled_t[:],
                                Alu.add, Alu.max)
                            lo_off += dlo
                            hi_off += dhi
                        # ---- pass 2 for this superblock's groups:
                        # transpose + W2ext, filling the scatter source
                        if L1STEP >= 5:
                            for g in sb:
                                psT = psTp.tile([128, 128], dt.bfloat16,
                                                tag="pt")
                                nc.tensor.transpose(psT[:], elu_all[:, g, :],
                                                    id_t[:])
                                eluT_t = epip.tile([128, 128], dt.bfloat16,
                                                   tag="et")
                                nc.scalar.activation(eluT_t[:], psT[:],
                                                     Act.Copy)
                                ps2 = ps2p.tile([128, 34], dt.float32,
                                                tag="p2")
                                nc.tensor.matmul(ps2[:], eluT_t[:], w2_t[:],
                                                 start=True, stop=True)
                                if g % 2 == 0:
                                    nc.scalar.activation(scat_t[:, g, 0:34],
                                                         ps2[:], Act.Copy)
                                else:
                                    nc.vector.tensor_copy(scat_t[:, g, 0:34],
                                                          ps2[:])
                    if L1STEP >= 6:
                        nc.gpsimd.dma_scatter_add(
                            h2sh[0:S + 1, :], scat_t[:], scat1_t[:],
                            128 * NG1, 128 * NG1, 64,
                            single_packet=False, queue_num=nextq())
            if PHASES >= 3:
                # ---------------- exchange ----------------
                nc.gpsimd.collective_compute(
                    "AllGather", mybir.AluOpType.bypass, replica_groups=rg,
                    ins=[h2sh[0:S, :]], outs=[table2[1:N + 1, :]])

            if PHASES >= 4:
                # ---------------- phase L2: edges + pool ----------------
                # preloads below overlap the AllGather
                nc.sync.dma_start(i2lo_all[:], t_i2lo[:])
                nc.sync.dma_start(i2hi_all[:], t_i2hi[:])
                with tc.tile_pool(name="aldtmp2", bufs=1) as atp:
                    atmp = atp.tile([128, NG2, 64], dt.float32)
                    nc.sync.dma_start(
                        atmp[:],
                        h2sh[0:128 * NG2, :].rearrange("(b p) e -> p b e",
                                                       p=128))
                    nc.vector.tensor_copy(ald2_t[:], atmp[:, :, 33:34])
                tab2_lo = table2[0:LO_MAX + 1, :]
                tab2_hi = table2[LO_MAX + 1:N + 2, :]
                Dmax2 = max(DLO2[g] + DHI2[g] for g in range(NG2))
                NBSB2 = max(sum(DLO2[g] + DHI2[g] for g in sb) for sb in SB2)
                olo2 = np.concatenate([[0], np.cumsum(DLO2)]).astype(int)
                ohi2 = np.concatenate([[0], np.cumsum(DHI2)]).astype(int)
                with (
                    tc.tile_pool(name="gath2", bufs=4) as gathp,
                    tc.tile_pool(name="small2", bufs=3) as smallp,
                    tc.tile_pool(name="epi2", bufs=3) as epip,
                    tc.tile_pool(name="agg2", bufs=2, space="PSUM") as aggp,
                    tc.tile_pool(name="poolps", bufs=1, space="PSUM") as poolpp,
                    tc.tile_pool(name="mp2", bufs=3) as mpp,
                ):
                    poolps = poolpp.tile([HID, GPOOL], dt.float32)
                    h2p_all = mpp.tile([128, NG2, HID], dt.bfloat16,
                                       tag="h2pall", bufs=1)
                    for sb in SB2:
                        g0 = sb[0]
                        nlo = sum(DLO2[g] for g in sb)
                        nhi = sum(DHI2[g] for g in sb)
                        gb_t = gathp.tile([128, NBSB2, 64], dt.float32,
                                          tag="gb")
                        if nlo > 0:
                            nc.gpsimd.dma_gather(
                                gb_t[:, :nlo, :], tab2_lo,
                                i2lo_all[:, 8 * olo2[g0]:8 * (olo2[g0] + nlo)],
                                128 * nlo, 128 * nlo, 64, single_packet=False,
                                queue_num=nextq())
                        if nhi > 0:
                            nc.gpsimd.dma_gather(
                                gb_t[:, nlo:nlo + nhi, :], tab2_hi,
                                i2hi_all[:, 8 * ohi2[g0]:8 * (ohi2[g0] + nhi)],
                                128 * nhi, 128 * nhi, 64, single_packet=False,
                                queue_num=nextq())
                        lo_off = 0
                        hi_off = 0
                        for gi, g in enumerate(sb):
                            dlo, dhi = DLO2[g], DHI2[g]
                            D = dlo + dhi
                            logit_t = smallp.tile([128, Dmax2, 1], dt.float32,
                                                  tag="lg")
                            exf_t = smallp.tile([128, Dmax2, 1], dt.float32,
                                                tag="exf")
                            den_t = smallp.tile([128, 1], dt.float32, tag="dn")
                            rec_t = smallp.tile([128, 1], dt.float32, tag="rc")
                            ald_ap = ald2_t[:, g, :]
                            if dlo > 0:
                                nc.vector.tensor_scalar(
                                    logit_t[:, :dlo, :],
                                    gb_t[:, lo_off:lo_off + dlo, 32:33],
                                    ald_ap, None, Alu.add)
                            if dhi > 0:
                                nc.vector.tensor_scalar(
                                    logit_t[:, dlo:D, :],
                                    gb_t[:, nlo + hi_off:nlo + hi_off + dhi,
                                         32:33],
                                    ald_ap, None, Alu.add)
                            nc.vector.scalar_tensor_tensor(
                                logit_t[:, :D, :], logit_t[:, :D, :], NEG,
                                logit_t[:, :D, :], Alu.mult, Alu.max)
                            nc.scalar.activation(exf_t[:, :D, :], logit_t[:, :D, :],
                                                 Act.Exp)
                            nc.vector.tensor_reduce(
                                den_t[:], exf_t[:, :D, :].transpose([0, 2, 1]),
                                axis=Axis.X, op=Alu.add)
                            nc.vector.reciprocal(rec_t[:], den_t[:])
                            exh_t = smallp.tile([128, Dmax2, HID], dt.bfloat16,
                                                tag="exh")
                            if dlo > 0:
                                nc.vector.tensor_tensor(
                                    exh_t[:, :dlo, :],
                                    gb_t[:, lo_off:lo_off + dlo, 0:HID],
                                    exf_t[:, :dlo, :].broadcast_to(
                                        (128, dlo, HID)), Alu.mult)
                            if dhi > 0:
                                nc.vector.tensor_tensor(
                                    exh_t[:, dlo:D, :],
                                    gb_t[:, nlo + hi_off:nlo + hi_off + dhi,
                                         0:HID],
                                    exf_t[:, dlo:D, :].broadcast_to(
                                        (128, dhi, HID)), Alu.mult)
                            agg = aggp.tile([128, HID], dt.float32, tag="agg")
                            for bi in range(D):
                                nc.tensor.matmul(agg[:], id_t[:], exh_t[:, bi, :],
                                                 start=(bi == 0),
                                                 stop=(bi == D - 1))
                            scaled_t = epip.tile([128, HID], dt.float32, tag="sd")
                            nc.vector.tensor_scalar(scaled_t[:], agg[:], rec_t[:],
                                                    None, Alu.mult)
                            if HASB2:
                                nc.vector.tensor_tensor(
                                    scaled_t[:], scaled_t[:], b2_t[:], Alu.add)
                            tmp_t = epip.tile([128, HID], dt.float32, tag="tm")
                            nc.scalar.activation(tmp_t[:], scaled_t[:], Act.Relu,
                                                 scale=-1.0)
                            nc.scalar.activation(tmp_t[:], tmp_t[:], Act.Exp,
                                                 scale=-1.0)
                            nc.vector.scalar_tensor_tensor(
                                h2p_all[:, g, :], tmp_t[:], -1.0, scaled_t[:],
                                Alu.add, Alu.max)
                            mp_t = mpp.tile([128, GPOOL], dt.bfloat16,
                                            tag="mp")
                            nc.sync.dma_start(
                                mp_t[:], t_mpool[g * 128:(g + 1) * 128, :])
                            nc.tensor.matmul(poolps[:], h2p_all[:, g, :],
                                             mp_t[:], start=(g == 0),
                                             stop=(g == NG2 - 1))
                            lo_off += dlo
                            hi_off += dhi
                    # ------------- pool + final linear -------------
                    with tc.tile_pool(name="fin", bufs=1) as finp, \
                            tc.tile_pool(name="finps", bufs=1, space="PSUM") as fpp:
                        poolsb = finp.tile([HID, GPOOL], dt.float32)
                        nc.vector.tensor_copy(poolsb[:], poolps[:])
                        nc.sync.dma_start(cc_in[:, :], poolsb[:])
                        nc.gpsimd.collective_compute(
                            "AllReduce", Alu.add, replica_groups=rg,
                            ins=[cc_in[:, :]], outs=[cc_out[:, :]])
                        psum_t = finp.tile([HID, GPOOL], dt.float32)
                        nc.sync.dma_start(psum_t[:], cc_out[:, :])
                        mean_t = finp.tile([HID, GPOOL], dt.float32)
                        nc.vector.tensor_tensor(
                            mean_t[:], psum_t[:],
                            rc_t[:], Alu.mult)
                        psO = fpp.tile([GPOOL, OUT], dt.float32)
                        nc.tensor.matmul(psO[:], mean_t[:], wl_t[:], start=True,
                                         stop=True)
                        out_t = finp.tile([GPOOL, OUT], dt.float32)
                        if HASBL:
                            nc.vector.tensor_tensor(out_t[:], psO[:], bl_t[:],
                                                    Alu.add)
                        else:
                            nc.vector.tensor_copy(out_t[:], psO[:])
                        nc.sync.dma_start(t_out[:, :], out_t[:])
            if PHASES < 4:
                _emit_dummy_out(nc, tc, t_out, dt)


    nc.compile()
    return nc


def core_inputs(prep, c):
    cd = prep["cores"][c]

    def padcols(a, cols):
        if a.shape[1] == cols:
            return a
        out = np.zeros((a.shape[0], cols), a.dtype)
        out[:, :a.shape[1]] = a
        return out

    n1lo = max(8 * sum(prep["DLO1"]), 8)
    n1hi = max(8 * sum(prep["DHI1"]), 8)
    n2lo = max(8 * sum(prep["DLO2"]), 8)
    n2hi = max(8 * sum(prep["DHI2"]), 8)
    return dict(
        xT=np.ascontiguousarray(cd["xT"]),
        w1ext=prep["W1ext"], w2ext=prep["W2ext"], wl=prep["Wl"],
        b1=prep["b1"], b2=prep["b2"], bl=prep["bl"], rcnt=prep["rcnt"],
        patch1=prep["patch1"], patch2=prep["patch2"], ident=prep["ident"],
        mpool=np.ascontiguousarray(cd["mpool"]),
        idx1lo=padcols(cd["w_idx1lo"], n1lo),
        idx1hi=padcols(cd["w_idx1hi"], n1hi),
        idx2lo=padcols(cd["w_idx2lo"], n2lo),
        idx2hi=padcols(cd["w_idx2hi"], n2hi),
        ald1=cd["w_ald1"], ald2=cd["w_ald2"], scat1=cd["w_scat1"],
    )


_CACHE = {}


def kernel(**inputs):
    from concourse.bass_utils import run_bass_kernel_spmd

    inputs = {k: np.asarray(v) for k, v in inputs.items()}
    prep = host_prep(**inputs)
    sc = make_sched(prep)
    key = str(sc)
    if key not in _CACHE:
        _CACHE[key] = build_bass(sc)
    nc = _CACHE[key]
    in_maps = [core_inputs(prep, c) for c in range(NCORES)]
    res = run_bass_kernel_spmd(nc, in_maps, list(range(NCORES)))
    return np.asarray(res.results[0]["out"], np.float32)



# revision 12
# speedup vs baseline: 1.5135x; 1.4834x over previous
# Self-contained 8-core Trainium2 Bass kernel for the 2-layer GAT + mean-pool
# problem (nn_GAT_83820581749190).
#
# Sharding: destination nodes (and all their incident edges) are partitioned
# across the 8 cores, so each layer's attention softmax and aggregation
# complete locally per core. Each core builds a replicated layer-1 feature
# table h1 (bf16, 256-byte logical rows) in HBM with a replicated x @ W1
# matmul, then edge-gathers PAIRS of rows (512B per descriptor, index =
# row//2, int16-safe) with the GPSIMD dma_gather custom op; host-precomputed
# parity masks select the correct half downstream. Attention logits are
# computed on-chip (DVE dot with a_src/a_dst), the edge softmax runs without
# segment-max (logits are small; pad slots use a patch row whose h gives
# al_src=-100), and aggregation is identity-matmul PSUM accumulation
# (destinations on partitions via degree-bucketed groups of 128).
# Layer-2 features are exchanged with an AllGather; the same paired-row
# gather runs against the fp32 layer-2 table; mean-pool is a matmul against
# a host-built one-hot graph matrix plus a tiny AllReduce.
import numpy as np
import ml_dtypes

N = 50000
E = 800000
IN = 128
HID = 32
HEADS = 4
OUT = 10
GPOOL = 64
NEG = 0.2
NCORES = 8
S = N // NCORES
SPECIAL1 = N          # layer-1 patch row (h chosen so h . a_src = -100)
SPECIAL2 = 0          # layer-2 patch row (al_src column = -100)
SPECIAL_ALS = -100.0
SB_BLOCK_BUDGET = 24  # max gather blocks per superblock
XCHUNK = 512
PHASES = 99
L1STEP = 99

bf16 = ml_dtypes.bfloat16


def _ceil_to(v, m):
    return (v + m - 1) // m * m


# ======================= host prep =========================================

def _build_layer(src, dstl):
    deg = np.bincount(dstl, minlength=S)
    P = np.argsort(-deg, kind="stable")
    Ppos = np.empty(S, np.int64)
    Ppos[P] = np.arange(S)
    ng = (S + 127) // 128
    D = np.zeros(ng, np.int64)
    dp = deg[P]
    for g in range(ng):
        D[g] = dp[g * 128:(g + 1) * 128].max()
    assert (D > 0).all()
    return dict(src=src, dstl=dstl, deg=deg, P=P, Ppos=Ppos, D=D)


def _emit_slots(l, Dg, row_of_src, special_row):
    """Per group g: rows[g] [D[g],128] of table ROW ids (special_row pads),
    plus slot2cmp mapping output slots -> compacted dst ids."""
    NG = len(Dg)
    Ppos = l["Ppos"]
    nreal = S
    slot2cmp = np.full(NG * 128, -1, np.int64)
    slot2cmp[:nreal] = np.arange(nreal)
    rows = [np.full((int(Dg[g]), 128), special_row, np.int64)
            for g in range(NG)]
    slot_of_edge = Ppos[l["dstl"]]
    order = np.argsort(slot_of_edge, kind="stable")
    so = slot_of_edge[order]
    sr = row_of_src[l["src"][order]]
    jj = np.arange(len(so)) - np.searchsorted(so, so, side="left")
    gg, kk = so // 128, so % 128
    for g in range(NG):
        sel = gg == g
        if sel.any():
            rows[g][jj[sel], kk[sel]] = sr[sel]
    return rows, slot2cmp


def _wrap16(idx):
    """[n] -> [128, n//16] int16: idx i at [i%16, i//16], replicated x8."""
    n = len(idx)
    assert n % 16 == 0
    w = np.ascontiguousarray(np.asarray(idx).reshape(n // 16, 16).T)
    w = w.astype(np.int16)
    return np.tile(w, (8, 1))


def _wrap_rows(rows_arrs):
    """idx stream (row//2) wrapped, plus even-parity masks [128, NB]."""
    idx_segs = []
    pme_segs = []
    for a in rows_arrs:
        if a.size:
            assert (a // 2 <= 32767).all()
            idx_segs.append(_wrap16((a // 2).reshape(-1)))
            pme_segs.append(np.ascontiguousarray((1 - (a % 2)).T))
    w_idx = (np.concatenate(idx_segs, axis=1) if idx_segs
             else np.zeros((128, 0), np.int16))
    pme = (np.concatenate(pme_segs, axis=1).astype(bf16) if pme_segs
           else np.zeros((128, 0), bf16))
    return w_idx, pme


def host_prep(x, edge_index, batch, W1, a1_src, a1_dst, b1, W2, a2_src, a2_dst,
              b2, Wl, bl):
    x = np.asarray(x, np.float32)
    edge_index = np.asarray(edge_index, np.int64)
    batch = np.asarray(batch, np.int64)
    src_all = np.concatenate([edge_index[0], np.arange(N, dtype=np.int64)])
    dst_all = np.concatenate([edge_index[1], np.arange(N, dtype=np.int64)])
    owner = dst_all // S

    a1_src = np.asarray(a1_src, np.float32)
    a1_dst = np.asarray(a1_dst, np.float32)
    W1 = np.asarray(W1, np.float32)
    W2 = np.asarray(W2, np.float32)
    W2ext = np.concatenate(
        [W2, W2 @ np.asarray(a2_src, np.float32)[0][:, None],
         W2 @ np.asarray(a2_dst, np.float32)[0][:, None]], axis=1)  # [128,34]

    # a1x: [0:128]=a_src flat, [128:256]=a_src flat, [256:384]=a_dst flat
    asf = a1_src.reshape(-1)
    adf = a1_dst.reshape(-1)
    a1x = np.tile(np.concatenate([asf, asf, adf])[None, :], (128, 1))

    # layer-1 patch row: h with h . a_src[h] = -100 for every head
    hp = np.concatenate([SPECIAL_ALS * a1_src[h] / (a1_src[h] ** 2).sum()
                         for h in range(HEADS)])
    assert np.abs(hp).max() < 1e4
    patch1 = np.tile(hp[None, :], (1, 1))

    cores = [dict(c=c) for c in range(NCORES)]
    for cd in cores:
        c = cd["c"]
        m = owner == c
        cd["src"] = src_all[m]
        cd["dstl"] = dst_all[m] - c * S

    # ---------- layer 1 ----------
    for cd in cores:
        c = cd["c"]
        l1 = _build_layer(cd["src"], cd["dstl"])
        pos_of = np.empty(N, np.int64)
        own = np.arange(c * S, (c + 1) * S)
        oth = np.concatenate([np.arange(0, c * S), np.arange((c + 1) * S, N)])
        pos_of[oth] = S + np.arange(N - S)
        pos_of[own] = l1["Ppos"]
        cd["l1"] = l1
        cd["row_of"] = pos_of
    NG1 = max(len(cd["l1"]["D"]) for cd in cores)
    D1 = np.zeros(NG1, np.int64)
    for cd in cores:
        d = cd["l1"]["D"]
        D1[:len(d)] = np.maximum(D1[:len(d)], d)
    for cd in cores:
        cd["rows1"], cd["slot2cmp1"] = _emit_slots(
            cd["l1"], D1, cd["row_of"], SPECIAL1)

    # ---------- layer 2 ----------
    for cd in cores:
        cd["l2"] = _build_layer(cd["src"], cd["dstl"])
    pos2_of = np.empty(N, np.int64)
    for cd in cores:
        c = cd["c"]
        pos2_of[c * S:(c + 1) * S] = c * S + cd["l2"]["Ppos"]
    row2_of = pos2_of + 1
    NG2 = max(len(cd["l2"]["D"]) for cd in cores)
    D2 = np.zeros(NG2, np.int64)
    for cd in cores:
        d = cd["l2"]["D"]
        D2[:len(d)] = np.maximum(D2[:len(d)], d)
    for cd in cores:
        cd["rows2"], cd["slot2cmp2"] = _emit_slots(
            cd["l2"], D2, row2_of, SPECIAL2)

    # ---------- aux ----------
    cnt = np.bincount(batch, minlength=GPOOL).astype(np.float32)
    recip_cnt = (1.0 / np.maximum(cnt, 1.0)).astype(np.float32)

    XT_COLS = _ceil_to(N + 2, XCHUNK)
    for cd in cores:
        c = cd["c"]
        gids = batch[c * S:(c + 1) * S]
        Mp = np.zeros((NG2 * 128, GPOOL), np.float32)
        s2c = cd["slot2cmp2"]
        real = s2c >= 0
        Mp[np.where(real)[0], gids[cd["l2"]["P"][s2c[real]]]] = 1.0
        cd["mpool"] = Mp.astype(bf16)

        s2c1 = cd["slot2cmp1"]
        tgt = np.full(len(s2c1), S, np.int64)  # trash row for dummy slots
        r1 = s2c1 >= 0
        tgt[r1] = cd["l2"]["Ppos"][cd["l1"]["P"][s2c1[r1]]]

        xt = np.zeros((IN, XT_COLS), np.float32)
        xt[:, cd["row_of"]] = x.T
        cd["xT"] = xt.astype(bf16)

        cd["w_idx1"], cd["pme1"] = _wrap_rows(cd["rows1"])
        cd["w_idx2"], cd["pme2"] = _wrap_rows(cd["rows2"])
        cd["w_scat1"] = _wrap16(tgt)

    patch2 = np.zeros((2, 64), np.float32)
    patch2[0, 32] = SPECIAL_ALS

    return dict(cores=cores,
                D1=[int(v) for v in D1], D2=[int(v) for v in D2],
                W1=W1.astype(bf16), W2ext=W2ext.astype(bf16),
                Wl=np.asarray(Wl, np.float32),
                a1x=a1x.astype(bf16),
                b1=np.tile(np.asarray(b1, np.float32).reshape(1, -1),
                           (128, 1)),
                b2=np.tile(np.asarray(b2, np.float32).reshape(1, -1),
                           (128, 1)),
                bl=np.tile(np.asarray(bl, np.float32).reshape(1, -1),
                           (GPOOL, 1)),
                rcnt=np.tile(recip_cnt.reshape(1, -1), (HID, 1)),
                patch1=patch1.astype(bf16), patch2=patch2,
                ident=np.eye(128, dtype=bf16))


def _pack_superblocks(D, budget=SB_BLOCK_BUDGET):
    sbs, cur, tot = [], [], 0
    for g in range(len(D)):
        d = int(D[g])
        if cur and tot + d > budget:
            sbs.append(cur)
            cur, tot = [], 0
        cur.append(g)
        tot += d
    if cur:
        sbs.append(cur)
    return sbs


def make_sched(prep):
    D1, D2 = prep["D1"], prep["D2"]
    return dict(D1=D1, D2=D2,
                SB1=_pack_superblocks(D1), SB2=_pack_superblocks(D2),
                HASB1=bool(np.any(prep["b1"])), HASB2=bool(np.any(prep["b2"])),
                HASBL=bool(np.any(prep["bl"])))


# ======================= bass kernel =======================================

def build_bass(sc):
    import concourse.bacc as bacc
    import concourse.tile as tile
    import concourse.mybir as mybir
    from concourse.library_config import mlp

    dt = mybir.dt
    Alu = mybir.AluOpType
    Act = mybir.ActivationFunctionType
    Axis = mybir.AxisListType

    D1, D2 = sc["D1"], sc["D2"]
    SB1, SB2 = sc["SB1"], sc["SB2"]
    HASB1 = sc.get("HASB1", True)
    HASB2 = sc.get("HASB2", True)
    HASBL = sc.get("HASBL", True)
    NG1, NG2 = len(D1), len(D2)
    XT_COLS = _ceil_to(N + 2, XCHUNK)
    NCHUNK = XT_COLS // XCHUNK
    SH2_ROWS = _ceil_to(S + 2, 128)
    NB1 = sum(D1)
    NB2 = sum(D2)
    o1 = np.concatenate([[0], np.cumsum(D1)]).astype(int)
    o2 = np.concatenate([[0], np.cumsum(D2)]).astype(int)

    nc = bacc.Bacc("TRN2", target_bir_lowering=False, debug=False,
                   num_devices=NCORES, num_swdge_queues=4)

    t_xT = nc.dram_tensor("xT", [IN, XT_COLS], dt.bfloat16,
                          kind="ExternalInput")
    t_w1 = nc.dram_tensor("w1", [IN, IN], dt.bfloat16, kind="ExternalInput")
    t_w2 = nc.dram_tensor("w2ext", [IN, 34], dt.bfloat16,
                          kind="ExternalInput")
    t_wl = nc.dram_tensor("wl", [HID, OUT], dt.float32, kind="ExternalInput")
    t_a1x = nc.dram_tensor("a1x", [128, 384], dt.bfloat16,
                           kind="ExternalInput")
    t_b1 = nc.dram_tensor("b1", [128, HEADS * HID], dt.float32,
                          kind="ExternalInput")
    t_b2 = nc.dram_tensor("b2", [128, HID], dt.float32, kind="ExternalInput")
    t_bl = nc.dram_tensor("bl", [GPOOL, OUT], dt.float32,
                          kind="ExternalInput")
    t_rcnt = nc.dram_tensor("rcnt", [HID, GPOOL], dt.float32,
                            kind="ExternalInput")
    t_patch1 = nc.dram_tensor("patch1", [1, 128], dt.bfloat16,
                              kind="ExternalInput")
    t_patch2 = nc.dram_tensor("patch2", [2, 64], dt.float32,
                              kind="ExternalInput")
    t_ident = nc.dram_tensor("ident", [128, 128], dt.bfloat16,
                             kind="ExternalInput")
    t_mpool = nc.dram_tensor("mpool", [NG2 * 128, GPOOL], dt.bfloat16,
                             kind="ExternalInput")
    n1 = max(8 * NB1, 8)
    n2 = max(8 * NB2, 8)
    t_i1 = nc.dram_tensor("idx1", [128, n1], dt.int16, kind="ExternalInput")
    t_i2 = nc.dram_tensor("idx2", [128, n2], dt.int16, kind="ExternalInput")
    t_pm1 = nc.dram_tensor("pme1", [128, max(NB1, 1)], dt.bfloat16,
                           kind="ExternalInput")
    t_pm2 = nc.dram_tensor("pme2", [128, max(NB2, 1)], dt.bfloat16,
                           kind="ExternalInput")
    t_scat1 = nc.dram_tensor("scat1", [128, 8 * NG1], dt.int16,
                             kind="ExternalInput")
    t_out = nc.dram_tensor("out", [GPOOL, OUT], dt.float32,
                           kind="ExternalOutput")

    rg = [list(range(NCORES))]
    _qc = [0]

    def nextq():
        _qc[0] = (_qc[0] + 1) % 4
        return _qc[0]

    with tile.TileContext(nc) as tc:
        with (
            tc.tile_pool(name="const", bufs=1) as constp,
            tc.tile_pool(name="pre", bufs=1) as prep_pool,
            tc.tile_pool(name="dram", bufs=1, space="DRAM") as dramp,
        ):
            nc.gpsimd.load_library(mlp)

            # logical row-major tables; gathers view them as paired rows
            table1 = dramp.tile([XT_COLS, 128], dt.bfloat16, tag="table1")
            table2 = dramp.tile([_ceil_to(N + 2, 4), 64], dt.float32,
                                tag="table2")
            h2sh = dramp.tile([SH2_ROWS, 64], dt.float32, tag="h2sh")
            cc_in = dramp.tile([HID, GPOOL], dt.float32, tag="ccin")
            cc_out = dramp.tile([HID, GPOOL], dt.float32, tag="ccout")

            w1_t = constp.tile([IN, IN], dt.bfloat16)
            nc.sync.dma_start(w1_t[:], t_w1[:])
            w2_t = constp.tile([IN, 34], dt.bfloat16)
            nc.sync.dma_start(w2_t[:], t_w2[:])
            wl_t = constp.tile([HID, OUT], dt.float32)
            nc.sync.dma_start(wl_t[:], t_wl[:])
            a1x_t = constp.tile([128, 384], dt.bfloat16)
            nc.sync.dma_start(a1x_t[:], t_a1x[:])
            b1_t = constp.tile([128, HEADS * HID], dt.float32)
            nc.sync.dma_start(b1_t[:], t_b1[:])
            b2_t = constp.tile([128, HID], dt.float32)
            nc.sync.dma_start(b2_t[:], t_b2[:])
            bl_t = constp.tile([GPOOL, OUT], dt.float32)
            nc.sync.dma_start(bl_t[:], t_bl[:])
            rc_t = constp.tile([HID, GPOOL], dt.float32)
            nc.sync.dma_start(rc_t[:], t_rcnt[:])
            id_t = constp.tile([128, 128], dt.bfloat16)
            nc.sync.dma_start(id_t[:], t_ident[:])

            # preload all gather indices and parity masks
            i1_all = prep_pool.tile([128, n1], dt.int16)
            nc.sync.dma_start(i1_all[:], t_i1[:])
            pm1_t = prep_pool.tile([128, max(NB1, 1)], dt.bfloat16)
            nc.sync.dma_start(pm1_t[:], t_pm1[:])
            i2_all = prep_pool.tile([128, n2], dt.int16)
            pm2_t = prep_pool.tile([128, max(NB2, 1)], dt.bfloat16)
            scat1_t = prep_pool.tile([128, 8 * NG1], dt.int16)
            nc.sync.dma_start(scat1_t[:], t_scat1[:])
            ald1_t = prep_pool.tile([128, NG1, 4], dt.float32)
            ald2_t = prep_pool.tile([128, NG2, 1], dt.float32)

            # zero the scatter_add target
            with tc.tile_pool(name="zp", bufs=1) as zp:
                z_t = zp.tile([128, SH2_ROWS // 128 * 64], dt.float32)
                nc.vector.memset(z_t[:], 0.0)
                nc.sync.dma_start(
                    h2sh[:, :].rearrange("(p k) e -> p (k e)", p=128), z_t[:])

            # ---------------- phase X: build table1 ----------------
            with (
                tc.tile_pool(name="xload", bufs=3) as xlp,
                tc.tile_pool(name="xout", bufs=3) as xop,
                tc.tile_pool(name="xpsum", bufs=4, space="PSUM") as xpp,
            ):
                for t in range(NCHUNK):
                    xt_t = xlp.tile([IN, XCHUNK], dt.bfloat16, tag="xt")
                    nc.sync.dma_start(xt_t[:],
                                      t_xT[:, t * XCHUNK:(t + 1) * XCHUNK])
                    o_t = xop.tile([128, 4, 128], dt.bfloat16, tag="xo")
                    for k in range(4):
                        ps = xpp.tile([128, 128], dt.float32, tag="xp")
                        nc.tensor.matmul(ps[:], xt_t[:, k * 128:(k + 1) * 128],
                                         w1_t[:], start=True, stop=True)
                        if k % 2 == 0:
                            nc.vector.tensor_copy(o_t[:, k, :], ps[:])
                        else:
                            nc.scalar.activation(o_t[:, k, :], ps[:],
                                                 Act.Copy)
                    nc.sync.dma_start(
                        table1[t * XCHUNK:(t + 1) * XCHUNK, :].rearrange(
                            "(k p) e -> p k e", p=128), o_t[:])
            with tc.tile_pool(name="patchp", bufs=1) as pp:
                p1_t = pp.tile([1, 128], dt.bfloat16)
                nc.sync.dma_start(p1_t[:], t_patch1[:])
                nc.sync.dma_start(table1[SPECIAL1:SPECIAL1 + 1, :],
                                  p1_t[0:1, :])
                p2_t = pp.tile([2, 64], dt.float32)
                nc.sync.dma_start(p2_t[:], t_patch2[:])
                nc.sync.dma_start(table2[0:1, :], p2_t[0:1, :])
                nc.sync.dma_start(table2[N + 1:N + 2, :], p2_t[1:2, :])

            if PHASES >= 2:
                # ---------------- phase L1: edges ----------------
                tab1p = table1[:, :].rearrange("(a h) c -> a (h c)", h=2)
                Dmax1 = max(D1)
                NBSB1 = max(sum(D1[g] for g in sb) for sb in SB1)
                # own-destination ald via DVE dot with a_dst
                with tc.tile_pool(name="aldtmp", bufs=1) as atp:
                    atmp = atp.tile([128, NG1, 128], dt.bfloat16)
                    nc.sync.dma_start(
                        atmp[:],
                        table1[0:128 * NG1, :].rearrange("(b p) e -> p b e",
                                                         p=128))
                    aprod = atp.tile([128, NG1, 128], dt.bfloat16)
                    nc.vector.tensor_tensor(
                        aprod[:], atmp[:],
                        a1x_t[:, 256:384].unsqueeze(1).broadcast_to(
                            (128, NG1, 128)), Alu.mult)
                    nc.vector.tensor_reduce(
                        ald1_t[:],
                        aprod[:].rearrange("p b (h c) -> p b h c", h=4),
                        axis=Axis.X, op=Alu.add)
                with (
                    tc.tile_pool(name="gath1", bufs=4) as gathp,
                    tc.tile_pool(name="als1", bufs=2) as alsp,
                    tc.tile_pool(name="small1", bufs=3) as smallp,
                    tc.tile_pool(name="epi1", bufs=3) as epip,
                    tc.tile_pool(name="scatp", bufs=1) as scatp,
                    tc.tile_pool(name="agg1", bufs=2, space="PSUM") as aggp,
                    tc.tile_pool(name="psT1", bufs=2, space="PSUM") as psTp,
                    tc.tile_pool(name="ps21", bufs=2, space="PSUM") as ps2p,
                ):
                    scat_t = scatp.tile([128, NG1, 64], dt.float32, tag="sc")
                    nc.vector.memset(scat_t[:], 0.0)
                    elu_all = scatp.tile([128, NG1, 128], dt.bfloat16,
                                         tag="eluall")
                    for sb in SB1:
                        g0 = sb[0]
                        nb = sum(D1[g] for g in sb)
                        boff = o1[g0]
                        gb_t = gathp.tile([128, NBSB1, 256],
                                          dt.bfloat16, tag="gb")
                        nc.gpsimd.dma_gather(
                            gb_t[:, :nb, :], tab1p,
                            i1_all[:, 8 * boff:8 * (boff + nb)],
                            128 * nb, 128 * nb, 256,
                            single_packet=False, queue_num=nextq())
                        if L1STEP < 2:
                            continue
                        # al_src for both pair-halves: prod + reduce
                        prod_t = alsp.tile([128, NBSB1, 256], dt.bfloat16,
                                           tag="prod")
                        als8_t = alsp.tile([128, NBSB1, 8], dt.float32,
                                           tag="als8")
                        als_t = alsp.tile([128, NBSB1, 4], dt.float32,
                                          tag="als")
                        nc.vector.tensor_tensor(
                            prod_t[:, :nb, :], gb_t[:, :nb, :],
                            a1x_t[:, 0:256].unsqueeze(1).broadcast_to(
                                (128, nb, 256)), Alu.mult)
                        nc.vector.tensor_reduce(
                            als8_t[:, :nb, :],
                            prod_t[:, :nb, :].rearrange(
                                "p b (j c) -> p b j c", j=8),
                            axis=Axis.X, op=Alu.add)
                        # parity-select: als = even*pme + odd*(1-pme)
                        #              = odd - (odd-even)*pme
                        pme_b = pm1_t[:, boff:boff + nb].unsqueeze(
                            2).broadcast_to((128, nb, 4))
                        t1_t = alsp.tile([128, NBSB1, 4], dt.float32,
                                         tag="t1")
                        nc.vector.tensor_tensor(
                            t1_t[:, :nb, :], als8_t[:, :nb, 4:8],
                            als8_t[:, :nb, 0:4], Alu.subtract)
                        nc.vector.tensor_tensor(
                            t1_t[:, :nb, :], t1_t[:, :nb, :], pme_b,
                            Alu.mult)
                        nc.vector.tensor_tensor(
                            als_t[:, :nb, :], als8_t[:, :nb, 4:8],
                            t1_t[:, :nb, :], Alu.subtract)
                        off = 0
                        for gi, g in enumerate(sb):
                            D = D1[g]
                            if L1STEP < 3:
                                off += D
                                continue
                            logit_t = smallp.tile([128, Dmax1, 4], dt.float32,
                                                  tag="lg")
                            exb_t = smallp.tile([128, Dmax1, 4], dt.bfloat16,
                                                tag="exb")
                            exe_t = smallp.tile([128, Dmax1, 4], dt.bfloat16,
                                                tag="exe")
                            exo_t = smallp.tile([128, Dmax1, 4], dt.bfloat16,
                                                tag="exo")
                            den_t = smallp.tile([128, 4], dt.float32,
                                                tag="dn")
                            rec_t = smallp.tile([128, 4], dt.float32,
                                                tag="rc")
                            ald_ap = ald1_t[:, g, :]
                            nc.vector.scalar_tensor_tensor(
                                logit_t[:, :D, :], als_t[:, off:off + D, :],
                                0.0,
                                ald_ap.unsqueeze(1).broadcast_to(
                                    (128, D, 4)), Alu.add, Alu.add)
                            nc.vector.scalar_tensor_tensor(
                                logit_t[:, :D, :], logit_t[:, :D, :], NEG,
                                logit_t[:, :D, :], Alu.mult, Alu.max)
                            nc.scalar.activation(exb_t[:, :D, :],
                                                 logit_t[:, :D, :], Act.Exp)
                            nc.vector.tensor_reduce(
                                den_t[:], exb_t[:, :D, :].transpose([0, 2, 1]),
                                axis=Axis.X, op=Alu.add)
                            nc.vector.reciprocal(rec_t[:], den_t[:])
                            pmg = pm1_t[:, boff + off:boff + off + D]
                            pmg_b = pmg.unsqueeze(2).broadcast_to((128, D, 4))
                            nc.vector.tensor_tensor(
                                exe_t[:, :D, :], exb_t[:, :D, :], pmg_b,
                                Alu.mult)
                            nc.vector.tensor_tensor(
                                exo_t[:, :D, :], exb_t[:, :D, :],
                                exe_t[:, :D, :], Alu.subtract)
                            if L1STEP < 4:
                                off += D
                                continue
                            h_e = gb_t[:, off:off + D, 0:128].rearrange(
                                "p b (h c) -> p b h c", h=4)
                            nc.vector.tensor_tensor(
                                h_e, h_e,
                                exe_t[:, :D, :].unsqueeze(3).broadcast_to(
                                    (128, D, 4, HID)), Alu.mult)
                            h_o = gb_t[:, off:off + D, 128:256].rearrange(
                                "p b (h c) -> p b h c", h=4)
                            nc.vector.tensor_tensor(
                                h_o, h_o,
                                exo_t[:, :D, :].unsqueeze(3).broadcast_to(
                                    (128, D, 4, HID)), Alu.mult)
                            if L1STEP < 5:
                                off += D
                                continue
                            agg = aggp.tile([128, 128], dt.float32, tag="agg")
                            for bi in range(2 * D):
                                rhs = gb_t[:, off + bi // 2,
                                           (bi % 2) * 128:(bi % 2 + 1) * 128]
                                nc.tensor.matmul(agg[:], id_t[:], rhs,
                                                 start=(bi == 0),
                                                 stop=(bi == 2 * D - 1))
                            scaled_t = epip.tile([128, 128], dt.float32,
                                                 tag="sd")
                            nc.vector.tensor_tensor(
                                scaled_t[:].rearrange("p (h c) -> p h c", h=4),
                                agg[:].rearrange("p (h c) -> p h c", h=4),
                                rec_t[:].unsqueeze(2).broadcast_to(
                                    (128, 4, HID)), Alu.mult)
                            if HASB1:
                                nc.vector.tensor_tensor(
                                    scaled_t[:], scaled_t[:], b1_t[:],
                                    Alu.add)
                            tmp_t = epip.tile([128, 128], dt.float32,
                                              tag="tm")
                            nc.scalar.activation(tmp_t[:], scaled_t[:],
                                                 Act.Relu, scale=-1.0)
                            nc.scalar.activation(tmp_t[:], tmp_t[:], Act.Exp,
                                                 scale=-1.0)
                            nc.vector.scalar_tensor_tensor(
                                elu_all[:, g, :], tmp_t[:], -1.0, scaled_t[:],
                                Alu.add, Alu.max)
                            off += D
                        # ---- pass 2 for this superblock's groups
                        if L1STEP >= 5:
                            for g in sb:
                                psT = psTp.tile([128, 128], dt.bfloat16,
                                                tag="pt")
                                nc.tensor.transpose(psT[:], elu_all[:, g, :],
                                                    id_t[:])
                                eluT_t = epip.tile([128, 128], dt.bfloat16,
                                                   tag="et")
                                nc.scalar.activation(eluT_t[:], psT[:],
                                                     Act.Copy)
                                ps2 = ps2p.tile([128, 34], dt.float32,
                                                tag="p2")
                                nc.tensor.matmul(ps2[:], eluT_t[:], w2_t[:],
                                                 start=True, stop=True)
                                if g % 2 == 0:
                                    nc.scalar.activation(scat_t[:, g, 0:34],
                                                         ps2[:], Act.Copy)
                                else:
                                    nc.vector.tensor_copy(scat_t[:, g, 0:34],
                                                          ps2[:])
                    if L1STEP >= 6:
                        nc.gpsimd.dma_scatter_add(
                            h2sh[0:S + 1, :], scat_t[:], scat1_t[:],
                            128 * NG1, 128 * NG1, 64,
                            single_packet=False, queue_num=nextq())
                        # pad slots scatter garbage into the trash row; zero
                        # it before the L2 ald extraction reads it
                        zt = scatp.tile([1, 64], dt.float32, tag="zt")
                        nc.vector.memset(zt[:], 0.0)
                        nc.sync.dma_start(h2sh[S:S + 1, :], zt[:])
            if PHASES >= 3:
                # ---------------- exchange ----------------
                nc.gpsimd.collective_compute(
                    "AllGather", mybir.AluOpType.bypass, replica_groups=rg,
                    ins=[h2sh[0:S, :]], outs=[table2[1:N + 1, :]])

            if PHASES >= 4:
                # ---------------- phase L2: edges + pool ----------------
                # preloads below overlap the AllGather
                nc.sync.dma_start(i2_all[:], t_i2[:])
                nc.sync.dma_start(pm2_t[:], t_pm2[:])
                with tc.tile_pool(name="aldtmp2", bufs=1) as atp:
                    atmp = atp.tile([128, NG2, 64], dt.float32)
                    nc.sync.dma_start(
                        atmp[:],
                        h2sh[0:128 * NG2, :].rearrange("(b p) e -> p b e",
                                                       p=128))
                    nc.vector.tensor_copy(ald2_t[:], atmp[:, :, 33:34])
                tab2p = table2[:, :].rearrange("(a h) c -> a (h c)", h=2)
                Dmax2 = max(D2)
                NBSB2 = max(sum(D2[g] for g in sb) for sb in SB2)
                with (
                    tc.tile_pool(name="gath2", bufs=4) as gathp,
                    tc.tile_pool(name="small2", bufs=3) as smallp,
                    tc.tile_pool(name="epi2", bufs=3) as epip,
                    tc.tile_pool(name="agg2", bufs=2, space="PSUM") as aggp,
                    tc.tile_pool(name="poolps", bufs=1,
                                 space="PSUM") as poolpp,
                    tc.tile_pool(name="mp2", bufs=3) as mpp,
                ):
                    poolps = poolpp.tile([HID, GPOOL], dt.float32)
                    h2p_all = mpp.tile([128, NG2, HID], dt.bfloat16,
                                       tag="h2pall", bufs=1)
                    for sb in SB2:
                        g0 = sb[0]
                        nb = sum(D2[g] for g in sb)
                        boff = o2[g0]
                        gb_t = gathp.tile([128, NBSB2, 128], dt.float32,
                                          tag="gb")
                        nc.gpsimd.dma_gather(
                            gb_t[:, :nb, :], tab2p,
                            i2_all[:, 8 * boff:8 * (boff + nb)],
                            128 * nb, 128 * nb, 128, single_packet=False,
                            queue_num=nextq())
                        off = 0
                        for gi, g in enumerate(sb):
                            D = D2[g]
                            logit_t = smallp.tile([128, Dmax2, 1], dt.float32,
                                                  tag="lg")
                            t2_t = smallp.tile([128, Dmax2, 1], dt.float32,
                                               tag="t2")
                            exf_t = smallp.tile([128, Dmax2, 1], dt.float32,
                                                tag="exf")
                            exe_t = smallp.tile([128, Dmax2, 1], dt.float32,
                                                tag="exe")
                            exo_t = smallp.tile([128, Dmax2, 1], dt.float32,
                                                tag="exo")
                            den_t = smallp.tile([128, 1], dt.float32,
                                                tag="dn")
                            rec_t = smallp.tile([128, 1], dt.float32,
                                                tag="rc")
                            ald_ap = ald2_t[:, g, :]
                            pmg = pm2_t[:, boff + off:boff + off + D]
                            pmg_b = pmg.unsqueeze(2)
                            # als = odd - (odd-even)*pme
                            nc.vector.tensor_tensor(
                                t2_t[:, :D, :],
                                gb_t[:, off:off + D, 96:97],
                                gb_t[:, off:off + D, 32:33], Alu.subtract)
                            nc.vector.tensor_tensor(
                                t2_t[:, :D, :], t2_t[:, :D, :], pmg_b,
                                Alu.mult)
                            nc.vector.tensor_tensor(
                                logit_t[:, :D, :],
                                gb_t[:, off:off + D, 96:97],
                                t2_t[:, :D, :], Alu.subtract)
                            nc.vector.tensor_scalar(
                                logit_t[:, :D, :], logit_t[:, :D, :],
                                ald_ap, None, Alu.add)
                            nc.vector.scalar_tensor_tensor(
                                logit_t[:, :D, :], logit_t[:, :D, :], NEG,
                                logit_t[:, :D, :], Alu.mult, Alu.max)
                            nc.scalar.activation(exf_t[:, :D, :],
                                                 logit_t[:, :D, :], Act.Exp)
                            nc.vector.tensor_reduce(
                                den_t[:], exf_t[:, :D, :].transpose([0, 2, 1]),
                                axis=Axis.X, op=Alu.add)
                            nc.vector.reciprocal(rec_t[:], den_t[:])
                            nc.vector.tensor_tensor(
                                exe_t[:, :D, :], exf_t[:, :D, :], pmg_b,
                                Alu.mult)
                            nc.vector.tensor_tensor(
                                exo_t[:, :D, :], exf_t[:, :D, :],
                                exe_t[:, :D, :], Alu.subtract)
                            exh_t = smallp.tile([128, Dmax2, 2, HID],
                                                dt.bfloat16, tag="exh")
                            nc.vector.tensor_tensor(
                                exh_t[:, :D, 0, :],
                                gb_t[:, off:off + D, 0:HID],
                                exe_t[:, :D, :].broadcast_to(
                                    (128, D, HID)), Alu.mult)
                            nc.vector.tensor_tensor(
                                exh_t[:, :D, 1, :],
                                gb_t[:, off:off + D, 64:64 + HID],
                                exo_t[:, :D, :].broadcast_to(
                                    (128, D, HID)), Alu.mult)
                            agg = aggp.tile([128, HID], dt.float32, tag="agg")
                            for bi in range(2 * D):
                                nc.tensor.matmul(
                                    agg[:], id_t[:],
                                    exh_t[:, bi // 2, bi % 2, :],
                                    start=(bi == 0),
                                    stop=(bi == 2 * D - 1))
                            scaled_t = epip.tile([128, HID], dt.float32,
                                                 tag="sd")
                            nc.vector.tensor_scalar(scaled_t[:], agg[:],
                                                    rec_t[:], None, Alu.mult)
                            if HASB2:
                                nc.vector.tensor_tensor(
                                    scaled_t[:], scaled_t[:], b2_t[:],
                                    Alu.add)
                            tmp_t = epip.tile([128, HID], dt.float32,
                                              tag="tm")
                            nc.scalar.activation(tmp_t[:], scaled_t[:],
                                                 Act.Relu, scale=-1.0)
                            nc.scalar.activation(tmp_t[:], tmp_t[:], Act.Exp,
                                                 scale=-1.0)
                            nc.vector.scalar_tensor_tensor(
                                h2p_all[:, g, :], tmp_t[:], -1.0, scaled_t[:],
                                Alu.add, Alu.max)
                            mp_t = mpp.tile([128, GPOOL], dt.bfloat16,
                                            tag="mp")
                            nc.sync.dma_start(
                                mp_t[:], t_mpool[g * 128:(g + 1) * 128, :])
                            nc.tensor.matmul(poolps[:], h2p_all[:, g, :],
                                             mp_t[:], start=(g == 0),
                                             stop=(g == NG2 - 1))
                            off += D
                    # ------------- pool + final linear -------------
                    with tc.tile_pool(name="fin", bufs=1) as finp, \
                            tc.tile_pool(name="finps", bufs=1,
                                         space="PSUM") as fpp:
                        poolsb = finp.tile([HID, GPOOL], dt.float32)
                        nc.vector.tensor_copy(poolsb[:], poolps[:])
                        nc.sync.dma_start(cc_in[:, :], poolsb[:])
                        nc.gpsimd.collective_compute(
                            "AllReduce", Alu.add, replica_groups=rg,
                            ins=[cc_in[:, :]], outs=[cc_out[:, :]])
                        psum_t = finp.tile([HID, GPOOL], dt.float32)
                        nc.sync.dma_start(psum_t[:], cc_out[:, :])
                        mean_t = finp.tile([HID, GPOOL], dt.float32)
                        nc.vector.tensor_tensor(
                            mean_t[:], psum_t[:],
                            rc_t[:], Alu.mult)
                        psO = fpp.tile([GPOOL, OUT], dt.float32)
                        nc.tensor.matmul(psO[:], mean_t[:], wl_t[:],
                                         start=True, stop=True)
                        out_t = finp.tile([GPOOL, OUT], dt.float32)
                        if HASBL:
                            nc.vector.tensor_tensor(out_t[:], psO[:], bl_t[:],
                                                    Alu.add)
                        else:
                            nc.vector.tensor_copy(out_t[:], psO[:])
                        nc.sync.dma_start(t_out[:, :], out_t[:])
            if PHASES < 4:
                with tc.tile_pool(name='dummy', bufs=1) as dp:
                    d = dp.tile([GPOOL, OUT], dt.float32)
                    nc.vector.memset(d[:], 0.0)
                    nc.sync.dma_start(t_out[:, :], d[:])

    nc.compile()
    return nc


def core_inputs(prep, c):
    cd = prep["cores"][c]
    sc_D1, sc_D2 = prep["D1"], prep["D2"]
    NB1, NB2 = sum(sc_D1), sum(sc_D2)
    n1 = max(8 * NB1, 8)
    n2 = max(8 * NB2, 8)

    def padcols(a, cols, dtype):
        if a.shape[1] == cols:
            return np.ascontiguousarray(a)
        out = np.zeros((a.shape[0], cols), dtype)
        out[:, :a.shape[1]] = a
        return out

    return dict(
        xT=np.ascontiguousarray(cd["xT"]),
        w1=prep["W1"], w2ext=prep["W2ext"], wl=prep["Wl"],
        a1x=prep["a1x"],
        b1=prep["b1"], b2=prep["b2"], bl=prep["bl"], rcnt=prep["rcnt"],
        patch1=prep["patch1"], patch2=prep["patch2"], ident=prep["ident"],
        mpool=np.ascontiguousarray(cd["mpool"]),
        idx1=padcols(cd["w_idx1"], n1, np.int16),
        idx2=padcols(cd["w_idx2"], n2, np.int16),
        pme1=padcols(cd["pme1"], max(NB1, 1), bf16),
        pme2=padcols(cd["pme2"], max(NB2, 1), bf16),
        scat1=cd["w_scat1"],
    )


_CACHE = {}


def kernel(**inputs):
    from concourse.bass_utils import run_bass_kernel_spmd

    inputs = {k: np.asarray(v) for k, v in inputs.items()}
    prep = host_prep(**inputs)
    sc = make_sched(prep)
    sc["D1"] = prep["D1"]
    sc["D2"] = prep["D2"]
    key = str(sc)
    if key not in _CACHE:
        _CACHE[key] = build_bass(sc)
    nc = _CACHE[key]
    in_maps = [core_inputs(prep, c) for c in range(NCORES)]
    res = run_bass_kernel_spmd(nc, in_maps, list(range(NCORES)))
    return np.asarray(res.results[0]["out"], np.float32)


# revision 27
# speedup vs baseline: 1.5546x; 1.0271x over previous
# Self-contained 8-core Trainium2 Bass kernel for the 2-layer GAT + mean-pool
# problem (nn_GAT_83820581749190).
#
# Sharding: destination nodes (and all their incident edges) are partitioned
# across the 8 cores, so each layer's attention softmax and aggregation
# complete locally per core. Each core builds a replicated layer-1 feature
# table h1 (bf16, 256-byte logical rows) in HBM with a replicated x @ W1
# matmul, then edge-gathers PAIRS of rows (512B per descriptor, index =
# row//2, int16-safe) with the GPSIMD dma_gather custom op; host-precomputed
# parity masks select the correct half downstream. Attention logits are
# computed on-chip (DVE dot with a_src/a_dst), the edge softmax runs without
# segment-max (logits are small; pad slots use a patch row whose h gives
# al_src=-100), and aggregation is identity-matmul PSUM accumulation
# (destinations on partitions via degree-bucketed groups of 128).
# Layer-2 features are exchanged with an AllGather; the same paired-row
# gather runs against the fp32 layer-2 table; mean-pool is a matmul against
# a host-built one-hot graph matrix plus a tiny AllReduce.
import numpy as np
import ml_dtypes

N = 50000
E = 800000
IN = 128
HID = 32
HEADS = 4
OUT = 10
GPOOL = 64
NEG = 0.2
NCORES = 8
S = N // NCORES
SPECIAL1 = N          # layer-1 patch row (h chosen so h . a_src = -100)
SPECIAL2 = 0          # layer-2 patch row (al_src column = -100)
SPECIAL_ALS = -100.0
SB_BLOCK_BUDGET = 24  # max gather blocks per superblock
XCHUNK = 512
PHASES = 99
L1STEP = 99

bf16 = ml_dtypes.bfloat16


def _ceil_to(v, m):
    return (v + m - 1) // m * m


# ======================= host prep =========================================

def _build_layer(src, dstl):
    deg = np.bincount(dstl, minlength=S)
    P = np.argsort(-deg, kind="stable")
    Ppos = np.empty(S, np.int64)
    Ppos[P] = np.arange(S)
    ng = (S + 127) // 128
    D = np.zeros(ng, np.int64)
    dp = deg[P]
    for g in range(ng):
        D[g] = dp[g * 128:(g + 1) * 128].max()
    assert (D > 0).all()
    return dict(src=src, dstl=dstl, deg=deg, P=P, Ppos=Ppos, D=D)


def _emit_slots(l, Dg, row_of_src, special_row):
    """Per group g: rows[g] [D[g],128] of table ROW ids (special_row pads),
    plus slot2cmp mapping output slots -> compacted dst ids."""
    NG = len(Dg)
    Ppos = l["Ppos"]
    nreal = S
    slot2cmp = np.full(NG * 128, -1, np.int64)
    slot2cmp[:nreal] = np.arange(nreal)
    rows = [np.full((int(Dg[g]), 128), special_row, np.int64)
            for g in range(NG)]
    slot_of_edge = Ppos[l["dstl"]]
    order = np.argsort(slot_of_edge, kind="stable")
    so = slot_of_edge[order]
    sr = row_of_src[l["src"][order]]
    jj = np.arange(len(so)) - np.searchsorted(so, so, side="left")
    gg, kk = so // 128, so % 128
    for g in range(NG):
        sel = gg == g
        if sel.any():
            rows[g][jj[sel], kk[sel]] = sr[sel]
    return rows, slot2cmp


def _wrap16(idx):
    """[n] -> [128, n//16] int16: idx i at [i%16, i//16], replicated x8."""
    n = len(idx)
    assert n % 16 == 0
    w = np.ascontiguousarray(np.asarray(idx).reshape(n // 16, 16).T)
    w = w.astype(np.int16)
    return np.tile(w, (8, 1))


def _wrap_rows(rows_arrs):
    """idx stream (row//2) wrapped, plus even-parity masks [128, NB]."""
    idx_segs = []
    pme_segs = []
    for a in rows_arrs:
        if a.size:
            assert (a // 2 <= 32767).all()
            idx_segs.append(_wrap16((a // 2).reshape(-1)))
            pme_segs.append(np.ascontiguousarray((1 - (a % 2)).T))
    w_idx = (np.concatenate(idx_segs, axis=1) if idx_segs
             else np.zeros((128, 0), np.int16))
    pme = (np.concatenate(pme_segs, axis=1).astype(np.float32) if pme_segs
           else np.zeros((128, 0), np.float32))
    return w_idx, pme


def host_prep(x, edge_index, batch, W1, a1_src, a1_dst, b1, W2, a2_src, a2_dst,
              b2, Wl, bl):
    x = np.asarray(x, np.float32)
    edge_index = np.asarray(edge_index, np.int64)
    batch = np.asarray(batch, np.int64)
    src_all = np.concatenate([edge_index[0], np.arange(N, dtype=np.int64)])
    dst_all = np.concatenate([edge_index[1], np.arange(N, dtype=np.int64)])
    owner = dst_all // S

    a1_src = np.asarray(a1_src, np.float32)
    a1_dst = np.asarray(a1_dst, np.float32)
    W1 = np.asarray(W1, np.float32)
    W2 = np.asarray(W2, np.float32)
    W2ext = np.concatenate(
        [W2, W2 @ np.asarray(a2_src, np.float32)[0][:, None],
         W2 @ np.asarray(a2_dst, np.float32)[0][:, None]], axis=1)  # [128,34]

    # a1x: [0:128]=a_src flat, [128:256]=a_src flat, [256:384]=a_dst flat
    asf = a1_src.reshape(-1)
    adf = a1_dst.reshape(-1)
    a1x = np.tile(np.concatenate([asf, asf, adf])[None, :], (128, 1))

    # layer-1 patch row: h with h . a_src[h] = -100 for every head
    hp = np.concatenate([SPECIAL_ALS * a1_src[h] / (a1_src[h] ** 2).sum()
                         for h in range(HEADS)])
    assert np.abs(hp).max() < 1e4
    patch1 = np.tile(hp[None, :], (1, 1))

    cores = [dict(c=c) for c in range(NCORES)]
    for cd in cores:
        c = cd["c"]
        m = owner == c
        cd["src"] = src_all[m]
        cd["dstl"] = dst_all[m] - c * S

    # ---------- layer 1 ----------
    for cd in cores:
        c = cd["c"]
        l1 = _build_layer(cd["src"], cd["dstl"])
        pos_of = np.empty(N, np.int64)
        own = np.arange(c * S, (c + 1) * S)
        oth = np.concatenate([np.arange(0, c * S), np.arange((c + 1) * S, N)])
        pos_of[oth] = S + np.arange(N - S)
        pos_of[own] = l1["Ppos"]
        cd["l1"] = l1
        cd["row_of"] = pos_of
    NG1 = max(len(cd["l1"]["D"]) for cd in cores)
    D1 = np.zeros(NG1, np.int64)
    for cd in cores:
        d = cd["l1"]["D"]
        D1[:len(d)] = np.maximum(D1[:len(d)], d)
    for cd in cores:
        cd["rows1"], cd["slot2cmp1"] = _emit_slots(
            cd["l1"], D1, cd["row_of"], SPECIAL1)

    # ---------- layer 2 ----------
    # layer-2 features live in a blocked bf16 table: core c's partition p,
    # group g at flat row (c*128+p)*NG2 + g (64 bf16 each; pairs of flat
    # rows share one 256B gather descriptor)
    for cd in cores:
        cd["l2"] = _build_layer(cd["src"], cd["dstl"])
    NG2 = max(len(cd["l2"]["D"]) for cd in cores)
    D2 = np.zeros(NG2, np.int64)
    for cd in cores:
        d = cd["l2"]["D"]
        D2[:len(d)] = np.maximum(D2[:len(d)], d)
    flat2_of = np.empty(N, np.int64)
    for cd in cores:
        c = cd["c"]
        q = cd["l2"]["Ppos"]
        flat2_of[c * S:(c + 1) * S] = \
            (c * 128 + q % 128) * NG2 + q // 128
    for cd in cores:
        c = cd["c"]
        special2 = (c * 128 + S % 128) * NG2 + S // 128  # own trash row
        cd["rows2"], cd["slot2cmp2"] = _emit_slots(
            cd["l2"], D2, flat2_of, special2)

    # ---------- aux ----------
    cnt = np.bincount(batch, minlength=GPOOL).astype(np.float32)
    recip_cnt = (1.0 / np.maximum(cnt, 1.0)).astype(np.float32)

    XT_COLS = _ceil_to(N + 2, XCHUNK)
    for cd in cores:
        c = cd["c"]
        gids = batch[c * S:(c + 1) * S]
        Mp = np.zeros((NG2 * 128, GPOOL), np.float32)
        s2c = cd["slot2cmp2"]
        real = s2c >= 0
        Mp[np.where(real)[0], gids[cd["l2"]["P"][s2c[real]]]] = 1.0
        cd["mpool"] = Mp.astype(bf16)

        s2c1 = cd["slot2cmp1"]
        tgt = np.full(len(s2c1), S, np.int64)  # trash row for dummy slots
        r1 = s2c1 >= 0
        tgt[r1] = cd["l2"]["Ppos"][cd["l1"]["P"][s2c1[r1]]]

        xt = np.zeros((IN, XT_COLS), np.float32)
        xt[:, cd["row_of"]] = x.T
        cd["xT"] = xt.astype(bf16)

        cd["w_idx1"], cd["pme1"] = _wrap_rows(cd["rows1"])
        cd["w_idx2"], cd["pme2"] = _wrap_rows(cd["rows2"])
        cd["w_scat1"] = _wrap16(tgt)

    # written over the trash row after the scatter: al_src=-100 kills pads
    patch2 = np.zeros((1, 64), np.float32)
    patch2[0, 32] = SPECIAL_ALS

    return dict(cores=cores,
                D1=[int(v) for v in D1], D2=[int(v) for v in D2],
                W1=W1.astype(bf16), W2ext=W2ext.astype(bf16),
                Wl=np.asarray(Wl, np.float32),
                a1x=a1x.astype(bf16),
                b1=np.tile(np.asarray(b1, np.float32).reshape(1, -1),
                           (128, 1)),
                b2=np.tile(np.asarray(b2, np.float32).reshape(1, -1),
                           (128, 1)),
                bl=np.tile(np.asarray(bl, np.float32).reshape(1, -1),
                           (GPOOL, 1)),
                rcnt=np.tile(recip_cnt.reshape(1, -1), (HID, 1)),
                patch1=patch1.astype(bf16), patch2=patch2,
                ident=np.eye(128, dtype=bf16))


def _pack_superblocks(D, budget=SB_BLOCK_BUDGET):
    sbs, cur, tot = [], [], 0
    for g in range(len(D)):
        d = int(D[g])
        if cur and tot + d > budget:
            sbs.append(cur)
            cur, tot = [], 0
        cur.append(g)
        tot += d
    if cur:
        sbs.append(cur)
    return sbs


def make_sched(prep):
    D1, D2 = prep["D1"], prep["D2"]
    return dict(D1=D1, D2=D2,
                SB1=_pack_superblocks(D1), SB2=_pack_superblocks(D2),
                HASB1=bool(np.any(prep["b1"])), HASB2=bool(np.any(prep["b2"])),
                HASBL=bool(np.any(prep["bl"])))


# ======================= bass kernel =======================================

def build_bass(sc):
    import concourse.bacc as bacc
    import concourse.tile as tile
    import concourse.mybir as mybir
    from concourse.library_config import mlp

    dt = mybir.dt
    Alu = mybir.AluOpType
    Act = mybir.ActivationFunctionType
    Axis = mybir.AxisListType

    D1, D2 = sc["D1"], sc["D2"]
    SB1, SB2 = sc["SB1"], sc["SB2"]
    HASB1 = sc.get("HASB1", True)
    HASB2 = sc.get("HASB2", True)
    HASBL = sc.get("HASBL", True)
    NG1, NG2 = len(D1), len(D2)
    XT_COLS = _ceil_to(N + 2, XCHUNK)
    NCHUNK = XT_COLS // XCHUNK
    SH2_ROWS = _ceil_to(S + 2, 128)
    NB1 = sum(D1)
    NB2 = sum(D2)
    o1 = np.concatenate([[0], np.cumsum(D1)]).astype(int)
    o2 = np.concatenate([[0], np.cumsum(D2)]).astype(int)

    nc = bacc.Bacc("TRN2", target_bir_lowering=False, debug=False,
                   num_devices=NCORES, num_swdge_queues=4)

    t_xT = nc.dram_tensor("xT", [IN, XT_COLS], dt.bfloat16,
                          kind="ExternalInput")
    t_w1 = nc.dram_tensor("w1", [IN, IN], dt.bfloat16, kind="ExternalInput")
    t_w2 = nc.dram_tensor("w2ext", [IN, 34], dt.bfloat16,
                          kind="ExternalInput")
    t_wl = nc.dram_tensor("wl", [HID, OUT], dt.float32, kind="ExternalInput")
    t_a1x = nc.dram_tensor("a1x", [128, 384], dt.bfloat16,
                           kind="ExternalInput")
    t_b1 = nc.dram_tensor("b1", [128, HEADS * HID], dt.float32,
                          kind="ExternalInput")
    t_b2 = nc.dram_tensor("b2", [128, HID], dt.float32, kind="ExternalInput")
    t_bl = nc.dram_tensor("bl", [GPOOL, OUT], dt.float32,
                          kind="ExternalInput")
    t_rcnt = nc.dram_tensor("rcnt", [HID, GPOOL], dt.float32,
                            kind="ExternalInput")
    t_patch1 = nc.dram_tensor("patch1", [1, 128], dt.bfloat16,
                              kind="ExternalInput")
    t_patch2 = nc.dram_tensor("patch2", [1, 64], dt.float32,
                              kind="ExternalInput")
    t_ident = nc.dram_tensor("ident", [128, 128], dt.bfloat16,
                             kind="ExternalInput")
    t_mpool = nc.dram_tensor("mpool", [NG2 * 128, GPOOL], dt.bfloat16,
                             kind="ExternalInput")
    n1 = max(8 * NB1, 8)
    n2 = max(8 * NB2, 8)
    t_i1 = nc.dram_tensor("idx1", [128, n1], dt.int16, kind="ExternalInput")
    t_i2 = nc.dram_tensor("idx2", [128, n2], dt.int16, kind="ExternalInput")
    t_pm1 = nc.dram_tensor("pme1", [128, max(NB1, 1)], dt.bfloat16,
                           kind="ExternalInput")
    t_pm2 = nc.dram_tensor("pme2", [128, max(NB2, 1)], dt.float32,
                           kind="ExternalInput")
    t_scat1 = nc.dram_tensor("scat1", [128, 8 * NG1], dt.int16,
                             kind="ExternalInput")
    t_out = nc.dram_tensor("out", [GPOOL, OUT], dt.float32,
                           kind="ExternalOutput")

    rg = [list(range(NCORES))]
    _qc = [0]

    def nextq():
        _qc[0] = (_qc[0] + 1) % 4
        return _qc[0]

    with tile.TileContext(nc) as tc:
        with (
            tc.tile_pool(name="const", bufs=1) as constp,
            tc.tile_pool(name="pre", bufs=1) as prep_pool,
            tc.tile_pool(name="dram", bufs=1, space="DRAM") as dramp,
        ):
            nc.gpsimd.load_library(mlp)

            # logical row-major tables; gathers view them as paired rows
            table1 = dramp.tile([XT_COLS, 128], dt.bfloat16, tag="table1")
            h2b = dramp.tile([128, NG2 * 64], dt.bfloat16, tag="h2b")
            table2b = dramp.tile([128 * NCORES * NG2 * 64], dt.bfloat16,
                                 tag="table2b")
            h2sh = dramp.tile([SH2_ROWS, 64], dt.float32, tag="h2sh")
            cc_in = dramp.tile([HID, GPOOL], dt.float32, tag="ccin")
            cc_out = dramp.tile([HID, GPOOL], dt.float32, tag="ccout")

            w1_t = constp.tile([IN, IN], dt.bfloat16)
            nc.sync.dma_start(w1_t[:], t_w1[:])
            w2_t = constp.tile([IN, 34], dt.bfloat16)
            nc.sync.dma_start(w2_t[:], t_w2[:])
            wl_t = constp.tile([HID, OUT], dt.float32)
            nc.sync.dma_start(wl_t[:], t_wl[:])
            a1x_t = constp.tile([128, 384], dt.bfloat16)
            nc.sync.dma_start(a1x_t[:], t_a1x[:])
            b1_t = constp.tile([128, HEADS * HID], dt.float32)
            nc.sync.dma_start(b1_t[:], t_b1[:])
            b2_t = constp.tile([128, HID], dt.float32)
            nc.sync.dma_start(b2_t[:], t_b2[:])
            bl_t = constp.tile([GPOOL, OUT], dt.float32)
            nc.sync.dma_start(bl_t[:], t_bl[:])
            rc_t = constp.tile([HID, GPOOL], dt.float32)
            nc.sync.dma_start(rc_t[:], t_rcnt[:])
            id_t = constp.tile([128, 128], dt.bfloat16)
            nc.sync.dma_start(id_t[:], t_ident[:])

            # preload all gather indices and parity masks
            i1_all = prep_pool.tile([128, n1], dt.int16)
            nc.sync.dma_start(i1_all[:], t_i1[:])
            pm1_t = prep_pool.tile([128, max(NB1, 1)], dt.bfloat16)
            nc.sync.dma_start(pm1_t[:], t_pm1[:])
            i2_all = prep_pool.tile([128, n2], dt.int16)
            pm2_t = prep_pool.tile([128, max(NB2, 1)], dt.float32)
            scat1_t = prep_pool.tile([128, 8 * NG1], dt.int16)
            nc.sync.dma_start(scat1_t[:], t_scat1[:])
            ald1_t = prep_pool.tile([128, NG1, 4], dt.float32)
            ald2_t = prep_pool.tile([128, NG2, 1], dt.float32)

            # zero the scatter_add target
            with tc.tile_pool(name="zp", bufs=1) as zp:
                z_t = zp.tile([128, SH2_ROWS // 128 * 64], dt.float32)
                nc.vector.memset(z_t[:], 0.0)
                nc.sync.dma_start(
                    h2sh[:, :].rearrange("(p k) e -> p (k e)", p=128), z_t[:])

            # ---------------- phase X: build table1 ----------------
            with (
                tc.tile_pool(name="xload", bufs=3) as xlp,
                tc.tile_pool(name="xout", bufs=3) as xop,
                tc.tile_pool(name="xpsum", bufs=4, space="PSUM") as xpp,
            ):
                for t in range(NCHUNK):
                    xt_t = xlp.tile([IN, XCHUNK], dt.bfloat16, tag="xt")
                    nc.sync.dma_start(xt_t[:],
                                      t_xT[:, t * XCHUNK:(t + 1) * XCHUNK])
                    o_t = xop.tile([128, 4, 128], dt.bfloat16, tag="xo")
                    for k in range(4):
                        ps = xpp.tile([128, 128], dt.float32, tag="xp")
                        nc.tensor.matmul(ps[:], xt_t[:, k * 128:(k + 1) * 128],
                                         w1_t[:], start=True, stop=True)
                        if k % 2 == 0:
                            nc.vector.tensor_copy(o_t[:, k, :], ps[:])
                        else:
                            nc.scalar.activation(o_t[:, k, :], ps[:],
                                                 Act.Copy)
                    nc.sync.dma_start(
                        table1[t * XCHUNK:(t + 1) * XCHUNK, :].rearrange(
                            "(k p) e -> p k e", p=128), o_t[:])
            with tc.tile_pool(name="patchp", bufs=1) as pp:
                p1_t = pp.tile([1, 128], dt.bfloat16)
                nc.sync.dma_start(p1_t[:], t_patch1[:])
                nc.sync.dma_start(table1[SPECIAL1:SPECIAL1 + 1, :],
                                  p1_t[0:1, :])
            p2_t = prep_pool.tile([1, 64], dt.float32)
            nc.sync.dma_start(p2_t[:], t_patch2[:])

            if PHASES >= 2:
                # ---------------- phase L1: edges ----------------
                tab1p = table1[:, :].rearrange("(a h) c -> a (h c)", h=2)
                Dmax1 = max(D1)
                NBSB1 = max(sum(D1[g] for g in sb) for sb in SB1)
                # own-destination ald via DVE dot with a_dst
                with tc.tile_pool(name="aldtmp", bufs=1) as atp:
                    atmp = atp.tile([128, NG1, 128], dt.bfloat16)
                    nc.sync.dma_start(
                        atmp[:],
                        table1[0:128 * NG1, :].rearrange("(b p) e -> p b e",
                                                         p=128))
                    aprod = atp.tile([128, NG1, 128], dt.bfloat16)
                    nc.vector.tensor_tensor(
                        aprod[:], atmp[:],
                        a1x_t[:, 256:384].unsqueeze(1).broadcast_to(
                            (128, NG1, 128)), Alu.mult)
                    nc.vector.tensor_reduce(
                        ald1_t[:],
                        aprod[:].rearrange("p b (h c) -> p b h c", h=4),
                        axis=Axis.X, op=Alu.add)
                with (
                    tc.tile_pool(name="gath1", bufs=4) as gathp,
                    tc.tile_pool(name="als1", bufs=2) as alsp,
                    tc.tile_pool(name="small1", bufs=3) as smallp,
                    tc.tile_pool(name="epi1", bufs=3) as epip,
                    tc.tile_pool(name="scatp", bufs=1) as scatp,
                    tc.tile_pool(name="agg1", bufs=2, space="PSUM") as aggp,
                    tc.tile_pool(name="psT1", bufs=2, space="PSUM") as psTp,
                    tc.tile_pool(name="ps21", bufs=2, space="PSUM") as ps2p,
                ):
                    scat_t = scatp.tile([128, NG1, 64], dt.float32, tag="sc")
                    nc.vector.memset(scat_t[:], 0.0)
                    elu_all = scatp.tile([128, NG1, 128], dt.bfloat16,
                                         tag="eluall")
                    for sb in SB1:
                        g0 = sb[0]
                        nb = sum(D1[g] for g in sb)
                        boff = o1[g0]
                        gb_t = gathp.tile([128, NBSB1, 256],
                                          dt.bfloat16, tag="gb")
                        nc.gpsimd.dma_gather(
                            gb_t[:, :nb, :], tab1p,
                            i1_all[:, 8 * boff:8 * (boff + nb)],
                            128 * nb, 128 * nb, 256,
                            single_packet=False, queue_num=nextq())
                        if L1STEP < 2:
                            continue
                        # al_src for both pair-halves: prod + reduce
                        prod_t = alsp.tile([128, NBSB1, 256], dt.bfloat16,
                                           tag="prod")
                        als8_t = alsp.tile([128, NBSB1, 8], dt.float32,
                                           tag="als8")
                        als_t = alsp.tile([128, NBSB1, 4], dt.float32,
                                          tag="als")
                        nc.vector.tensor_tensor(
                            prod_t[:, :nb, :], gb_t[:, :nb, :],
                            a1x_t[:, 0:256].unsqueeze(1).broadcast_to(
                                (128, nb, 256)), Alu.mult)
                        nc.vector.tensor_reduce(
                            als8_t[:, :nb, :],
                            prod_t[:, :nb, :].rearrange(
                                "p b (j c) -> p b j c", j=8),
                            axis=Axis.X, op=Alu.add)
                        # parity-select: als = even*pme + odd*(1-pme)
                        #              = odd - (odd-even)*pme
                        pme_b = pm1_t[:, boff:boff + nb].unsqueeze(
                            2).broadcast_to((128, nb, 4))
                        t1_t = alsp.tile([128, NBSB1, 4], dt.float32,
                                         tag="t1")
                        nc.vector.tensor_tensor(
                            t1_t[:, :nb, :], als8_t[:, :nb, 4:8],
                            als8_t[:, :nb, 0:4], Alu.subtract)
                        nc.vector.tensor_tensor(
                            t1_t[:, :nb, :], t1_t[:, :nb, :], pme_b,
                            Alu.mult)
                        nc.vector.tensor_tensor(
                            als_t[:, :nb, :], als8_t[:, :nb, 4:8],
                            t1_t[:, :nb, :], Alu.subtract)
                        off = 0
                        for gi, g in enumerate(sb):
                            D = D1[g]
                            if L1STEP < 3:
                                off += D
                                continue
                            logit_t = smallp.tile([128, Dmax1, 4], dt.float32,
                                                  tag="lg")
                            exb_t = smallp.tile([128, Dmax1, 4], dt.bfloat16,
                                                tag="exb")
                            exe_t = smallp.tile([128, Dmax1, 4], dt.bfloat16,
                                                tag="exe")
                            exo_t = smallp.tile([128, Dmax1, 4], dt.bfloat16,
                                                tag="exo")
                            den_t = smallp.tile([128, 4], dt.float32,
                                                tag="dn")
                            rec_t = smallp.tile([128, 4], dt.float32,
                                                tag="rc")
                            ald_ap = ald1_t[:, g, :]
                            nc.vector.scalar_tensor_tensor(
                                logit_t[:, :D, :], als_t[:, off:off + D, :],
                                0.0,
                                ald_ap.unsqueeze(1).broadcast_to(
                                    (128, D, 4)), Alu.add, Alu.add)
                            nc.vector.scalar_tensor_tensor(
                                logit_t[:, :D, :], logit_t[:, :D, :], NEG,
                                logit_t[:, :D, :], Alu.mult, Alu.max)
                            nc.scalar.activation(exb_t[:, :D, :],
                                                 logit_t[:, :D, :], Act.Exp)
                            nc.vector.tensor_reduce(
                                den_t[:], exb_t[:, :D, :].transpose([0, 2, 1]),
                                axis=Axis.X, op=Alu.add)
                            nc.vector.reciprocal(rec_t[:], den_t[:])
                            pmg = pm1_t[:, boff + off:boff + off + D]
                            pmg_b = pmg.unsqueeze(2).broadcast_to((128, D, 4))
                            nc.vector.tensor_tensor(
                                exe_t[:, :D, :], exb_t[:, :D, :], pmg_b,
                                Alu.mult)
                            nc.vector.tensor_tensor(
                                exo_t[:, :D, :], exb_t[:, :D, :],
                                exe_t[:, :D, :], Alu.subtract)
                            if L1STEP < 4:
                                off += D
                                continue
                            h_e = gb_t[:, off:off + D, 0:128].rearrange(
                                "p b (h c) -> p b h c", h=4)
                            nc.vector.tensor_tensor(
                                h_e, h_e,
                                exe_t[:, :D, :].unsqueeze(3).broadcast_to(
                                    (128, D, 4, HID)), Alu.mult)
                            h_o = gb_t[:, off:off + D, 128:256].rearrange(
                                "p b (h c) -> p b h c", h=4)
                            nc.vector.tensor_tensor(
                                h_o, h_o,
                                exo_t[:, :D, :].unsqueeze(3).broadcast_to(
                                    (128, D, 4, HID)), Alu.mult)
                            if L1STEP < 5:
                                off += D
                                continue
                            agg = aggp.tile([128, 128], dt.float32, tag="agg")
                            for bi in range(2 * D):
                                rhs = gb_t[:, off + bi // 2,
                                           (bi % 2) * 128:(bi % 2 + 1) * 128]
                                nc.tensor.matmul(agg[:], id_t[:], rhs,
                                                 start=(bi == 0),
                                                 stop=(bi == 2 * D - 1))
                            scaled_t = epip.tile([128, 128], dt.float32,
                                                 tag="sd")
                            nc.vector.tensor_tensor(
                                scaled_t[:].rearrange("p (h c) -> p h c", h=4),
                                agg[:].rearrange("p (h c) -> p h c", h=4),
                                rec_t[:].unsqueeze(2).broadcast_to(
                                    (128, 4, HID)), Alu.mult)
                            if HASB1:
                                nc.vector.tensor_tensor(
                                    scaled_t[:], scaled_t[:], b1_t[:],
                                    Alu.add)
                            tmp_t = epip.tile([128, 128], dt.float32,
                                              tag="tm")
                            nc.scalar.activation(tmp_t[:], scaled_t[:],
                                                 Act.Relu, scale=-1.0)
                            nc.scalar.activation(tmp_t[:], tmp_t[:], Act.Exp,
                                                 scale=-1.0)
                            nc.vector.scalar_tensor_tensor(
                                elu_all[:, g, :], tmp_t[:], -1.0, scaled_t[:],
                                Alu.add, Alu.max)
                            off += D
                        # ---- pass 2 for this superblock's groups
                        if L1STEP >= 5:
                            for g in sb:
                                psT = psTp.tile([128, 128], dt.bfloat16,
                                                tag="pt")
                                nc.tensor.transpose(psT[:], elu_all[:, g, :],
                                                    id_t[:])
                                eluT_t = epip.tile([128, 128], dt.bfloat16,
                                                   tag="et")
                                nc.scalar.activation(eluT_t[:], psT[:],
                                                     Act.Copy)
                                ps2 = ps2p.tile([128, 34], dt.float32,
                                                tag="p2")
                                nc.tensor.matmul(ps2[:], eluT_t[:], w2_t[:],
                                                 start=True, stop=True)
                                if g % 2 == 0:
                                    nc.scalar.activation(scat_t[:, g, 0:34],
                                                         ps2[:], Act.Copy)
                                else:
                                    nc.vector.tensor_copy(scat_t[:, g, 0:34],
                                                          ps2[:])
                    if L1STEP >= 6:
                        nc.gpsimd.dma_scatter_add(
                            h2sh[0:S + 1, :], scat_t[:], scat1_t[:],
                            128 * NG1, 128 * NG1, 64,
                            single_packet=False, queue_num=nextq())
                        # pad slots scatter garbage into the trash row;
                        # overwrite with the al_src=-100 pad row before the
                        # L2 extraction reads it
                        nc.sync.dma_start(h2sh[S:S + 1, :], p2_t[0:1, :])
            if PHASES >= 3:
                # ---------------- exchange ----------------
                # pack own rows to blocked bf16 (also yields the ald column),
                # then AllGather the compact table
                with tc.tile_pool(name="aldtmp2", bufs=1) as atp:
                    atmp = atp.tile([128, NG2, 64], dt.float32)
                    nc.sync.dma_start(
                        atmp[:],
                        h2sh[0:128 * NG2, :].rearrange("(b p) e -> p b e",
                                                       p=128))
                    nc.vector.tensor_copy(ald2_t[:], atmp[:, :, 33:34])
                    atmpb = atp.tile([128, NG2, 64], dt.bfloat16)
                    nc.scalar.activation(atmpb[:], atmp[:], Act.Copy)
                    nc.sync.dma_start(
                        h2b[:, :], atmpb[:].rearrange("p b e -> p (b e)"))
                nc.gpsimd.collective_compute(
                    "AllGather", mybir.AluOpType.bypass, replica_groups=rg,
                    ins=[h2b[:, :]],
                    outs=[table2b[:].rearrange("(r x) -> r x",
                                               r=128 * NCORES)])

            if PHASES >= 4:
                # ---------------- phase L2: edges + pool ----------------
                # preloads below overlap the AllGather
                nc.sync.dma_start(i2_all[:], t_i2[:])
                nc.sync.dma_start(pm2_t[:], t_pm2[:])
                tab2p = table2b[:].rearrange("(y c) -> y c", c=128)
                Dmax2 = max(D2)
                NBSB2 = max(sum(D2[g] for g in sb) for sb in SB2)
                with (
                    tc.tile_pool(name="gath2", bufs=4) as gathp,
                    tc.tile_pool(name="small2", bufs=3) as smallp,
                    tc.tile_pool(name="epi2", bufs=3) as epip,
                    tc.tile_pool(name="agg2", bufs=2, space="PSUM") as aggp,
                    tc.tile_pool(name="poolps", bufs=1,
                                 space="PSUM") as poolpp,
                    tc.tile_pool(name="mp2", bufs=3) as mpp,
                ):
                    poolps = poolpp.tile([HID, GPOOL], dt.float32)
                    h2p_all = mpp.tile([128, NG2, HID], dt.bfloat16,
                                       tag="h2pall", bufs=1)
                    for sb in SB2:
                        g0 = sb[0]
                        nb = sum(D2[g] for g in sb)
                        boff = o2[g0]
                        gb_t = gathp.tile([128, NBSB2, 128], dt.bfloat16,
                                          tag="gb")
                        nc.gpsimd.dma_gather(
                            gb_t[:, :nb, :], tab2p,
                            i2_all[:, 8 * boff:8 * (boff + nb)],
                            128 * nb, 128 * nb, 128, single_packet=False,
                            queue_num=nextq())
                        off = 0
                        for gi, g in enumerate(sb):
                            D = D2[g]
                            logit_t = smallp.tile([128, Dmax2, 1], dt.float32,
                                                  tag="lg")
                            t2_t = smallp.tile([128, Dmax2, 1], dt.float32,
                                               tag="t2")
                            exf_t = smallp.tile([128, Dmax2, 1], dt.float32,
                                                tag="exf")
                            exe_t = smallp.tile([128, Dmax2, 1], dt.float32,
                                                tag="exe")
                            exo_t = smallp.tile([128, Dmax2, 1], dt.float32,
                                                tag="exo")
                            den_t = smallp.tile([128, 1], dt.float32,
                                                tag="dn")
                            rec_t = smallp.tile([128, 1], dt.float32,
                                                tag="rc")
                            ald_ap = ald2_t[:, g, :]
                            pmg = pm2_t[:, boff + off:boff + off + D]
                            pmg_b = pmg.unsqueeze(2)
                            # als = odd - (odd-even)*pme
                            nc.vector.tensor_tensor(
                                t2_t[:, :D, :],
                                gb_t[:, off:off + D, 96:97],
                                gb_t[:, off:off + D, 32:33], Alu.subtract)
                            nc.vector.tensor_tensor(
                                t2_t[:, :D, :], t2_t[:, :D, :], pmg_b,
                                Alu.mult)
                            nc.vector.tensor_tensor(
                                logit_t[:, :D, :],
                                gb_t[:, off:off + D, 96:97],
                                t2_t[:, :D, :], Alu.subtract)
                            nc.vector.tensor_scalar(
                                logit_t[:, :D, :], logit_t[:, :D, :],
                                ald_ap, None, Alu.add)
                            nc.vector.scalar_tensor_tensor(
                                logit_t[:, :D, :], logit_t[:, :D, :], NEG,
                                logit_t[:, :D, :], Alu.mult, Alu.max)
                            nc.scalar.activation(exf_t[:, :D, :],
                                                 logit_t[:, :D, :], Act.Exp)
                            nc.vector.tensor_reduce(
                                den_t[:], exf_t[:, :D, :].transpose([0, 2, 1]),
                                axis=Axis.X, op=Alu.add)
                            nc.vector.reciprocal(rec_t[:], den_t[:])
                            nc.vector.tensor_tensor(
                                exe_t[:, :D, :], exf_t[:, :D, :], pmg_b,
                                Alu.mult)
                            nc.vector.tensor_tensor(
                                exo_t[:, :D, :], exf_t[:, :D, :],
                                exe_t[:, :D, :], Alu.subtract)
                            exh_t = smallp.tile([128, Dmax2, 2, HID],
                                                dt.bfloat16, tag="exh")
                            nc.vector.tensor_tensor(
                                exh_t[:, :D, 0, :],
                                gb_t[:, off:off + D, 0:HID],
                                exe_t[:, :D, :].broadcast_to(
                                    (128, D, HID)), Alu.mult)
                            nc.vector.tensor_tensor(
                                exh_t[:, :D, 1, :],
                                gb_t[:, off:off + D, 64:64 + HID],
                                exo_t[:, :D, :].broadcast_to(
                                    (128, D, HID)), Alu.mult)
                            agg = aggp.tile([128, HID], dt.float32, tag="agg")
                            for bi in range(2 * D):
                                nc.tensor.matmul(
                                    agg[:], id_t[:],
                                    exh_t[:, bi // 2, bi % 2, :],
                                    start=(bi == 0),
                                    stop=(bi == 2 * D - 1))
                            scaled_t = epip.tile([128, HID], dt.float32,
                                                 tag="sd")
                            nc.vector.tensor_scalar(scaled_t[:], agg[:],
                                                    rec_t[:], None, Alu.mult)
                            if HASB2:
                                nc.vector.tensor_tensor(
                                    scaled_t[:], scaled_t[:], b2_t[:],
                                    Alu.add)
                            tmp_t = epip.tile([128, HID], dt.float32,
                                              tag="tm")
                            nc.scalar.activation(tmp_t[:], scaled_t[:],
                                                 Act.Relu, scale=-1.0)
                            nc.scalar.activation(tmp_t[:], tmp_t[:], Act.Exp,
                                                 scale=-1.0)
                            nc.vector.scalar_tensor_tensor(
                                h2p_all[:, g, :], tmp_t[:], -1.0, scaled_t[:],
                                Alu.add, Alu.max)
                            mp_t = mpp.tile([128, GPOOL], dt.bfloat16,
                                            tag="mp")
                            nc.sync.dma_start(
                                mp_t[:], t_mpool[g * 128:(g + 1) * 128, :])
                            nc.tensor.matmul(poolps[:], h2p_all[:, g, :],
                                             mp_t[:], start=(g == 0),
                                             stop=(g == NG2 - 1))
                            off += D
                    # ------------- pool + final linear -------------
                    with tc.tile_pool(name="fin", bufs=1) as finp, \
                            tc.tile_pool(name="finps", bufs=1,
                                         space="PSUM") as fpp:
                        poolsb = finp.tile([HID, GPOOL], dt.float32)
                        nc.vector.tensor_copy(poolsb[:], poolps[:])
                        nc.sync.dma_start(cc_in[:, :], poolsb[:])
                        nc.gpsimd.collective_compute(
                            "AllReduce", Alu.add, replica_groups=rg,
                            ins=[cc_in[:, :]], outs=[cc_out[:, :]])
                        psum_t = finp.tile([HID, GPOOL], dt.float32)
                        nc.sync.dma_start(psum_t[:], cc_out[:, :])
                        mean_t = finp.tile([HID, GPOOL], dt.float32)
                        nc.vector.tensor_tensor(
                            mean_t[:], psum_t[:],
                            rc_t[:], Alu.mult)
                        psO = fpp.tile([GPOOL, OUT], dt.float32)
                        nc.tensor.matmul(psO[:], mean_t[:], wl_t[:],
                                         start=True, stop=True)
                        out_t = finp.tile([GPOOL, OUT], dt.float32)
                        if HASBL:
                            nc.vector.tensor_tensor(out_t[:], psO[:], bl_t[:],
                                                    Alu.add)
                        else:
                            nc.vector.tensor_copy(out_t[:], psO[:])
                        nc.sync.dma_start(t_out[:, :], out_t[:])
            if PHASES < 4:
                with tc.tile_pool(name='dummy', bufs=1) as dp:
                    d = dp.tile([GPOOL, OUT], dt.float32)
                    nc.vector.memset(d[:], 0.0)
                    nc.sync.dma_start(t_out[:, :], d[:])

    nc.compile()
    return nc


def core_inputs(prep, c):
    cd = prep["cores"][c]
    sc_D1, sc_D2 = prep["D1"], prep["D2"]
    NB1, NB2 = sum(sc_D1), sum(sc_D2)
    n1 = max(8 * NB1, 8)
    n2 = max(8 * NB2, 8)

    def padcols(a, cols, dtype):
        if a.shape[1] == cols:
            return np.ascontiguousarray(a)
        out = np.zeros((a.shape[0], cols), dtype)
        out[:, :a.shape[1]] = a
        return out

    return dict(
        xT=np.ascontiguousarray(cd["xT"]),
        w1=prep["W1"], w2ext=prep["W2ext"], wl=prep["Wl"],
        a1x=prep["a1x"],
        b1=prep["b1"], b2=prep["b2"], bl=prep["bl"], rcnt=prep["rcnt"],
        patch1=prep["patch1"], patch2=prep["patch2"], ident=prep["ident"],
        mpool=np.ascontiguousarray(cd["mpool"]),
        idx1=padcols(cd["w_idx1"], n1, np.int16),
        idx2=padcols(cd["w_idx2"], n2, np.int16),
        pme1=padcols(cd["pme1"], max(NB1, 1), np.float32).astype(bf16),
        pme2=padcols(cd["pme2"], max(NB2, 1), np.float32),
        scat1=cd["w_scat1"],
    )


_CACHE = {}


def kernel(**inputs):
    from concourse.bass_utils import run_bass_kernel_spmd

    inputs = {k: np.asarray(v) for k, v in inputs.items()}
    prep = host_prep(**inputs)
    sc = make_sched(prep)
    sc["D1"] = prep["D1"]
    sc["D2"] = prep["D2"]
    key = str(sc)
    if key not in _CACHE:
        _CACHE[key] = build_bass(sc)
    nc = _CACHE[key]
    in_maps = [core_inputs(prep, c) for c in range(NCORES)]
    res = run_bass_kernel_spmd(nc, in_maps, list(range(NCORES)))
    return np.asarray(res.results[0]["out"], np.float32)


# revision 29
# speedup vs baseline: 1.5956x; 1.0264x over previous
# Self-contained 8-core Trainium2 Bass kernel for the 2-layer GAT + mean-pool
# problem (nn_GAT_83820581749190).
#
# Sharding: destination nodes (and all their incident edges) are partitioned
# across the 8 cores, so each layer's attention softmax and aggregation
# complete locally per core. Each core builds a replicated layer-1 feature
# table h1 (bf16, 256-byte logical rows) in HBM with a replicated x @ W1
# matmul, then edge-gathers PAIRS of rows (512B per descriptor, index =
# row//2, int16-safe) with the GPSIMD dma_gather custom op; host-precomputed
# parity masks select the correct half downstream. Attention logits are
# computed on-chip (DVE dot with a_src/a_dst), the edge softmax runs without
# segment-max (logits are small; pad slots use a patch row whose h gives
# al_src=-100), and aggregation is identity-matmul PSUM accumulation
# (destinations on partitions via degree-bucketed groups of 128).
# Layer-2 features are exchanged with an AllGather; the same paired-row
# gather runs against the fp32 layer-2 table; mean-pool is a matmul against
# a host-built one-hot graph matrix plus a tiny AllReduce.
import numpy as np
import ml_dtypes

N = 50000
E = 800000
IN = 128
HID = 32
HEADS = 4
OUT = 10
GPOOL = 64
NEG = 0.2
NCORES = 8
S = N // NCORES
SPECIAL1 = N          # layer-1 patch row (h chosen so h . a_src = -100)
SPECIAL2 = 0          # layer-2 patch row (al_src column = -100)
SPECIAL_ALS = -100.0
SB_BLOCK_BUDGET = 24  # max gather blocks per superblock
XCHUNK = 512
PHASES = 99
L1STEP = 99

bf16 = ml_dtypes.bfloat16


def _ceil_to(v, m):
    return (v + m - 1) // m * m


# ======================= host prep =========================================

def _build_layer(src, dstl):
    deg = np.bincount(dstl, minlength=S)
    P = np.argsort(-deg, kind="stable")
    Ppos = np.empty(S, np.int64)
    Ppos[P] = np.arange(S)
    ng = (S + 127) // 128
    D = np.zeros(ng, np.int64)
    dp = deg[P]
    for g in range(ng):
        D[g] = dp[g * 128:(g + 1) * 128].max()
    assert (D > 0).all()
    return dict(src=src, dstl=dstl, deg=deg, P=P, Ppos=Ppos, D=D)


def _emit_slots(l, Dg, row_of_src, special_row):
    """Per group g: rows[g] [D[g],128] of table ROW ids (special_row pads),
    plus slot2cmp mapping output slots -> compacted dst ids."""
    NG = len(Dg)
    Ppos = l["Ppos"]
    nreal = S
    slot2cmp = np.full(NG * 128, -1, np.int64)
    slot2cmp[:nreal] = np.arange(nreal)
    rows = [np.full((int(Dg[g]), 128), special_row, np.int64)
            for g in range(NG)]
    slot_of_edge = Ppos[l["dstl"]]
    order = np.argsort(slot_of_edge, kind="stable")
    so = slot_of_edge[order]
    sr = row_of_src[l["src"][order]]
    jj = np.arange(len(so)) - np.searchsorted(so, so, side="left")
    gg, kk = so // 128, so % 128
    for g in range(NG):
        sel = gg == g
        if sel.any():
            rows[g][jj[sel], kk[sel]] = sr[sel]
    return rows, slot2cmp


def _wrap16(idx):
    """[n] -> [128, n//16] int16: idx i at [i%16, i//16], replicated x8."""
    n = len(idx)
    assert n % 16 == 0
    w = np.ascontiguousarray(np.asarray(idx).reshape(n // 16, 16).T)
    w = w.astype(np.int16)
    return np.tile(w, (8, 1))


def _wrap_rows(rows_arrs):
    """idx stream (row//2) wrapped, plus even-parity masks [128, NB]."""
    idx_segs = []
    pme_segs = []
    for a in rows_arrs:
        if a.size:
            assert (a // 2 <= 32767).all()
            idx_segs.append(_wrap16((a // 2).reshape(-1)))
            pme_segs.append(np.ascontiguousarray((1 - (a % 2)).T))
    w_idx = (np.concatenate(idx_segs, axis=1) if idx_segs
             else np.zeros((128, 0), np.int16))
    pme = (np.concatenate(pme_segs, axis=1).astype(np.float32) if pme_segs
           else np.zeros((128, 0), np.float32))
    return w_idx, pme


def host_prep(x, edge_index, batch, W1, a1_src, a1_dst, b1, W2, a2_src, a2_dst,
              b2, Wl, bl):
    x = np.asarray(x, np.float32)
    edge_index = np.asarray(edge_index, np.int64)
    batch = np.asarray(batch, np.int64)
    src_all = np.concatenate([edge_index[0], np.arange(N, dtype=np.int64)])
    dst_all = np.concatenate([edge_index[1], np.arange(N, dtype=np.int64)])
    owner = dst_all // S

    a1_src = np.asarray(a1_src, np.float32)
    a1_dst = np.asarray(a1_dst, np.float32)
    W1 = np.asarray(W1, np.float32)
    W2 = np.asarray(W2, np.float32)
    W2ext = np.concatenate(
        [W2, W2 @ np.asarray(a2_src, np.float32)[0][:, None],
         W2 @ np.asarray(a2_dst, np.float32)[0][:, None]], axis=1)  # [128,34]

    # a1x: [0:128]=a_src flat, [128:256]=a_src flat, [256:384]=a_dst flat
    asf = a1_src.reshape(-1)
    adf = a1_dst.reshape(-1)
    a1x = np.tile(np.concatenate([asf, asf, adf])[None, :], (128, 1))

    # layer-1 patch row: h with h . a_src[h] = -100 for every head
    hp = np.concatenate([SPECIAL_ALS * a1_src[h] / (a1_src[h] ** 2).sum()
                         for h in range(HEADS)])
    assert np.abs(hp).max() < 1e4
    patch1 = np.tile(hp[None, :], (1, 1))

    cores = [dict(c=c) for c in range(NCORES)]
    for cd in cores:
        c = cd["c"]
        m = owner == c
        cd["src"] = src_all[m]
        cd["dstl"] = dst_all[m] - c * S

    # ---------- layer 1 ----------
    for cd in cores:
        c = cd["c"]
        l1 = _build_layer(cd["src"], cd["dstl"])
        pos_of = np.empty(N, np.int64)
        own = np.arange(c * S, (c + 1) * S)
        oth = np.concatenate([np.arange(0, c * S), np.arange((c + 1) * S, N)])
        pos_of[oth] = S + np.arange(N - S)
        pos_of[own] = l1["Ppos"]
        cd["l1"] = l1
        cd["row_of"] = pos_of
    NG1 = max(len(cd["l1"]["D"]) for cd in cores)
    D1 = np.zeros(NG1, np.int64)
    for cd in cores:
        d = cd["l1"]["D"]
        D1[:len(d)] = np.maximum(D1[:len(d)], d)
    for cd in cores:
        cd["rows1"], cd["slot2cmp1"] = _emit_slots(
            cd["l1"], D1, cd["row_of"], SPECIAL1)

    # ---------- layer 2 ----------
    # layer-2 features live in a blocked bf16 table: core c's partition p,
    # group g at flat row (c*128+p)*NG2 + g (64 bf16 each; pairs of flat
    # rows share one 256B gather descriptor)
    for cd in cores:
        cd["l2"] = _build_layer(cd["src"], cd["dstl"])
    NG2 = max(len(cd["l2"]["D"]) for cd in cores)
    D2 = np.zeros(NG2, np.int64)
    for cd in cores:
        d = cd["l2"]["D"]
        D2[:len(d)] = np.maximum(D2[:len(d)], d)
    flat2_of = np.empty(N, np.int64)
    for cd in cores:
        c = cd["c"]
        q = cd["l2"]["Ppos"]
        flat2_of[c * S:(c + 1) * S] = \
            (c * 128 + q % 128) * NG2 + q // 128
    for cd in cores:
        c = cd["c"]
        special2 = (c * 128 + S % 128) * NG2 + S // 128  # own trash row
        cd["rows2"], cd["slot2cmp2"] = _emit_slots(
            cd["l2"], D2, flat2_of, special2)

    # ---------- aux ----------
    cnt = np.bincount(batch, minlength=GPOOL).astype(np.float32)
    recip_cnt = (1.0 / np.maximum(cnt, 1.0)).astype(np.float32)

    XT_COLS = _ceil_to(N + 2, XCHUNK)
    for cd in cores:
        c = cd["c"]
        gids = batch[c * S:(c + 1) * S]
        Mp = np.zeros((NG2 * 128, GPOOL), np.float32)
        s2c = cd["slot2cmp2"]
        real = s2c >= 0
        Mp[np.where(real)[0], gids[cd["l2"]["P"][s2c[real]]]] = 1.0
        cd["mpool"] = Mp.astype(bf16)

        s2c1 = cd["slot2cmp1"]
        tgt = np.full(len(s2c1), S, np.int64)  # trash row for dummy slots
        r1 = s2c1 >= 0
        tgt[r1] = cd["l2"]["Ppos"][cd["l1"]["P"][s2c1[r1]]]

        xt = np.zeros((IN, XT_COLS), np.float32)
        xt[:, cd["row_of"]] = x.T
        cd["xT"] = xt.astype(bf16)

        cd["w_idx1"], cd["pme1"] = _wrap_rows(cd["rows1"])
        cd["w_idx2"], cd["pme2"] = _wrap_rows(cd["rows2"])
        cd["w_scat1"] = _wrap16(tgt)

    # written over the trash row after the scatter: al_src=-100 kills pads
    patch2 = np.zeros((1, 64), np.float32)
    patch2[0, 32] = SPECIAL_ALS

    return dict(cores=cores,
                D1=[int(v) for v in D1], D2=[int(v) for v in D2],
                W1=W1.astype(bf16), W2ext=W2ext.astype(bf16),
                Wl=np.asarray(Wl, np.float32),
                a1x=a1x.astype(bf16),
                b1=np.tile(np.asarray(b1, np.float32).reshape(1, -1),
                           (128, 1)),
                b2=np.tile(np.asarray(b2, np.float32).reshape(1, -1),
                           (128, 1)),
                bl=np.tile(np.asarray(bl, np.float32).reshape(1, -1),
                           (GPOOL, 1)),
                rcnt=np.tile(recip_cnt.reshape(1, -1), (HID, 1)),
                patch1=patch1.astype(bf16), patch2=patch2,
                ident=np.eye(128, dtype=bf16))


def _pack_superblocks(D, budget=SB_BLOCK_BUDGET):
    sbs, cur, tot = [], [], 0
    for g in range(len(D)):
        d = int(D[g])
        if cur and tot + d > budget:
            sbs.append(cur)
            cur, tot = [], 0
        cur.append(g)
        tot += d
    if cur:
        sbs.append(cur)
    return sbs


def make_sched(prep):
    D1, D2 = prep["D1"], prep["D2"]
    return dict(D1=D1, D2=D2,
                SB1=_pack_superblocks(D1), SB2=_pack_superblocks(D2),
                HASB1=bool(np.any(prep["b1"])), HASB2=bool(np.any(prep["b2"])),
                HASBL=bool(np.any(prep["bl"])))


# ======================= bass kernel =======================================

def build_bass(sc):
    import concourse.bacc as bacc
    import concourse.tile as tile
    import concourse.mybir as mybir
    from concourse.library_config import mlp

    dt = mybir.dt
    Alu = mybir.AluOpType
    Act = mybir.ActivationFunctionType
    Axis = mybir.AxisListType

    D1, D2 = sc["D1"], sc["D2"]
    SB1, SB2 = sc["SB1"], sc["SB2"]
    HASB1 = sc.get("HASB1", True)
    HASB2 = sc.get("HASB2", True)
    HASBL = sc.get("HASBL", True)
    NG1, NG2 = len(D1), len(D2)
    XT_COLS = _ceil_to(N + 2, XCHUNK)
    NCHUNK = XT_COLS // XCHUNK
    SH2_ROWS = _ceil_to(S + 2, 128)
    NB1 = sum(D1)
    NB2 = sum(D2)
    o1 = np.concatenate([[0], np.cumsum(D1)]).astype(int)
    o2 = np.concatenate([[0], np.cumsum(D2)]).astype(int)

    nc = bacc.Bacc("TRN2", target_bir_lowering=False, debug=False,
                   num_devices=NCORES, num_swdge_queues=4)

    t_xT = nc.dram_tensor("xT", [IN, XT_COLS], dt.bfloat16,
                          kind="ExternalInput")
    t_w1 = nc.dram_tensor("w1", [IN, IN], dt.bfloat16, kind="ExternalInput")
    t_w2 = nc.dram_tensor("w2ext", [IN, 34], dt.bfloat16,
                          kind="ExternalInput")
    t_wl = nc.dram_tensor("wl", [HID, OUT], dt.float32, kind="ExternalInput")
    t_a1x = nc.dram_tensor("a1x", [128, 384], dt.bfloat16,
                           kind="ExternalInput")
    t_b1 = nc.dram_tensor("b1", [128, HEADS * HID], dt.float32,
                          kind="ExternalInput")
    t_b2 = nc.dram_tensor("b2", [128, HID], dt.float32, kind="ExternalInput")
    t_bl = nc.dram_tensor("bl", [GPOOL, OUT], dt.float32,
                          kind="ExternalInput")
    t_rcnt = nc.dram_tensor("rcnt", [HID, GPOOL], dt.float32,
                            kind="ExternalInput")
    t_patch1 = nc.dram_tensor("patch1", [1, 128], dt.bfloat16,
                              kind="ExternalInput")
    t_patch2 = nc.dram_tensor("patch2", [1, 64], dt.float32,
                              kind="ExternalInput")
    t_ident = nc.dram_tensor("ident", [128, 128], dt.bfloat16,
                             kind="ExternalInput")
    t_mpool = nc.dram_tensor("mpool", [NG2 * 128, GPOOL], dt.bfloat16,
                             kind="ExternalInput")
    n1 = max(8 * NB1, 8)
    n2 = max(8 * NB2, 8)
    t_i1 = nc.dram_tensor("idx1", [128, n1], dt.int16, kind="ExternalInput")
    t_i2 = nc.dram_tensor("idx2", [128, n2], dt.int16, kind="ExternalInput")
    t_pm1 = nc.dram_tensor("pme1", [128, max(NB1, 1)], dt.bfloat16,
                           kind="ExternalInput")
    t_pm2 = nc.dram_tensor("pme2", [128, max(NB2, 1)], dt.float32,
                           kind="ExternalInput")
    t_scat1 = nc.dram_tensor("scat1", [128, 8 * NG1], dt.int16,
                             kind="ExternalInput")
    t_out = nc.dram_tensor("out", [GPOOL, OUT], dt.float32,
                           kind="ExternalOutput")

    rg = [list(range(NCORES))]
    _qc = [0]

    def nextq():
        _qc[0] = (_qc[0] + 1) % 4
        return _qc[0]

    with tile.TileContext(nc) as tc:
        with (
            tc.tile_pool(name="const", bufs=1) as constp,
            tc.tile_pool(name="pre", bufs=1) as prep_pool,
            tc.tile_pool(name="dram", bufs=1, space="DRAM") as dramp,
        ):
            nc.gpsimd.load_library(mlp)

            # logical row-major tables; gathers view them as paired rows
            table1 = dramp.tile([XT_COLS, 128], dt.bfloat16, tag="table1")
            h2b = dramp.tile([128, NG2 * 64], dt.bfloat16, tag="h2b")
            table2b = dramp.tile([128 * NCORES * NG2 * 64], dt.bfloat16,
                                 tag="table2b")
            h2sh = dramp.tile([SH2_ROWS, 64], dt.float32, tag="h2sh")
            cc_in = dramp.tile([HID, GPOOL], dt.float32, tag="ccin")
            cc_out = dramp.tile([HID, GPOOL], dt.float32, tag="ccout")

            w1_t = constp.tile([IN, IN], dt.bfloat16)
            nc.sync.dma_start(w1_t[:], t_w1[:])
            w2_t = constp.tile([IN, 34], dt.bfloat16)
            nc.sync.dma_start(w2_t[:], t_w2[:])
            wl_t = constp.tile([HID, OUT], dt.float32)
            nc.sync.dma_start(wl_t[:], t_wl[:])
            a1x_t = constp.tile([128, 384], dt.bfloat16)
            nc.sync.dma_start(a1x_t[:], t_a1x[:])
            b1_t = constp.tile([128, HEADS * HID], dt.float32)
            nc.sync.dma_start(b1_t[:], t_b1[:])
            b2_t = constp.tile([128, HID], dt.float32)
            nc.sync.dma_start(b2_t[:], t_b2[:])
            bl_t = constp.tile([GPOOL, OUT], dt.float32)
            nc.sync.dma_start(bl_t[:], t_bl[:])
            rc_t = constp.tile([HID, GPOOL], dt.float32)
            nc.sync.dma_start(rc_t[:], t_rcnt[:])
            id_t = constp.tile([128, 128], dt.bfloat16)
            nc.sync.dma_start(id_t[:], t_ident[:])

            # preload all gather indices and parity masks
            i1_all = prep_pool.tile([128, n1], dt.int16)
            nc.sync.dma_start(i1_all[:], t_i1[:])
            pm1_t = prep_pool.tile([128, max(NB1, 1)], dt.bfloat16)
            nc.sync.dma_start(pm1_t[:], t_pm1[:])
            i2_all = prep_pool.tile([128, n2], dt.int16)
            pm2_t = prep_pool.tile([128, max(NB2, 1)], dt.float32)
            scat1_t = prep_pool.tile([128, 8 * NG1], dt.int16)
            nc.sync.dma_start(scat1_t[:], t_scat1[:])
            ald1_t = prep_pool.tile([128, NG1, 4], dt.float32)
            ald2_t = prep_pool.tile([128, NG2, 1], dt.float32)

            # zero the scatter_add target
            with tc.tile_pool(name="zp", bufs=1) as zp:
                z_t = zp.tile([128, SH2_ROWS // 128 * 64], dt.float32)
                nc.vector.memset(z_t[:], 0.0)
                nc.sync.dma_start(
                    h2sh[:, :].rearrange("(p k) e -> p (k e)", p=128), z_t[:])

            # ---------------- phase X: build table1 ----------------
            with (
                tc.tile_pool(name="xload", bufs=3) as xlp,
                tc.tile_pool(name="xout", bufs=3) as xop,
                tc.tile_pool(name="xpsum", bufs=4, space="PSUM") as xpp,
            ):
                for t in range(NCHUNK):
                    xt_t = xlp.tile([IN, XCHUNK], dt.bfloat16, tag="xt")
                    nc.sync.dma_start(xt_t[:],
                                      t_xT[:, t * XCHUNK:(t + 1) * XCHUNK])
                    o_t = xop.tile([128, 4, 128], dt.bfloat16, tag="xo")
                    for k in range(4):
                        ps = xpp.tile([128, 128], dt.float32, tag="xp")
                        nc.tensor.matmul(ps[:], xt_t[:, k * 128:(k + 1) * 128],
                                         w1_t[:], start=True, stop=True)
                        if k % 2 == 0:
                            nc.vector.tensor_copy(o_t[:, k, :], ps[:])
                        else:
                            nc.scalar.activation(o_t[:, k, :], ps[:],
                                                 Act.Copy)
                        gix = 4 * t + k
                        if gix < NG1:
                            # own-row ald = h . a_dst, straight off the PSUM
                            ap_t = xop.tile([128, 128], dt.bfloat16,
                                            tag="apr")
                            nc.vector.tensor_tensor(
                                ap_t[:], ps[:], a1x_t[:, 256:384], Alu.mult)
                            nc.vector.tensor_reduce(
                                ald1_t[:, gix, :],
                                ap_t[:].rearrange("p (h c) -> p h c", h=4),
                                axis=Axis.X, op=Alu.add)
                    nc.sync.dma_start(
                        table1[t * XCHUNK:(t + 1) * XCHUNK, :].rearrange(
                            "(k p) e -> p k e", p=128), o_t[:])
            with tc.tile_pool(name="patchp", bufs=1) as pp:
                p1_t = pp.tile([1, 128], dt.bfloat16)
                nc.sync.dma_start(p1_t[:], t_patch1[:])
                nc.sync.dma_start(table1[SPECIAL1:SPECIAL1 + 1, :],
                                  p1_t[0:1, :])
            p2_t = prep_pool.tile([1, 64], dt.float32)
            nc.sync.dma_start(p2_t[:], t_patch2[:])

            if PHASES >= 2:
                # ---------------- phase L1: edges ----------------
                tab1p = table1[:, :].rearrange("(a h) c -> a (h c)", h=2)
                Dmax1 = max(D1)
                NBSB1 = max(sum(D1[g] for g in sb) for sb in SB1)
                with (
                    tc.tile_pool(name="gath1", bufs=4) as gathp,
                    tc.tile_pool(name="als1", bufs=2) as alsp,
                    tc.tile_pool(name="small1", bufs=3) as smallp,
                    tc.tile_pool(name="epi1", bufs=3) as epip,
                    tc.tile_pool(name="scatp", bufs=1) as scatp,
                    tc.tile_pool(name="agg1", bufs=2, space="PSUM") as aggp,
                    tc.tile_pool(name="psT1", bufs=2, space="PSUM") as psTp,
                    tc.tile_pool(name="ps21", bufs=2, space="PSUM") as ps2p,
                ):
                    scat_t = scatp.tile([128, NG1, 64], dt.float32, tag="sc")
                    nc.vector.memset(scat_t[:], 0.0)
                    elu_all = scatp.tile([128, NG1, 128], dt.bfloat16,
                                         tag="eluall")
                    for sb in SB1:
                        g0 = sb[0]
                        nb = sum(D1[g] for g in sb)
                        boff = o1[g0]
                        gb_t = gathp.tile([128, NBSB1, 256],
                                          dt.bfloat16, tag="gb")
                        nc.gpsimd.dma_gather(
                            gb_t[:, :nb, :], tab1p,
                            i1_all[:, 8 * boff:8 * (boff + nb)],
                            128 * nb, 128 * nb, 256,
                            single_packet=False, queue_num=nextq())
                        if L1STEP < 2:
                            continue
                        # al_src for both pair-halves: prod + reduce
                        prod_t = alsp.tile([128, NBSB1, 256], dt.bfloat16,
                                           tag="prod")
                        als8_t = alsp.tile([128, NBSB1, 8], dt.float32,
                                           tag="als8")
                        als_t = alsp.tile([128, NBSB1, 4], dt.float32,
                                          tag="als")
                        nc.vector.tensor_tensor(
                            prod_t[:, :nb, :], gb_t[:, :nb, :],
                            a1x_t[:, 0:256].unsqueeze(1).broadcast_to(
                                (128, nb, 256)), Alu.mult)
                        nc.vector.tensor_reduce(
                            als8_t[:, :nb, :],
                            prod_t[:, :nb, :].rearrange(
                                "p b (j c) -> p b j c", j=8),
                            axis=Axis.X, op=Alu.add)
                        # parity-select: als = even*pme + odd*(1-pme)
                        #              = odd - (odd-even)*pme
                        pme_b = pm1_t[:, boff:boff + nb].unsqueeze(
                            2).broadcast_to((128, nb, 4))
                        t1_t = alsp.tile([128, NBSB1, 4], dt.float32,
                                         tag="t1")
                        nc.vector.tensor_tensor(
                            t1_t[:, :nb, :], als8_t[:, :nb, 4:8],
                            als8_t[:, :nb, 0:4], Alu.subtract)
                        nc.vector.tensor_tensor(
                            t1_t[:, :nb, :], t1_t[:, :nb, :], pme_b,
                            Alu.mult)
                        nc.vector.tensor_tensor(
                            als_t[:, :nb, :], als8_t[:, :nb, 4:8],
                            t1_t[:, :nb, :], Alu.subtract)
                        off = 0
                        for gi, g in enumerate(sb):
                            D = D1[g]
                            if L1STEP < 3:
                                off += D
                                continue
                            logit_t = smallp.tile([128, Dmax1, 4], dt.float32,
                                                  tag="lg")
                            exb_t = smallp.tile([128, Dmax1, 4], dt.bfloat16,
                                                tag="exb")
                            exe_t = smallp.tile([128, Dmax1, 4], dt.bfloat16,
                                                tag="exe")
                            exo_t = smallp.tile([128, Dmax1, 4], dt.bfloat16,
                                                tag="exo")
                            den_t = smallp.tile([128, 4], dt.float32,
                                                tag="dn")
                            rec_t = smallp.tile([128, 4], dt.float32,
                                                tag="rc")
                            ald_ap = ald1_t[:, g, :]
                            nc.vector.scalar_tensor_tensor(
                                logit_t[:, :D, :], als_t[:, off:off + D, :],
                                0.0,
                                ald_ap.unsqueeze(1).broadcast_to(
                                    (128, D, 4)), Alu.add, Alu.add)
                            nc.vector.scalar_tensor_tensor(
                                logit_t[:, :D, :], logit_t[:, :D, :], NEG,
                                logit_t[:, :D, :], Alu.mult, Alu.max)
                            nc.scalar.activation(exb_t[:, :D, :],
                                                 logit_t[:, :D, :], Act.Exp)
                            nc.vector.tensor_reduce(
                                den_t[:], exb_t[:, :D, :].transpose([0, 2, 1]),
                                axis=Axis.X, op=Alu.add)
                            nc.vector.reciprocal(rec_t[:], den_t[:])
                            pmg = pm1_t[:, boff + off:boff + off + D]
                            pmg_b = pmg.unsqueeze(2).broadcast_to((128, D, 4))
                            nc.vector.tensor_tensor(
                                exe_t[:, :D, :], exb_t[:, :D, :], pmg_b,
                                Alu.mult)
                            nc.vector.tensor_tensor(
                                exo_t[:, :D, :], exb_t[:, :D, :],
                                exe_t[:, :D, :], Alu.subtract)
                            if L1STEP < 4:
                                off += D
                                continue
                            h_e = gb_t[:, off:off + D, 0:128].rearrange(
                                "p b (h c) -> p b h c", h=4)
                            nc.vector.tensor_tensor(
                                h_e, h_e,
                                exe_t[:, :D, :].unsqueeze(3).broadcast_to(
                                    (128, D, 4, HID)), Alu.mult)
                            h_o = gb_t[:, off:off + D, 128:256].rearrange(
                                "p b (h c) -> p b h c", h=4)
                            nc.vector.tensor_tensor(
                                h_o, h_o,
                                exo_t[:, :D, :].unsqueeze(3).broadcast_to(
                                    (128, D, 4, HID)), Alu.mult)
                            if L1STEP < 5:
                                off += D
                                continue
                            agg = aggp.tile([128, 128], dt.float32, tag="agg")
                            for bi in range(2 * D):
                                rhs = gb_t[:, off + bi // 2,
                                           (bi % 2) * 128:(bi % 2 + 1) * 128]
                                nc.tensor.matmul(agg[:], id_t[:], rhs,
                                                 start=(bi == 0),
                                                 stop=(bi == 2 * D - 1))
                            scaled_t = epip.tile([128, 128], dt.float32,
                                                 tag="sd")
                            nc.vector.tensor_tensor(
                                scaled_t[:].rearrange("p (h c) -> p h c", h=4),
                                agg[:].rearrange("p (h c) -> p h c", h=4),
                                rec_t[:].unsqueeze(2).broadcast_to(
                                    (128, 4, HID)), Alu.mult)
                            if HASB1:
                                nc.vector.tensor_tensor(
                                    scaled_t[:], scaled_t[:], b1_t[:],
                                    Alu.add)
                            tmp_t = epip.tile([128, 128], dt.float32,
                                              tag="tm")
                            nc.scalar.activation(tmp_t[:], scaled_t[:],
                                                 Act.Relu, scale=-1.0)
                            nc.scalar.activation(tmp_t[:], tmp_t[:], Act.Exp,
                                                 scale=-1.0)
                            nc.vector.scalar_tensor_tensor(
                                elu_all[:, g, :], tmp_t[:], -1.0, scaled_t[:],
                                Alu.add, Alu.max)
                            off += D
                        # ---- pass 2 for this superblock's groups
                        if L1STEP >= 5:
                            for g in sb:
                                psT = psTp.tile([128, 128], dt.bfloat16,
                                                tag="pt")
                                nc.tensor.transpose(psT[:], elu_all[:, g, :],
                                                    id_t[:])
                                eluT_t = epip.tile([128, 128], dt.bfloat16,
                                                   tag="et")
                                nc.scalar.activation(eluT_t[:], psT[:],
                                                     Act.Copy)
                                ps2 = ps2p.tile([128, 34], dt.float32,
                                                tag="p2")
                                nc.tensor.matmul(ps2[:], eluT_t[:], w2_t[:],
                                                 start=True, stop=True)
                                if g % 2 == 0:
                                    nc.scalar.activation(scat_t[:, g, 0:34],
                                                         ps2[:], Act.Copy)
                                else:
                                    nc.vector.tensor_copy(scat_t[:, g, 0:34],
                                                          ps2[:])
                    if L1STEP >= 6:
                        nc.gpsimd.dma_scatter_add(
                            h2sh[0:S + 1, :], scat_t[:], scat1_t[:],
                            128 * NG1, 128 * NG1, 64,
                            single_packet=False, queue_num=nextq())
                        # pad slots scatter garbage into the trash row;
                        # overwrite with the al_src=-100 pad row before the
                        # L2 extraction reads it
                        nc.sync.dma_start(h2sh[S:S + 1, :], p2_t[0:1, :])
            if PHASES >= 3:
                # ---------------- exchange ----------------
                # pack own rows to blocked bf16 (also yields the ald column),
                # then AllGather the compact table
                with tc.tile_pool(name="aldtmp2", bufs=1) as atp:
                    atmp = atp.tile([128, NG2, 64], dt.float32)
                    nc.sync.dma_start(
                        atmp[:],
                        h2sh[0:128 * NG2, :].rearrange("(b p) e -> p b e",
                                                       p=128))
                    nc.vector.tensor_copy(ald2_t[:], atmp[:, :, 33:34])
                    atmpb = atp.tile([128, NG2, 64], dt.bfloat16)
                    nc.scalar.activation(atmpb[:], atmp[:], Act.Copy)
                    nc.sync.dma_start(
                        h2b[:, :], atmpb[:].rearrange("p b e -> p (b e)"))
                nc.gpsimd.collective_compute(
                    "AllGather", mybir.AluOpType.bypass, replica_groups=rg,
                    ins=[h2b[:, :]],
                    outs=[table2b[:].rearrange("(r x) -> r x",
                                               r=128 * NCORES)])

            if PHASES >= 4:
                # ---------------- phase L2: edges + pool ----------------
                # preloads below overlap the AllGather
                nc.sync.dma_start(i2_all[:], t_i2[:])
                nc.sync.dma_start(pm2_t[:], t_pm2[:])
                tab2p = table2b[:].rearrange("(y c) -> y c", c=128)
                Dmax2 = max(D2)
                NBSB2 = max(sum(D2[g] for g in sb) for sb in SB2)
                with (
                    tc.tile_pool(name="gath2", bufs=4) as gathp,
                    tc.tile_pool(name="small2", bufs=3) as smallp,
                    tc.tile_pool(name="epi2", bufs=3) as epip,
                    tc.tile_pool(name="agg2", bufs=2, space="PSUM") as aggp,
                    tc.tile_pool(name="poolps", bufs=1,
                                 space="PSUM") as poolpp,
                    tc.tile_pool(name="mp2", bufs=3) as mpp,
                ):
                    poolps = poolpp.tile([HID, GPOOL], dt.float32)
                    h2p_all = mpp.tile([128, NG2, HID], dt.bfloat16,
                                       tag="h2pall", bufs=1)
                    for sb in SB2:
                        g0 = sb[0]
                        nb = sum(D2[g] for g in sb)
                        boff = o2[g0]
                        gb_t = gathp.tile([128, NBSB2, 128], dt.bfloat16,
                                          tag="gb")
                        nc.gpsimd.dma_gather(
                            gb_t[:, :nb, :], tab2p,
                            i2_all[:, 8 * boff:8 * (boff + nb)],
                            128 * nb, 128 * nb, 128, single_packet=False,
                            queue_num=nextq())
                        off = 0
                        for gi, g in enumerate(sb):
                            D = D2[g]
                            logit_t = smallp.tile([128, Dmax2, 1], dt.float32,
                                                  tag="lg")
                            t2_t = smallp.tile([128, Dmax2, 1], dt.float32,
                                               tag="t2")
                            exf_t = smallp.tile([128, Dmax2, 1], dt.float32,
                                                tag="exf")
                            exe_t = smallp.tile([128, Dmax2, 1], dt.float32,
                                                tag="exe")
                            exo_t = smallp.tile([128, Dmax2, 1], dt.float32,
                                                tag="exo")
                            den_t = smallp.tile([128, 1], dt.float32,
                                                tag="dn")
                            rec_t = smallp.tile([128, 1], dt.float32,
                                                tag="rc")
                            ald_ap = ald2_t[:, g, :]
                            pmg = pm2_t[:, boff + off:boff + off + D]
                            pmg_b = pmg.unsqueeze(2)
                            # als = odd - (odd-even)*pme
                            nc.vector.tensor_tensor(
                                t2_t[:, :D, :],
                                gb_t[:, off:off + D, 96:97],
                                gb_t[:, off:off + D, 32:33], Alu.subtract)
                            nc.vector.tensor_tensor(
                                t2_t[:, :D, :], t2_t[:, :D, :], pmg_b,
                                Alu.mult)
                            nc.vector.tensor_tensor(
                                logit_t[:, :D, :],
                                gb_t[:, off:off + D, 96:97],
                                t2_t[:, :D, :], Alu.subtract)
                            nc.vector.tensor_scalar(
                                logit_t[:, :D, :], logit_t[:, :D, :],
                                ald_ap, None, Alu.add)
                            nc.vector.scalar_tensor_tensor(
                                logit_t[:, :D, :], logit_t[:, :D, :], NEG,
                                logit_t[:, :D, :], Alu.mult, Alu.max)
                            nc.scalar.activation(exf_t[:, :D, :],
                                                 logit_t[:, :D, :], Act.Exp)
                            nc.vector.tensor_reduce(
                                den_t[:], exf_t[:, :D, :].transpose([0, 2, 1]),
                                axis=Axis.X, op=Alu.add)
                            nc.vector.reciprocal(rec_t[:], den_t[:])
                            nc.vector.tensor_tensor(
                                exe_t[:, :D, :], exf_t[:, :D, :], pmg_b,
                                Alu.mult)
                            nc.vector.tensor_tensor(
                                exo_t[:, :D, :], exf_t[:, :D, :],
                                exe_t[:, :D, :], Alu.subtract)
                            exh_t = smallp.tile([128, Dmax2, 2, HID],
                                                dt.bfloat16, tag="exh")
                            nc.vector.tensor_tensor(
                                exh_t[:, :D, 0, :],
                                gb_t[:, off:off + D, 0:HID],
                                exe_t[:, :D, :].broadcast_to(
                                    (128, D, HID)), Alu.mult)
                            nc.vector.tensor_tensor(
                                exh_t[:, :D, 1, :],
                                gb_t[:, off:off + D, 64:64 + HID],
                                exo_t[:, :D, :].broadcast_to(
                                    (128, D, HID)), Alu.mult)
                            agg = aggp.tile([128, HID], dt.float32, tag="agg")
                            for bi in range(2 * D):
                                nc.tensor.matmul(
                                    agg[:], id_t[:],
                                    exh_t[:, bi // 2, bi % 2, :],
                                    start=(bi == 0),
                                    stop=(bi == 2 * D - 1))
                            scaled_t = epip.tile([128, HID], dt.float32,
                                                 tag="sd")
                            nc.vector.tensor_scalar(scaled_t[:], agg[:],
                                                    rec_t[:], None, Alu.mult)
                            if HASB2:
                                nc.vector.tensor_tensor(
                                    scaled_t[:], scaled_t[:], b2_t[:],
                                    Alu.add)
                            tmp_t = epip.tile([128, HID], dt.float32,
                                              tag="tm")
                            nc.scalar.activation(tmp_t[:], scaled_t[:],
                                                 Act.Relu, scale=-1.0)
                            nc.scalar.activation(tmp_t[:], tmp_t[:], Act.Exp,
                                                 scale=-1.0)
                            nc.vector.scalar_tensor_tensor(
                                h2p_all[:, g, :], tmp_t[:], -1.0, scaled_t[:],
                                Alu.add, Alu.max)
                            mp_t = mpp.tile([128, GPOOL], dt.bfloat16,
                                            tag="mp")
                            nc.sync.dma_start(
                                mp_t[:], t_mpool[g * 128:(g + 1) * 128, :])
                            nc.tensor.matmul(poolps[:], h2p_all[:, g, :],
                                             mp_t[:], start=(g == 0),
                                             stop=(g == NG2 - 1))
                            off += D
                    # ------------- pool + final linear -------------
                    with tc.tile_pool(name="fin", bufs=1) as finp, \
                            tc.tile_pool(name="finps", bufs=1,
                                         space="PSUM") as fpp:
                        poolsb = finp.tile([HID, GPOOL], dt.float32)
                        nc.vector.tensor_copy(poolsb[:], poolps[:])
                        nc.sync.dma_start(cc_in[:, :], poolsb[:])
                        nc.gpsimd.collective_compute(
                            "AllReduce", Alu.add, replica_groups=rg,
                            ins=[cc_in[:, :]], outs=[cc_out[:, :]])
                        psum_t = finp.tile([HID, GPOOL], dt.float32)
                        nc.sync.dma_start(psum_t[:], cc_out[:, :])
                        mean_t = finp.tile([HID, GPOOL], dt.float32)
                        nc.vector.tensor_tensor(
                            mean_t[:], psum_t[:],
                            rc_t[:], Alu.mult)
                        psO = fpp.tile([GPOOL, OUT], dt.float32)
                        nc.tensor.matmul(psO[:], mean_t[:], wl_t[:],
                                         start=True, stop=True)
                        out_t = finp.tile([GPOOL, OUT], dt.float32)
                        if HASBL:
                            nc.vector.tensor_tensor(out_t[:], psO[:], bl_t[:],
                                                    Alu.add)
                        else:
                            nc.vector.tensor_copy(out_t[:], psO[:])
                        nc.sync.dma_start(t_out[:, :], out_t[:])
            if PHASES < 4:
                with tc.tile_pool(name='dummy', bufs=1) as dp:
                    d = dp.tile([GPOOL, OUT], dt.float32)
                    nc.vector.memset(d[:], 0.0)
                    nc.sync.dma_start(t_out[:, :], d[:])

    nc.compile()
    return nc


def core_inputs(prep, c):
    cd = prep["cores"][c]
    sc_D1, sc_D2 = prep["D1"], prep["D2"]
    NB1, NB2 = sum(sc_D1), sum(sc_D2)
    n1 = max(8 * NB1, 8)
    n2 = max(8 * NB2, 8)

    def padcols(a, cols, dtype):
        if a.shape[1] == cols:
            return np.ascontiguousarray(a)
        out = np.zeros((a.shape[0], cols), dtype)
        out[:, :a.shape[1]] = a
        return out

    return dict(
        xT=np.ascontiguousarray(cd["xT"]),
        w1=prep["W1"], w2ext=prep["W2ext"], wl=prep["Wl"],
        a1x=prep["a1x"],
        b1=prep["b1"], b2=prep["b2"], bl=prep["bl"], rcnt=prep["rcnt"],
        patch1=prep["patch1"], patch2=prep["patch2"], ident=prep["ident"],
        mpool=np.ascontiguousarray(cd["mpool"]),
        idx1=padcols(cd["w_idx1"], n1, np.int16),
        idx2=padcols(cd["w_idx2"], n2, np.int16),
        pme1=padcols(cd["pme1"], max(NB1, 1), np.float32).astype(bf16),
        pme2=padcols(cd["pme2"], max(NB2, 1), np.float32),
        scat1=cd["w_scat1"],
    )


_CACHE = {}


def kernel(**inputs):
    from concourse.bass_utils import run_bass_kernel_spmd

    inputs = {k: np.asarray(v) for k, v in inputs.items()}
    prep = host_prep(**inputs)
    sc = make_sched(prep)
    sc["D1"] = prep["D1"]
    sc["D2"] = prep["D2"]
    key = str(sc)
    if key not in _CACHE:
        _CACHE[key] = build_bass(sc)
    nc = _CACHE[key]
    in_maps = [core_inputs(prep, c) for c in range(NCORES)]
    res = run_bass_kernel_spmd(nc, in_maps, list(range(NCORES)))
    return np.asarray(res.results[0]["out"], np.float32)


# revision 30
# speedup vs baseline: 1.8132x; 1.1364x over previous
# Self-contained 8-core Trainium2 Bass kernel for the 2-layer GAT + mean-pool
# problem (nn_GAT_83820581749190).
#
# Sharding: destination nodes (and all their incident edges) are partitioned
# across the 8 cores, so each layer's attention softmax and aggregation
# complete locally per core. Each core builds a replicated layer-1 feature
# table h1 (bf16, 256-byte logical rows) in HBM with a replicated x @ W1
# matmul, then edge-gathers PAIRS of rows (512B per descriptor, index =
# row//2, int16-safe) with the GPSIMD dma_gather custom op; host-precomputed
# parity masks select the correct half downstream. Attention logits are
# computed on-chip (DVE dot with a_src/a_dst), the edge softmax runs without
# segment-max (logits are small; pad slots use a patch row whose h gives
# al_src=-100), and aggregation is identity-matmul PSUM accumulation
# (destinations on partitions via degree-bucketed groups of 128).
# Layer-2 features are exchanged with an AllGather; the same paired-row
# gather runs against the fp32 layer-2 table; mean-pool is a matmul against
# a host-built one-hot graph matrix plus a tiny AllReduce.
import numpy as np
import ml_dtypes

N = 50000
E = 800000
IN = 128
HID = 32
HEADS = 4
OUT = 10
GPOOL = 64
NEG = 0.2
NCORES = 8
S = N // NCORES
SPECIAL1 = N          # layer-1 patch row (h chosen so h . a_src = -100)
SPECIAL2 = 0          # layer-2 patch row (al_src column = -100)
SPECIAL_ALS = -100.0
SB_BLOCK_BUDGET = 24  # max gather blocks per superblock
XCHUNK = 512
PHASES = 99
L1STEP = 99

bf16 = ml_dtypes.bfloat16


def _ceil_to(v, m):
    return (v + m - 1) // m * m


# ======================= host prep =========================================

def _build_layer(src, dstl):
    deg = np.bincount(dstl, minlength=S)
    P = np.argsort(-deg, kind="stable")
    Ppos = np.empty(S, np.int64)
    Ppos[P] = np.arange(S)
    ng = (S + 127) // 128
    D = np.zeros(ng, np.int64)
    dp = deg[P]
    for g in range(ng):
        D[g] = dp[g * 128:(g + 1) * 128].max()
    assert (D > 0).all()
    return dict(src=src, dstl=dstl, deg=deg, P=P, Ppos=Ppos, D=D)


def _emit_slots(l, Dg, row_of_src, special_row):
    """Per group g: rows[g] [D[g],128] of table ROW ids (special_row pads),
    plus slot2cmp mapping output slots -> compacted dst ids."""
    NG = len(Dg)
    Ppos = l["Ppos"]
    nreal = S
    slot2cmp = np.full(NG * 128, -1, np.int64)
    slot2cmp[:nreal] = np.arange(nreal)
    rows = [np.full((int(Dg[g]), 128), special_row, np.int64)
            for g in range(NG)]
    slot_of_edge = Ppos[l["dstl"]]
    order = np.argsort(slot_of_edge, kind="stable")
    so = slot_of_edge[order]
    sr = row_of_src[l["src"][order]]
    jj = np.arange(len(so)) - np.searchsorted(so, so, side="left")
    gg, kk = so // 128, so % 128
    for g in range(NG):
        sel = gg == g
        if sel.any():
            rows[g][jj[sel], kk[sel]] = sr[sel]
    return rows, slot2cmp


def _wrap16(idx):
    """[n] -> [128, n//16] int16: idx i at [i%16, i//16], replicated x8."""
    n = len(idx)
    assert n % 16 == 0
    w = np.ascontiguousarray(np.asarray(idx).reshape(n // 16, 16).T)
    w = w.astype(np.int16)
    return np.tile(w, (8, 1))


def _wrap_rows(rows_arrs):
    """idx stream (row//2) wrapped, plus even-parity masks [128, NB]."""
    idx_segs = []
    pme_segs = []
    for a in rows_arrs:
        if a.size:
            assert (a // 2 <= 32767).all()
            idx_segs.append(_wrap16((a // 2).reshape(-1)))
            pme_segs.append(np.ascontiguousarray((1 - (a % 2)).T))
    w_idx = (np.concatenate(idx_segs, axis=1) if idx_segs
             else np.zeros((128, 0), np.int16))
    pme = (np.concatenate(pme_segs, axis=1).astype(np.float32) if pme_segs
           else np.zeros((128, 0), np.float32))
    return w_idx, pme


def host_prep(x, edge_index, batch, W1, a1_src, a1_dst, b1, W2, a2_src, a2_dst,
              b2, Wl, bl):
    x = np.asarray(x, np.float32)
    edge_index = np.asarray(edge_index, np.int64)
    batch = np.asarray(batch, np.int64)
    src_all = np.concatenate([edge_index[0], np.arange(N, dtype=np.int64)])
    dst_all = np.concatenate([edge_index[1], np.arange(N, dtype=np.int64)])
    owner = dst_all // S

    a1_src = np.asarray(a1_src, np.float32)
    a1_dst = np.asarray(a1_dst, np.float32)
    W1 = np.asarray(W1, np.float32)
    W2 = np.asarray(W2, np.float32)
    W2ext = np.concatenate(
        [W2, W2 @ np.asarray(a2_src, np.float32)[0][:, None],
         W2 @ np.asarray(a2_dst, np.float32)[0][:, None]], axis=1)  # [128,34]

    # a1x: [0:128]=a_src flat, [128:256]=a_src flat, [256:384]=a_dst flat
    asf = a1_src.reshape(-1)
    adf = a1_dst.reshape(-1)
    a1x = np.tile(np.concatenate([asf, asf, adf])[None, :], (128, 1))

    # layer-1 patch row: h with h . a_src[h] = -100 for every head
    hp = np.concatenate([SPECIAL_ALS * a1_src[h] / (a1_src[h] ** 2).sum()
                         for h in range(HEADS)])
    assert np.abs(hp).max() < 1e4
    patch1 = np.tile(hp[None, :], (1, 1))

    cores = [dict(c=c) for c in range(NCORES)]
    for cd in cores:
        c = cd["c"]
        m = owner == c
        cd["src"] = src_all[m]
        cd["dstl"] = dst_all[m] - c * S

    # ---------- layer 1 ----------
    for cd in cores:
        c = cd["c"]
        l1 = _build_layer(cd["src"], cd["dstl"])
        pos_of = np.empty(N, np.int64)
        own = np.arange(c * S, (c + 1) * S)
        oth = np.concatenate([np.arange(0, c * S), np.arange((c + 1) * S, N)])
        pos_of[oth] = S + np.arange(N - S)
        pos_of[own] = l1["Ppos"]
        cd["l1"] = l1
        cd["row_of"] = pos_of
    NG1 = max(len(cd["l1"]["D"]) for cd in cores)
    D1 = np.zeros(NG1, np.int64)
    for cd in cores:
        d = cd["l1"]["D"]
        D1[:len(d)] = np.maximum(D1[:len(d)], d)
    for cd in cores:
        cd["rows1"], cd["slot2cmp1"] = _emit_slots(
            cd["l1"], D1, cd["row_of"], SPECIAL1)

    # ---------- layer 2 ----------
    # layer-2 features live in a blocked bf16 table: core c's partition p,
    # group g at flat row (c*128+p)*NG2 + g (64 bf16 each; pairs of flat
    # rows share one 256B gather descriptor)
    for cd in cores:
        cd["l2"] = _build_layer(cd["src"], cd["dstl"])
    NG2 = max(len(cd["l2"]["D"]) for cd in cores)
    D2 = np.zeros(NG2, np.int64)
    for cd in cores:
        d = cd["l2"]["D"]
        D2[:len(d)] = np.maximum(D2[:len(d)], d)
    flat2_of = np.empty(N, np.int64)
    for cd in cores:
        c = cd["c"]
        q = cd["l2"]["Ppos"]
        flat2_of[c * S:(c + 1) * S] = \
            (c * 128 + q % 128) * NG2 + q // 128
    for cd in cores:
        c = cd["c"]
        special2 = (c * 128 + S % 128) * NG2 + S // 128  # own trash row
        cd["rows2"], cd["slot2cmp2"] = _emit_slots(
            cd["l2"], D2, flat2_of, special2)

    # ---------- aux ----------
    cnt = np.bincount(batch, minlength=GPOOL).astype(np.float32)
    recip_cnt = (1.0 / np.maximum(cnt, 1.0)).astype(np.float32)

    XT_COLS = _ceil_to(N + 2, XCHUNK)
    for cd in cores:
        c = cd["c"]
        gids = batch[c * S:(c + 1) * S]
        Mp = np.zeros((NG2 * 128, GPOOL), np.float32)
        s2c = cd["slot2cmp2"]
        real = s2c >= 0
        Mp[np.where(real)[0], gids[cd["l2"]["P"][s2c[real]]]] = 1.0
        cd["mpool"] = Mp.astype(bf16)

        s2c1 = cd["slot2cmp1"]
        tgt = np.full(len(s2c1), S, np.int64)  # trash row for dummy slots
        r1 = s2c1 >= 0
        tgt[r1] = cd["l2"]["Ppos"][cd["l1"]["P"][s2c1[r1]]]

        xt = np.zeros((IN, XT_COLS), np.float32)
        xt[:, cd["row_of"]] = x.T
        cd["xT"] = xt.astype(bf16)

        cd["w_idx1"], cd["pme1"] = _wrap_rows(cd["rows1"])
        cd["w_idx2"], cd["pme2"] = _wrap_rows(cd["rows2"])
        cd["w_scat1"] = _wrap16(tgt)

    # written over the trash row after the scatter: al_src=-100 kills pads
    patch2 = np.zeros((1, 64), np.float32)
    patch2[0, 32] = SPECIAL_ALS

    return dict(cores=cores,
                D1=[int(v) for v in D1], D2=[int(v) for v in D2],
                W1=W1.astype(bf16), W2ext=W2ext.astype(bf16),
                Wl=np.asarray(Wl, np.float32),
                a1x=a1x.astype(bf16),
                b1=np.tile(np.asarray(b1, np.float32).reshape(1, -1),
                           (128, 1)),
                b2=np.tile(np.asarray(b2, np.float32).reshape(1, -1),
                           (128, 1)),
                bl=np.tile(np.asarray(bl, np.float32).reshape(1, -1),
                           (GPOOL, 1)),
                rcnt=np.tile(recip_cnt.reshape(1, -1), (HID, 1)),
                patch1=patch1.astype(bf16), patch2=patch2,
                ident=np.eye(128, dtype=bf16))


def _pack_superblocks(D, budget=SB_BLOCK_BUDGET):
    sbs, cur, tot = [], [], 0
    for g in range(len(D)):
        d = int(D[g])
        if cur and tot + d > budget:
            sbs.append(cur)
            cur, tot = [], 0
        cur.append(g)
        tot += d
    if cur:
        sbs.append(cur)
    return sbs


def make_sched(prep):
    D1, D2 = prep["D1"], prep["D2"]
    return dict(D1=D1, D2=D2,
                SB1=_pack_superblocks(D1), SB2=_pack_superblocks(D2),
                HASB1=bool(np.any(prep["b1"])), HASB2=bool(np.any(prep["b2"])),
                HASBL=bool(np.any(prep["bl"])))


# ======================= bass kernel =======================================

def build_bass(sc):
    import concourse.bacc as bacc
    import concourse.tile as tile
    import concourse.mybir as mybir
    from concourse.library_config import mlp

    dt = mybir.dt
    Alu = mybir.AluOpType
    Act = mybir.ActivationFunctionType
    Axis = mybir.AxisListType

    D1, D2 = sc["D1"], sc["D2"]
    SB1, SB2 = sc["SB1"], sc["SB2"]
    HASB1 = sc.get("HASB1", True)
    HASB2 = sc.get("HASB2", True)
    HASBL = sc.get("HASBL", True)
    NG1, NG2 = len(D1), len(D2)
    XT_COLS = _ceil_to(N + 2, XCHUNK)
    NCHUNK = XT_COLS // XCHUNK
    SH2_ROWS = _ceil_to(S + 2, 128)
    NB1 = sum(D1)
    NB2 = sum(D2)
    o1 = np.concatenate([[0], np.cumsum(D1)]).astype(int)
    o2 = np.concatenate([[0], np.cumsum(D2)]).astype(int)

    nc = bacc.Bacc("TRN2", target_bir_lowering=False, debug=False,
                   num_devices=NCORES, num_swdge_queues=4)

    t_xT = nc.dram_tensor("xT", [IN, XT_COLS], dt.bfloat16,
                          kind="ExternalInput")
    t_w1 = nc.dram_tensor("w1", [IN, IN], dt.bfloat16, kind="ExternalInput")
    t_w2 = nc.dram_tensor("w2ext", [IN, 34], dt.bfloat16,
                          kind="ExternalInput")
    t_wl = nc.dram_tensor("wl", [HID, OUT], dt.float32, kind="ExternalInput")
    t_a1x = nc.dram_tensor("a1x", [128, 384], dt.bfloat16,
                           kind="ExternalInput")
    t_b1 = nc.dram_tensor("b1", [128, HEADS * HID], dt.float32,
                          kind="ExternalInput")
    t_b2 = nc.dram_tensor("b2", [128, HID], dt.float32, kind="ExternalInput")
    t_bl = nc.dram_tensor("bl", [GPOOL, OUT], dt.float32,
                          kind="ExternalInput")
    t_rcnt = nc.dram_tensor("rcnt", [HID, GPOOL], dt.float32,
                            kind="ExternalInput")
    t_patch1 = nc.dram_tensor("patch1", [1, 128], dt.bfloat16,
                              kind="ExternalInput")
    t_patch2 = nc.dram_tensor("patch2", [1, 64], dt.float32,
                              kind="ExternalInput")
    t_ident = nc.dram_tensor("ident", [128, 128], dt.bfloat16,
                             kind="ExternalInput")
    t_mpool = nc.dram_tensor("mpool", [NG2 * 128, GPOOL], dt.bfloat16,
                             kind="ExternalInput")
    n1 = max(8 * NB1, 8)
    n2 = max(8 * NB2, 8)
    t_i1 = nc.dram_tensor("idx1", [128, n1], dt.int16, kind="ExternalInput")
    t_i2 = nc.dram_tensor("idx2", [128, n2], dt.int16, kind="ExternalInput")
    t_pm1 = nc.dram_tensor("pme1", [128, max(NB1, 1)], dt.bfloat16,
                           kind="ExternalInput")
    t_pm2 = nc.dram_tensor("pme2", [128, max(NB2, 1)], dt.float32,
                           kind="ExternalInput")
    t_scat1 = nc.dram_tensor("scat1", [128, 8 * NG1], dt.int16,
                             kind="ExternalInput")
    t_out = nc.dram_tensor("out", [GPOOL, OUT], dt.float32,
                           kind="ExternalOutput")

    rg = [list(range(NCORES))]
    _qc = [0]

    def nextq():
        _qc[0] = (_qc[0] + 1) % 4
        return _qc[0]

    with tile.TileContext(nc) as tc:
        with (
            tc.tile_pool(name="const", bufs=1) as constp,
            tc.tile_pool(name="pre", bufs=1) as prep_pool,
            tc.tile_pool(name="dram", bufs=1, space="DRAM") as dramp,
        ):
            nc.gpsimd.load_library(mlp)

            # logical row-major tables; gathers view them as paired rows
            table1 = dramp.tile([XT_COLS, 128], dt.bfloat16, tag="table1")
            h2b = dramp.tile([128, NG2 * 64], dt.bfloat16, tag="h2b")
            table2b = dramp.tile([128 * NCORES * NG2 * 64], dt.bfloat16,
                                 tag="table2b")
            h2sh = dramp.tile([SH2_ROWS, 64], dt.float32, tag="h2sh")
            cc_in = dramp.tile([HID, GPOOL], dt.float32, tag="ccin")
            cc_out = dramp.tile([HID, GPOOL], dt.float32, tag="ccout")

            w1_t = constp.tile([IN, IN], dt.bfloat16)
            nc.sync.dma_start(w1_t[:], t_w1[:])
            w2_t = constp.tile([IN, 34], dt.bfloat16)
            nc.sync.dma_start(w2_t[:], t_w2[:])
            wl_t = constp.tile([HID, OUT], dt.float32)
            nc.sync.dma_start(wl_t[:], t_wl[:])
            a1x_t = constp.tile([128, 384], dt.bfloat16)
            nc.sync.dma_start(a1x_t[:], t_a1x[:])
            b1_t = constp.tile([128, HEADS * HID], dt.float32)
            nc.sync.dma_start(b1_t[:], t_b1[:])
            b2_t = constp.tile([128, HID], dt.float32)
            nc.sync.dma_start(b2_t[:], t_b2[:])
            bl_t = constp.tile([GPOOL, OUT], dt.float32)
            nc.sync.dma_start(bl_t[:], t_bl[:])
            rc_t = constp.tile([HID, GPOOL], dt.float32)
            nc.sync.dma_start(rc_t[:], t_rcnt[:])
            id_t = constp.tile([128, 128], dt.bfloat16)
            nc.sync.dma_start(id_t[:], t_ident[:])

            # preload all gather indices and parity masks
            i1_all = prep_pool.tile([128, n1], dt.int16)
            nc.sync.dma_start(i1_all[:], t_i1[:])
            pm1_t = prep_pool.tile([128, max(NB1, 1)], dt.bfloat16)
            nc.sync.dma_start(pm1_t[:], t_pm1[:])
            i2_all = prep_pool.tile([128, n2], dt.int16)
            pm2_t = prep_pool.tile([128, max(NB2, 1)], dt.float32)
            scat1_t = prep_pool.tile([128, 8 * NG1], dt.int16)
            nc.sync.dma_start(scat1_t[:], t_scat1[:])
            ald1_t = prep_pool.tile([128, NG1, 4], dt.float32)
            ald2_t = prep_pool.tile([128, NG2, 1], dt.float32)

            # zero the scatter_add target
            with tc.tile_pool(name="zp", bufs=1) as zp:
                z_t = zp.tile([128, SH2_ROWS // 128 * 64], dt.float32)
                nc.vector.memset(z_t[:], 0.0)
                nc.sync.dma_start(
                    h2sh[:, :].rearrange("(p k) e -> p (k e)", p=128), z_t[:])

            # ---------------- phase X: build table1 ----------------
            with (
                tc.tile_pool(name="xload", bufs=3) as xlp,
                tc.tile_pool(name="xout", bufs=3) as xop,
                tc.tile_pool(name="xpsum", bufs=4, space="PSUM") as xpp,
            ):
                for t in range(NCHUNK):
                    xt_t = xlp.tile([IN, XCHUNK], dt.bfloat16, tag="xt")
                    nc.sync.dma_start(xt_t[:],
                                      t_xT[:, t * XCHUNK:(t + 1) * XCHUNK])
                    o_t = xop.tile([128, 4, 128], dt.bfloat16, tag="xo")
                    for k in range(4):
                        ps = xpp.tile([128, 128], dt.float32, tag="xp")
                        nc.tensor.matmul(ps[:], xt_t[:, k * 128:(k + 1) * 128],
                                         w1_t[:], start=True, stop=True)
                        if k % 2 == 0:
                            nc.vector.tensor_copy(o_t[:, k, :], ps[:])
                        else:
                            nc.scalar.activation(o_t[:, k, :], ps[:],
                                                 Act.Copy)
                        gix = 4 * t + k
                        if gix < NG1:
                            # own-row ald = h . a_dst, straight off the PSUM
                            ap_t = xop.tile([128, 128], dt.bfloat16,
                                            tag="apr")
                            nc.vector.tensor_tensor(
                                ap_t[:], ps[:], a1x_t[:, 256:384], Alu.mult)
                            nc.vector.tensor_reduce(
                                ald1_t[:, gix, :],
                                ap_t[:].rearrange("p (h c) -> p h c", h=4),
                                axis=Axis.X, op=Alu.add)
                    nc.sync.dma_start(
                        table1[t * XCHUNK:(t + 1) * XCHUNK, :].rearrange(
                            "(k p) e -> p k e", p=128), o_t[:])
            with tc.tile_pool(name="patchp", bufs=1) as pp:
                p1_t = pp.tile([1, 128], dt.bfloat16)
                nc.sync.dma_start(p1_t[:], t_patch1[:])
                nc.sync.dma_start(table1[SPECIAL1:SPECIAL1 + 1, :],
                                  p1_t[0:1, :])
            p2_t = prep_pool.tile([1, 64], dt.float32)
            nc.sync.dma_start(p2_t[:], t_patch2[:])

            if PHASES >= 2:
                # ---------------- phase L1: edges ----------------
                tab1p = table1[:, :].rearrange("(a h) c -> a (h c)", h=2)
                Dmax1 = max(D1)
                NBSB1 = max(sum(D1[g] for g in sb) for sb in SB1)
                with (
                    tc.tile_pool(name="gath1", bufs=5) as gathp,
                    tc.tile_pool(name="als1", bufs=2) as alsp,
                    tc.tile_pool(name="small1", bufs=3) as smallp,
                    tc.tile_pool(name="epi1", bufs=3) as epip,
                    tc.tile_pool(name="scatp", bufs=1) as scatp,
                    tc.tile_pool(name="agg1", bufs=2, space="PSUM") as aggp,
                    tc.tile_pool(name="psT1", bufs=2, space="PSUM") as psTp,
                    tc.tile_pool(name="ps21", bufs=2, space="PSUM") as ps2p,
                ):
                    scat_t = scatp.tile([128, NG1, 64], dt.float32, tag="sc")
                    nc.vector.memset(scat_t[:], 0.0)
                    elu_all = scatp.tile([128, NG1, 128], dt.bfloat16,
                                         tag="eluall")
                    for sb in SB1:
                        g0 = sb[0]
                        nb = sum(D1[g] for g in sb)
                        boff = o1[g0]
                        gb_t = gathp.tile([128, NBSB1, 256],
                                          dt.bfloat16, tag="gb")
                        nc.gpsimd.dma_gather(
                            gb_t[:, :nb, :], tab1p,
                            i1_all[:, 8 * boff:8 * (boff + nb)],
                            128 * nb, 128 * nb, 256,
                            single_packet=False, queue_num=nextq())
                        if L1STEP < 2:
                            continue
                        # al_src for both pair-halves: prod + reduce
                        prod_t = alsp.tile([128, NBSB1, 256], dt.bfloat16,
                                           tag="prod")
                        als8_t = alsp.tile([128, NBSB1, 8], dt.float32,
                                           tag="als8")
                        als_t = alsp.tile([128, NBSB1, 4], dt.float32,
                                          tag="als")
                        nc.vector.tensor_tensor(
                            prod_t[:, :nb, :], gb_t[:, :nb, :],
                            a1x_t[:, 0:256].unsqueeze(1).broadcast_to(
                                (128, nb, 256)), Alu.mult)
                        nc.vector.tensor_reduce(
                            als8_t[:, :nb, :],
                            prod_t[:, :nb, :].rearrange(
                                "p b (j c) -> p b j c", j=8),
                            axis=Axis.X, op=Alu.add)
                        # parity-select: als = even*pme + odd*(1-pme)
                        #              = odd - (odd-even)*pme
                        pme_b = pm1_t[:, boff:boff + nb].unsqueeze(
                            2).broadcast_to((128, nb, 4))
                        t1_t = alsp.tile([128, NBSB1, 4], dt.float32,
                                         tag="t1")
                        nc.vector.tensor_tensor(
                            t1_t[:, :nb, :], als8_t[:, :nb, 4:8],
                            als8_t[:, :nb, 0:4], Alu.subtract)
                        nc.vector.tensor_tensor(
                            t1_t[:, :nb, :], t1_t[:, :nb, :], pme_b,
                            Alu.mult)
                        nc.vector.tensor_tensor(
                            als_t[:, :nb, :], als8_t[:, :nb, 4:8],
                            t1_t[:, :nb, :], Alu.subtract)
                        off = 0
                        for gi, g in enumerate(sb):
                            D = D1[g]
                            if L1STEP < 3:
                                off += D
                                continue
                            logit_t = smallp.tile([128, Dmax1, 4], dt.float32,
                                                  tag="lg")
                            exb_t = smallp.tile([128, Dmax1, 4], dt.bfloat16,
                                                tag="exb")
                            exe_t = smallp.tile([128, Dmax1, 4], dt.bfloat16,
                                                tag="exe")
                            exo_t = smallp.tile([128, Dmax1, 4], dt.bfloat16,
                                                tag="exo")
                            den_t = smallp.tile([128, 4], dt.float32,
                                                tag="dn")
                            rec_t = smallp.tile([128, 4], dt.float32,
                                                tag="rc")
                            ald_ap = ald1_t[:, g, :]
                            nc.vector.scalar_tensor_tensor(
                                logit_t[:, :D, :], als_t[:, off:off + D, :],
                                0.0,
                                ald_ap.unsqueeze(1).broadcast_to(
                                    (128, D, 4)), Alu.add, Alu.add)
                            nc.vector.scalar_tensor_tensor(
                                logit_t[:, :D, :], logit_t[:, :D, :], NEG,
                                logit_t[:, :D, :], Alu.mult, Alu.max)
                            nc.scalar.activation(exb_t[:, :D, :],
                                                 logit_t[:, :D, :], Act.Exp)
                            nc.vector.tensor_reduce(
                                den_t[:], exb_t[:, :D, :].transpose([0, 2, 1]),
                                axis=Axis.X, op=Alu.add)
                            nc.vector.reciprocal(rec_t[:], den_t[:])
                            pmg = pm1_t[:, boff + off:boff + off + D]
                            pmg_b = pmg.unsqueeze(2).broadcast_to((128, D, 4))
                            nc.vector.tensor_tensor(
                                exe_t[:, :D, :], exb_t[:, :D, :], pmg_b,
                                Alu.mult)
                            nc.vector.tensor_tensor(
                                exo_t[:, :D, :], exb_t[:, :D, :],
                                exe_t[:, :D, :], Alu.subtract)
                            if L1STEP < 4:
                                off += D
                                continue
                            h_e = gb_t[:, off:off + D, 0:128].rearrange(
                                "p b (h c) -> p b h c", h=4)
                            nc.vector.tensor_tensor(
                                h_e, h_e,
                                exe_t[:, :D, :].unsqueeze(3).broadcast_to(
                                    (128, D, 4, HID)), Alu.mult)
                            h_o = gb_t[:, off:off + D, 128:256].rearrange(
                                "p b (h c) -> p b h c", h=4)
                            nc.vector.tensor_tensor(
                                h_o, h_o,
                                exo_t[:, :D, :].unsqueeze(3).broadcast_to(
                                    (128, D, 4, HID)), Alu.mult)
                            if L1STEP < 5:
                                off += D
                                continue
                            agg = aggp.tile([128, 128], dt.float32, tag="agg")
                            for bi in range(2 * D):
                                rhs = gb_t[:, off + bi // 2,
                                           (bi % 2) * 128:(bi % 2 + 1) * 128]
                                nc.tensor.matmul(agg[:], id_t[:], rhs,
                                                 start=(bi == 0),
                                                 stop=(bi == 2 * D - 1))
                            scaled_t = epip.tile([128, 128], dt.float32,
                                                 tag="sd")
                            nc.vector.tensor_tensor(
                                scaled_t[:].rearrange("p (h c) -> p h c", h=4),
                                agg[:].rearrange("p (h c) -> p h c", h=4),
                                rec_t[:].unsqueeze(2).broadcast_to(
                                    (128, 4, HID)), Alu.mult)
                            if HASB1:
                                nc.vector.tensor_tensor(
                                    scaled_t[:], scaled_t[:], b1_t[:],
                                    Alu.add)
                            tmp_t = epip.tile([128, 128], dt.float32,
                                              tag="tm")
                            nc.scalar.activation(tmp_t[:], scaled_t[:],
                                                 Act.Relu, scale=-1.0)
                            nc.scalar.activation(tmp_t[:], tmp_t[:], Act.Exp,
                                                 scale=-1.0)
                            nc.vector.scalar_tensor_tensor(
                                elu_all[:, g, :], tmp_t[:], -1.0, scaled_t[:],
                                Alu.add, Alu.max)
                            off += D
                        # ---- pass 2 for this superblock's groups
                        if L1STEP >= 5:
                            for g in sb:
                                psT = psTp.tile([128, 128], dt.bfloat16,
                                                tag="pt")
                                nc.tensor.transpose(psT[:], elu_all[:, g, :],
                                                    id_t[:])
                                eluT_t = epip.tile([128, 128], dt.bfloat16,
                                                   tag="et")
                                nc.scalar.activation(eluT_t[:], psT[:],
                                                     Act.Copy)
                                ps2 = ps2p.tile([128, 34], dt.float32,
                                                tag="p2")
                                nc.tensor.matmul(ps2[:], eluT_t[:], w2_t[:],
                                                 start=True, stop=True)
                                if g % 2 == 0:
                                    nc.scalar.activation(scat_t[:, g, 0:34],
                                                         ps2[:], Act.Copy)
                                else:
                                    nc.vector.tensor_copy(scat_t[:, g, 0:34],
                                                          ps2[:])
                    if L1STEP >= 6:
                        nc.gpsimd.dma_scatter_add(
                            h2sh[0:S + 1, :], scat_t[:], scat1_t[:],
                            128 * NG1, 128 * NG1, 64,
                            single_packet=False, queue_num=nextq())
                        # pad slots scatter garbage into the trash row;
                        # overwrite with the al_src=-100 pad row before the
                        # L2 extraction reads it
                        nc.sync.dma_start(h2sh[S:S + 1, :], p2_t[0:1, :])
            if PHASES >= 3:
                # ---------------- exchange ----------------
                # pack own rows to blocked bf16 (also yields the ald column),
                # then AllGather the compact table
                with tc.tile_pool(name="aldtmp2", bufs=1) as atp:
                    atmp = atp.tile([128, NG2, 64], dt.float32)
                    nc.sync.dma_start(
                        atmp[:],
                        h2sh[0:128 * NG2, :].rearrange("(b p) e -> p b e",
                                                       p=128))
                    nc.vector.tensor_copy(ald2_t[:], atmp[:, :, 33:34])
                    atmpb = atp.tile([128, NG2, 64], dt.bfloat16)
                    nc.scalar.activation(atmpb[:], atmp[:], Act.Copy)
                    nc.sync.dma_start(
                        h2b[:, :], atmpb[:].rearrange("p b e -> p (b e)"))
                nc.gpsimd.collective_compute(
                    "AllGather", mybir.AluOpType.bypass, replica_groups=rg,
                    ins=[h2b[:, :]],
                    outs=[table2b[:].rearrange("(r x) -> r x",
                                               r=128 * NCORES)])

            if PHASES >= 4:
                # ---------------- phase L2: edges + pool ----------------
                # preloads below overlap the AllGather
                nc.sync.dma_start(i2_all[:], t_i2[:])
                nc.sync.dma_start(pm2_t[:], t_pm2[:])
                tab2p = table2b[:].rearrange("(y c) -> y c", c=128)
                Dmax2 = max(D2)
                NBSB2 = max(sum(D2[g] for g in sb) for sb in SB2)
                with (
                    tc.tile_pool(name="gath2", bufs=6) as gathp,
                    tc.tile_pool(name="small2", bufs=3) as smallp,
                    tc.tile_pool(name="epi2", bufs=3) as epip,
                    tc.tile_pool(name="agg2", bufs=2, space="PSUM") as aggp,
                    tc.tile_pool(name="poolps", bufs=1,
                                 space="PSUM") as poolpp,
                    tc.tile_pool(name="mp2", bufs=3) as mpp,
                ):
                    poolps = poolpp.tile([HID, GPOOL], dt.float32)
                    h2p_all = mpp.tile([128, NG2, HID], dt.bfloat16,
                                       tag="h2pall", bufs=1)
                    for sb in SB2:
                        g0 = sb[0]
                        nb = sum(D2[g] for g in sb)
                        boff = o2[g0]
                        gb_t = gathp.tile([128, NBSB2, 128], dt.bfloat16,
                                          tag="gb")
                        nc.gpsimd.dma_gather(
                            gb_t[:, :nb, :], tab2p,
                            i2_all[:, 8 * boff:8 * (boff + nb)],
                            128 * nb, 128 * nb, 128, single_packet=False,
                            queue_num=nextq())
                        off = 0
                        for gi, g in enumerate(sb):
                            D = D2[g]
                            logit_t = smallp.tile([128, Dmax2, 1], dt.float32,
                                                  tag="lg")
                            t2_t = smallp.tile([128, Dmax2, 1], dt.float32,
                                               tag="t2")
                            exf_t = smallp.tile([128, Dmax2, 1], dt.float32,
                                                tag="exf")
                            exe_t = smallp.tile([128, Dmax2, 1], dt.float32,
                                                tag="exe")
                            exo_t = smallp.tile([128, Dmax2, 1], dt.float32,
                                                tag="exo")
                            den_t = smallp.tile([128, 1], dt.float32,
                                                tag="dn")
                            rec_t = smallp.tile([128, 1], dt.float32,
                                                tag="rc")
                            ald_ap = ald2_t[:, g, :]
                            pmg = pm2_t[:, boff + off:boff + off + D]
                            pmg_b = pmg.unsqueeze(2)
                            # als = odd - (odd-even)*pme
                            nc.vector.tensor_tensor(
                                t2_t[:, :D, :],
                                gb_t[:, off:off + D, 96:97],
                                gb_t[:, off:off + D, 32:33], Alu.subtract)
                            nc.vector.tensor_tensor(
                                t2_t[:, :D, :], t2_t[:, :D, :], pmg_b,
                                Alu.mult)
                            nc.vector.tensor_tensor(
                                logit_t[:, :D, :],
                                gb_t[:, off:off + D, 96:97],
                                t2_t[:, :D, :], Alu.subtract)
                            nc.vector.tensor_scalar(
                                logit_t[:, :D, :], logit_t[:, :D, :],
                                ald_ap, None, Alu.add)
                            nc.vector.scalar_tensor_tensor(
                                logit_t[:, :D, :], logit_t[:, :D, :], NEG,
                                logit_t[:, :D, :], Alu.mult, Alu.max)
                            nc.scalar.activation(exf_t[:, :D, :],
                                                 logit_t[:, :D, :], Act.Exp)
                            nc.vector.tensor_reduce(
                                den_t[:], exf_t[:, :D, :].transpose([0, 2, 1]),
                                axis=Axis.X, op=Alu.add)
                            nc.vector.reciprocal(rec_t[:], den_t[:])
                            nc.vector.tensor_tensor(
                                exe_t[:, :D, :], exf_t[:, :D, :], pmg_b,
                                Alu.mult)
                            nc.vector.tensor_tensor(
                                exo_t[:, :D, :], exf_t[:, :D, :],
                                exe_t[:, :D, :], Alu.subtract)
                            exh_t = smallp.tile([128, Dmax2, 2, HID],
                                                dt.bfloat16, tag="exh")
                            nc.vector.tensor_tensor(
                                exh_t[:, :D, 0, :],
                                gb_t[:, off:off + D, 0:HID],
                                exe_t[:, :D, :].broadcast_to(
                                    (128, D, HID)), Alu.mult)
                            nc.vector.tensor_tensor(
                                exh_t[:, :D, 1, :],
                                gb_t[:, off:off + D, 64:64 + HID],
                                exo_t[:, :D, :].broadcast_to(
                                    (128, D, HID)), Alu.mult)
                            agg = aggp.tile([128, HID], dt.float32, tag="agg")
                            for bi in range(2 * D):
                                nc.tensor.matmul(
                                    agg[:], id_t[:],
                                    exh_t[:, bi // 2, bi % 2, :],
                                    start=(bi == 0),
                                    stop=(bi == 2 * D - 1))
                            scaled_t = epip.tile([128, HID], dt.float32,
                                                 tag="sd")
                            nc.vector.tensor_scalar(scaled_t[:], agg[:],
                                                    rec_t[:], None, Alu.mult)
                            if HASB2:
                                nc.vector.tensor_tensor(
                                    scaled_t[:], scaled_t[:], b2_t[:],
                                    Alu.add)
                            tmp_t = epip.tile([128, HID], dt.float32,
                                              tag="tm")
                            nc.scalar.activation(tmp_t[:], scaled_t[:],
                                                 Act.Relu, scale=-1.0)
                            nc.scalar.activation(tmp_t[:], tmp_t[:], Act.Exp,
                                                 scale=-1.0)
                            nc.vector.scalar_tensor_tensor(
                                h2p_all[:, g, :], tmp_t[:], -1.0, scaled_t[:],
                                Alu.add, Alu.max)
                            mp_t = mpp.tile([128, GPOOL], dt.bfloat16,
                                            tag="mp")
                            nc.sync.dma_start(
                                mp_t[:], t_mpool[g * 128:(g + 1) * 128, :])
                            nc.tensor.matmul(poolps[:], h2p_all[:, g, :],
                                             mp_t[:], start=(g == 0),
                                             stop=(g == NG2 - 1))
                            off += D
                    # ------------- pool + final linear -------------
                    with tc.tile_pool(name="fin", bufs=1) as finp, \
                            tc.tile_pool(name="finps", bufs=1,
                                         space="PSUM") as fpp:
                        poolsb = finp.tile([HID, GPOOL], dt.float32)
                        nc.vector.tensor_copy(poolsb[:], poolps[:])
                        nc.sync.dma_start(cc_in[:, :], poolsb[:])
                        nc.gpsimd.collective_compute(
                            "AllReduce", Alu.add, replica_groups=rg,
                            ins=[cc_in[:, :]], outs=[cc_out[:, :]])
                        psum_t = finp.tile([HID, GPOOL], dt.float32)
                        nc.sync.dma_start(psum_t[:], cc_out[:, :])
                        mean_t = finp.tile([HID, GPOOL], dt.float32)
                        nc.vector.tensor_tensor(
                            mean_t[:], psum_t[:],
                            rc_t[:], Alu.mult)
                        psO = fpp.tile([GPOOL, OUT], dt.float32)
                        nc.tensor.matmul(psO[:], mean_t[:], wl_t[:],
                                         start=True, stop=True)
                        out_t = finp.tile([GPOOL, OUT], dt.float32)
                        if HASBL:
                            nc.vector.tensor_tensor(out_t[:], psO[:], bl_t[:],
                                                    Alu.add)
                        else:
                            nc.vector.tensor_copy(out_t[:], psO[:])
                        nc.sync.dma_start(t_out[:, :], out_t[:])
            if PHASES < 4:
                with tc.tile_pool(name='dummy', bufs=1) as dp:
                    d = dp.tile([GPOOL, OUT], dt.float32)
                    nc.vector.memset(d[:], 0.0)
                    nc.sync.dma_start(t_out[:, :], d[:])

    nc.compile()
    return nc


def core_inputs(prep, c):
    cd = prep["cores"][c]
    sc_D1, sc_D2 = prep["D1"], prep["D2"]
    NB1, NB2 = sum(sc_D1), sum(sc_D2)
    n1 = max(8 * NB1, 8)
    n2 = max(8 * NB2, 8)

    def padcols(a, cols, dtype):
        if a.shape[1] == cols:
            return np.ascontiguousarray(a)
        out = np.zeros((a.shape[0], cols), dtype)
        out[:, :a.shape[1]] = a
        return out

    return dict(
        xT=np.ascontiguousarray(cd["xT"]),
        w1=prep["W1"], w2ext=prep["W2ext"], wl=prep["Wl"],
        a1x=prep["a1x"],
        b1=prep["b1"], b2=prep["b2"], bl=prep["bl"], rcnt=prep["rcnt"],
        patch1=prep["patch1"], patch2=prep["patch2"], ident=prep["ident"],
        mpool=np.ascontiguousarray(cd["mpool"]),
        idx1=padcols(cd["w_idx1"], n1, np.int16),
        idx2=padcols(cd["w_idx2"], n2, np.int16),
        pme1=padcols(cd["pme1"], max(NB1, 1), np.float32).astype(bf16),
        pme2=padcols(cd["pme2"], max(NB2, 1), np.float32),
        scat1=cd["w_scat1"],
    )


_CACHE = {}


def kernel(**inputs):
    from concourse.bass_utils import run_bass_kernel_spmd

    inputs = {k: np.asarray(v) for k, v in inputs.items()}
    prep = host_prep(**inputs)
    sc = make_sched(prep)
    sc["D1"] = prep["D1"]
    sc["D2"] = prep["D2"]
    key = str(sc)
    if key not in _CACHE:
        _CACHE[key] = build_bass(sc)
    nc = _CACHE[key]
    in_maps = [core_inputs(prep, c) for c in range(NCORES)]
    res = run_bass_kernel_spmd(nc, in_maps, list(range(NCORES)))
    return np.asarray(res.results[0]["out"], np.float32)


# revision 34
# speedup vs baseline: 1.8472x; 1.0188x over previous
# Self-contained 8-core Trainium2 Bass kernel for the 2-layer GAT + mean-pool
# problem (nn_GAT_83820581749190).
#
# Sharding: destination nodes (and all their incident edges) are partitioned
# across the 8 cores, so each layer's attention softmax and aggregation
# complete locally per core. Each core builds a replicated layer-1 feature
# table h1 (bf16, 256-byte logical rows) in HBM with a replicated x @ W1
# matmul, then edge-gathers PAIRS of rows (512B per descriptor, index =
# row//2, int16-safe) with the GPSIMD dma_gather custom op; host-precomputed
# parity masks select the correct half downstream. Attention logits are
# computed on-chip (DVE dot with a_src/a_dst), the edge softmax runs without
# segment-max (logits are small; pad slots use a patch row whose h gives
# al_src=-100), and aggregation is identity-matmul PSUM accumulation
# (destinations on partitions via degree-bucketed groups of 128).
# Layer-2 features are exchanged with an AllGather; the same paired-row
# gather runs against the fp32 layer-2 table; mean-pool is a matmul against
# a host-built one-hot graph matrix plus a tiny AllReduce.
import numpy as np
import ml_dtypes

N = 50000
E = 800000
IN = 128
HID = 32
HEADS = 4
OUT = 10
GPOOL = 64
NEG = 0.2
NCORES = 8
S = N // NCORES
SPECIAL1 = N          # layer-1 patch row (h chosen so h . a_src = -100)
SPECIAL2 = 0          # layer-2 patch row (al_src column = -100)
SPECIAL_ALS = -100.0
SB_BLOCK_BUDGET = 24  # max gather blocks per superblock
XCHUNK = 512
PHASES = 99
L1STEP = 99

bf16 = ml_dtypes.bfloat16


def _ceil_to(v, m):
    return (v + m - 1) // m * m


# ======================= host prep =========================================

def _build_layer(src, dstl):
    deg = np.bincount(dstl, minlength=S)
    P = np.argsort(-deg, kind="stable")
    Ppos = np.empty(S, np.int64)
    Ppos[P] = np.arange(S)
    ng = (S + 127) // 128
    D = np.zeros(ng, np.int64)
    dp = deg[P]
    for g in range(ng):
        D[g] = dp[g * 128:(g + 1) * 128].max()
    assert (D > 0).all()
    return dict(src=src, dstl=dstl, deg=deg, P=P, Ppos=Ppos, D=D)


def _emit_slots(l, Dg, row_of_src, special_row):
    """Per group g: rows[g] [D[g],128] of table ROW ids (special_row pads),
    plus slot2cmp mapping output slots -> compacted dst ids."""
    NG = len(Dg)
    Ppos = l["Ppos"]
    nreal = S
    slot2cmp = np.full(NG * 128, -1, np.int64)
    slot2cmp[:nreal] = np.arange(nreal)
    rows = [np.full((int(Dg[g]), 128), special_row, np.int64)
            for g in range(NG)]
    slot_of_edge = Ppos[l["dstl"]]
    order = np.argsort(slot_of_edge, kind="stable")
    so = slot_of_edge[order]
    sr = row_of_src[l["src"][order]]
    jj = np.arange(len(so)) - np.searchsorted(so, so, side="left")
    gg, kk = so // 128, so % 128
    for g in range(NG):
        sel = gg == g
        if sel.any():
            rows[g][jj[sel], kk[sel]] = sr[sel]
    return rows, slot2cmp


def _wrap16(idx):
    """[n] -> [128, n//16] int16: idx i at [i%16, i//16], replicated x8."""
    n = len(idx)
    assert n % 16 == 0
    w = np.ascontiguousarray(np.asarray(idx).reshape(n // 16, 16).T)
    w = w.astype(np.int16)
    return np.tile(w, (8, 1))


def _wrap_rows(rows_arrs):
    """idx stream (row//2) wrapped, plus even-parity masks [128, NB]."""
    idx_segs = []
    pme_segs = []
    for a in rows_arrs:
        if a.size:
            assert (a // 2 <= 32767).all()
            idx_segs.append(_wrap16((a // 2).reshape(-1)))
            pme_segs.append(np.ascontiguousarray((1 - (a % 2)).T))
    w_idx = (np.concatenate(idx_segs, axis=1) if idx_segs
             else np.zeros((128, 0), np.int16))
    pme = (np.concatenate(pme_segs, axis=1).astype(np.float32) if pme_segs
           else np.zeros((128, 0), np.float32))
    return w_idx, pme


def host_prep(x, edge_index, batch, W1, a1_src, a1_dst, b1, W2, a2_src, a2_dst,
              b2, Wl, bl):
    x = np.asarray(x, np.float32)
    edge_index = np.asarray(edge_index, np.int64)
    batch = np.asarray(batch, np.int64)
    src_all = np.concatenate([edge_index[0], np.arange(N, dtype=np.int64)])
    dst_all = np.concatenate([edge_index[1], np.arange(N, dtype=np.int64)])
    owner = dst_all // S

    a1_src = np.asarray(a1_src, np.float32)
    a1_dst = np.asarray(a1_dst, np.float32)
    W1 = np.asarray(W1, np.float32)
    W2 = np.asarray(W2, np.float32)
    W2ext = np.concatenate(
        [W2, W2 @ np.asarray(a2_src, np.float32)[0][:, None],
         W2 @ np.asarray(a2_dst, np.float32)[0][:, None]], axis=1)  # [128,34]

    # a1x: [0:128]=a_src flat, [128:256]=a_src flat, [256:384]=a_dst flat
    asf = a1_src.reshape(-1)
    adf = a1_dst.reshape(-1)
    a1x = np.tile(np.concatenate([asf, asf, adf])[None, :], (128, 1))

    # layer-1 patch row: h with h . a_src[h] = -100 for every head
    hp = np.concatenate([SPECIAL_ALS * a1_src[h] / (a1_src[h] ** 2).sum()
                         for h in range(HEADS)])
    assert np.abs(hp).max() < 1e4
    patch1 = np.tile(hp[None, :], (1, 1))

    cores = [dict(c=c) for c in range(NCORES)]
    for cd in cores:
        c = cd["c"]
        m = owner == c
        cd["src"] = src_all[m]
        cd["dstl"] = dst_all[m] - c * S

    # ---------- layer 1 ----------
    for cd in cores:
        c = cd["c"]
        l1 = _build_layer(cd["src"], cd["dstl"])
        pos_of = np.empty(N, np.int64)
        own = np.arange(c * S, (c + 1) * S)
        oth = np.concatenate([np.arange(0, c * S), np.arange((c + 1) * S, N)])
        pos_of[oth] = S + np.arange(N - S)
        pos_of[own] = l1["Ppos"]
        cd["l1"] = l1
        cd["row_of"] = pos_of
    NG1 = max(len(cd["l1"]["D"]) for cd in cores)
    D1 = np.zeros(NG1, np.int64)
    for cd in cores:
        d = cd["l1"]["D"]
        D1[:len(d)] = np.maximum(D1[:len(d)], d)
    for cd in cores:
        cd["rows1"], cd["slot2cmp1"] = _emit_slots(
            cd["l1"], D1, cd["row_of"], SPECIAL1)

    # ---------- layer 2 ----------
    # layer-2 features live in a blocked bf16 table: core c's partition p,
    # group g at flat row (c*128+p)*NG2 + g (64 bf16 each; pairs of flat
    # rows share one 256B gather descriptor)
    for cd in cores:
        cd["l2"] = _build_layer(cd["src"], cd["dstl"])
    NG2 = max(len(cd["l2"]["D"]) for cd in cores)
    D2 = np.zeros(NG2, np.int64)
    for cd in cores:
        d = cd["l2"]["D"]
        D2[:len(d)] = np.maximum(D2[:len(d)], d)
    flat2_of = np.empty(N, np.int64)
    for cd in cores:
        c = cd["c"]
        q = cd["l2"]["Ppos"]
        flat2_of[c * S:(c + 1) * S] = \
            (c * 128 + q % 128) * NG2 + q // 128
    for cd in cores:
        c = cd["c"]
        special2 = (c * 128 + S % 128) * NG2 + S // 128  # own trash row
        cd["rows2"], cd["slot2cmp2"] = _emit_slots(
            cd["l2"], D2, flat2_of, special2)

    # ---------- aux ----------
    cnt = np.bincount(batch, minlength=GPOOL).astype(np.float32)
    recip_cnt = (1.0 / np.maximum(cnt, 1.0)).astype(np.float32)

    XT_COLS = _ceil_to(N + 2, XCHUNK)
    for cd in cores:
        c = cd["c"]
        gids = batch[c * S:(c + 1) * S]
        Mp = np.zeros((NG2 * 128, GPOOL), np.float32)
        s2c = cd["slot2cmp2"]
        real = s2c >= 0
        Mp[np.where(real)[0], gids[cd["l2"]["P"][s2c[real]]]] = 1.0
        cd["mpool"] = Mp.astype(bf16)

        s2c1 = cd["slot2cmp1"]
        tgt = np.full(len(s2c1), S, np.int64)  # trash row for dummy slots
        r1 = s2c1 >= 0
        tgt[r1] = cd["l2"]["Ppos"][cd["l1"]["P"][s2c1[r1]]]

        xt = np.zeros((IN, XT_COLS), np.float32)
        xt[:, cd["row_of"]] = x.T
        cd["xT"] = xt.astype(bf16)

        cd["w_idx1"], cd["pme1"] = _wrap_rows(cd["rows1"])
        cd["w_idx2"], cd["pme2"] = _wrap_rows(cd["rows2"])
        cd["w_scat1"] = _wrap16(tgt)

    # written over the trash row after the scatter: al_src=-100 kills pads
    patch2 = np.zeros((1, 64), np.float32)
    patch2[0, 32] = SPECIAL_ALS

    return dict(cores=cores,
                D1=[int(v) for v in D1], D2=[int(v) for v in D2],
                W1=W1.astype(bf16), W2ext=W2ext.astype(bf16),
                Wl=np.asarray(Wl, np.float32),
                a1x=a1x.astype(bf16),
                b1=np.tile(np.asarray(b1, np.float32).reshape(1, -1),
                           (128, 1)),
                b2=np.tile(np.asarray(b2, np.float32).reshape(1, -1),
                           (128, 1)),
                bl=np.tile(np.asarray(bl, np.float32).reshape(1, -1),
                           (GPOOL, 1)),
                rcnt=np.tile(recip_cnt.reshape(1, -1), (HID, 1)),
                patch1=patch1.astype(bf16), patch2=patch2,
                ident=np.eye(128, dtype=bf16))


def _pack_superblocks(D, budget=SB_BLOCK_BUDGET):
    sbs, cur, tot = [], [], 0
    for g in range(len(D)):
        d = int(D[g])
        if cur and tot + d > budget:
            sbs.append(cur)
            cur, tot = [], 0
        cur.append(g)
        tot += d
    if cur:
        sbs.append(cur)
    return sbs


def make_sched(prep):
    D1, D2 = prep["D1"], prep["D2"]
    return dict(D1=D1, D2=D2,
                SB1=_pack_superblocks(D1), SB2=_pack_superblocks(D2),
                HASB1=bool(np.any(prep["b1"])), HASB2=bool(np.any(prep["b2"])),
                HASBL=bool(np.any(prep["bl"])))


# ======================= bass kernel =======================================

def build_bass(sc):
    import concourse.bacc as bacc
    import concourse.tile as tile
    import concourse.mybir as mybir
    from concourse.library_config import mlp

    dt = mybir.dt
    Alu = mybir.AluOpType
    Act = mybir.ActivationFunctionType
    Axis = mybir.AxisListType

    D1, D2 = sc["D1"], sc["D2"]
    SB1, SB2 = sc["SB1"], sc["SB2"]
    HASB1 = sc.get("HASB1", True)
    HASB2 = sc.get("HASB2", True)
    HASBL = sc.get("HASBL", True)
    NG1, NG2 = len(D1), len(D2)
    XT_COLS = _ceil_to(N + 2, XCHUNK)
    NCHUNK = XT_COLS // XCHUNK
    SH2_ROWS = _ceil_to(S + 2, 128)
    NB1 = sum(D1)
    NB2 = sum(D2)
    o1 = np.concatenate([[0], np.cumsum(D1)]).astype(int)
    o2 = np.concatenate([[0], np.cumsum(D2)]).astype(int)

    nc = bacc.Bacc("TRN2", target_bir_lowering=False, debug=False,
                   num_devices=NCORES, num_swdge_queues=4)

    t_xT = nc.dram_tensor("xT", [IN, XT_COLS], dt.bfloat16,
                          kind="ExternalInput")
    t_w1 = nc.dram_tensor("w1", [IN, IN], dt.bfloat16, kind="ExternalInput")
    t_w2 = nc.dram_tensor("w2ext", [IN, 34], dt.bfloat16,
                          kind="ExternalInput")
    t_wl = nc.dram_tensor("wl", [HID, OUT], dt.float32, kind="ExternalInput")
    t_a1x = nc.dram_tensor("a1x", [128, 384], dt.bfloat16,
                           kind="ExternalInput")
    t_b1 = nc.dram_tensor("b1", [128, HEADS * HID], dt.float32,
                          kind="ExternalInput")
    t_b2 = nc.dram_tensor("b2", [128, HID], dt.float32, kind="ExternalInput")
    t_bl = nc.dram_tensor("bl", [GPOOL, OUT], dt.float32,
                          kind="ExternalInput")
    t_rcnt = nc.dram_tensor("rcnt", [HID, GPOOL], dt.float32,
                            kind="ExternalInput")
    t_patch1 = nc.dram_tensor("patch1", [1, 128], dt.bfloat16,
                              kind="ExternalInput")
    t_patch2 = nc.dram_tensor("patch2", [1, 64], dt.float32,
                              kind="ExternalInput")
    t_ident = nc.dram_tensor("ident", [128, 128], dt.bfloat16,
                             kind="ExternalInput")
    t_mpool = nc.dram_tensor("mpool", [NG2 * 128, GPOOL], dt.bfloat16,
                             kind="ExternalInput")
    n1 = max(8 * NB1, 8)
    n2 = max(8 * NB2, 8)
    t_i1 = nc.dram_tensor("idx1", [128, n1], dt.int16, kind="ExternalInput")
    t_i2 = nc.dram_tensor("idx2", [128, n2], dt.int16, kind="ExternalInput")
    t_pm1 = nc.dram_tensor("pme1", [128, max(NB1, 1)], dt.bfloat16,
                           kind="ExternalInput")
    t_pm2 = nc.dram_tensor("pme2", [128, max(NB2, 1)], dt.float32,
                           kind="ExternalInput")
    t_scat1 = nc.dram_tensor("scat1", [128, 8 * NG1], dt.int16,
                             kind="ExternalInput")
    t_out = nc.dram_tensor("out", [GPOOL, OUT], dt.float32,
                           kind="ExternalOutput")

    rg = [list(range(NCORES))]
    _qc = [0]

    def nextq():
        _qc[0] = (_qc[0] + 1) % 4
        return _qc[0]

    with tile.TileContext(nc) as tc:
        with (
            tc.tile_pool(name="const", bufs=1) as constp,
            tc.tile_pool(name="pre", bufs=1) as prep_pool,
            tc.tile_pool(name="dram", bufs=1, space="DRAM") as dramp,
        ):
            nc.gpsimd.load_library(mlp)

            # logical row-major tables; gathers view them as paired rows
            table1 = dramp.tile([XT_COLS, 128], dt.bfloat16, tag="table1")
            h2b = dramp.tile([128, NG2 * 64], dt.bfloat16, tag="h2b")
            table2b = dramp.tile([128 * NCORES * NG2 * 64], dt.bfloat16,
                                 tag="table2b")
            h2sh = dramp.tile([SH2_ROWS, 64], dt.float32, tag="h2sh")
            cc_in = dramp.tile([HID, GPOOL], dt.float32, tag="ccin")
            cc_out = dramp.tile([HID, GPOOL], dt.float32, tag="ccout")

            w1_t = constp.tile([IN, IN], dt.bfloat16)
            nc.sync.dma_start(w1_t[:], t_w1[:])
            w2_t = constp.tile([IN, 34], dt.bfloat16)
            nc.sync.dma_start(w2_t[:], t_w2[:])
            wl_t = constp.tile([HID, OUT], dt.float32)
            nc.sync.dma_start(wl_t[:], t_wl[:])
            a1x_t = constp.tile([128, 384], dt.bfloat16)
            nc.sync.dma_start(a1x_t[:], t_a1x[:])
            b1_t = constp.tile([128, HEADS * HID], dt.float32)
            nc.sync.dma_start(b1_t[:], t_b1[:])
            b2_t = constp.tile([128, HID], dt.float32)
            nc.sync.dma_start(b2_t[:], t_b2[:])
            bl_t = constp.tile([GPOOL, OUT], dt.float32)
            nc.sync.dma_start(bl_t[:], t_bl[:])
            rc_t = constp.tile([HID, GPOOL], dt.float32)
            nc.sync.dma_start(rc_t[:], t_rcnt[:])
            id_t = constp.tile([128, 128], dt.bfloat16)
            nc.sync.dma_start(id_t[:], t_ident[:])

            # preload all gather indices and parity masks
            i1_all = prep_pool.tile([128, n1], dt.int16)
            nc.sync.dma_start(i1_all[:], t_i1[:])
            pm1_t = prep_pool.tile([128, max(NB1, 1)], dt.bfloat16)
            nc.sync.dma_start(pm1_t[:], t_pm1[:])
            i2_all = prep_pool.tile([128, n2], dt.int16)
            pm2_t = prep_pool.tile([128, max(NB2, 1)], dt.float32)
            scat1_t = prep_pool.tile([128, 8 * NG1], dt.int16)
            nc.sync.dma_start(scat1_t[:], t_scat1[:])
            ald1_t = prep_pool.tile([128, NG1, 4], dt.float32)
            ald2_t = prep_pool.tile([128, NG2, 1], dt.float32)

            # zero the scatter_add target
            with tc.tile_pool(name="zp", bufs=1) as zp:
                z_t = zp.tile([128, SH2_ROWS // 128 * 64], dt.float32)
                nc.vector.memset(z_t[:], 0.0)
                nc.sync.dma_start(
                    h2sh[:, :].rearrange("(p k) e -> p (k e)", p=128), z_t[:])

            # ---------------- phase X: build table1 ----------------
            with (
                tc.tile_pool(name="xload", bufs=3) as xlp,
                tc.tile_pool(name="xout", bufs=3) as xop,
                tc.tile_pool(name="xpsum", bufs=4, space="PSUM") as xpp,
            ):
                for t in range(NCHUNK):
                    # alternate loads/writes across the two HWDGE rings so
                    # neither ring serializes the whole 25.6MB stream
                    ld_eng = nc.sync if t % 2 == 0 else nc.scalar
                    st_eng = nc.scalar if t % 2 == 0 else nc.sync
                    xt_t = xlp.tile([IN, XCHUNK], dt.bfloat16, tag="xt")
                    ld_eng.dma_start(xt_t[:],
                                     t_xT[:, t * XCHUNK:(t + 1) * XCHUNK])
                    o_t = xop.tile([128, 4, 128], dt.bfloat16, tag="xo")
                    for k in range(4):
                        ps = xpp.tile([128, 128], dt.float32, tag="xp")
                        nc.tensor.matmul(ps[:], xt_t[:, k * 128:(k + 1) * 128],
                                         w1_t[:], start=True, stop=True)
                        if k % 2 == 0:
                            nc.vector.tensor_copy(o_t[:, k, :], ps[:])
                        else:
                            nc.scalar.activation(o_t[:, k, :], ps[:],
                                                 Act.Copy)
                        gix = 4 * t + k
                        if gix < NG1:
                            # own-row ald = h . a_dst, straight off the PSUM
                            ap_t = xop.tile([128, 128], dt.bfloat16,
                                            tag="apr")
                            nc.vector.tensor_tensor(
                                ap_t[:], ps[:], a1x_t[:, 256:384], Alu.mult)
                            nc.vector.tensor_reduce(
                                ald1_t[:, gix, :],
                                ap_t[:].rearrange("p (h c) -> p h c", h=4),
                                axis=Axis.X, op=Alu.add)
                    st_eng.dma_start(
                        table1[t * XCHUNK:(t + 1) * XCHUNK, :].rearrange(
                            "(k p) e -> p k e", p=128), o_t[:])
            with tc.tile_pool(name="patchp", bufs=1) as pp:
                p1_t = pp.tile([1, 128], dt.bfloat16)
                nc.sync.dma_start(p1_t[:], t_patch1[:])
                nc.sync.dma_start(table1[SPECIAL1:SPECIAL1 + 1, :],
                                  p1_t[0:1, :])
            p2_t = prep_pool.tile([1, 64], dt.float32)
            nc.sync.dma_start(p2_t[:], t_patch2[:])

            if PHASES >= 2:
                # ---------------- phase L1: edges ----------------
                tab1p = table1[:, :].rearrange("(a h) c -> a (h c)", h=2)
                Dmax1 = max(D1)
                NBSB1 = max(sum(D1[g] for g in sb) for sb in SB1)
                with (
                    tc.tile_pool(name="gath1", bufs=5) as gathp,
                    tc.tile_pool(name="als1", bufs=2) as alsp,
                    tc.tile_pool(name="small1", bufs=3) as smallp,
                    tc.tile_pool(name="epi1", bufs=3) as epip,
                    tc.tile_pool(name="scatp", bufs=1) as scatp,
                    tc.tile_pool(name="agg1", bufs=2, space="PSUM") as aggp,
                    tc.tile_pool(name="psT1", bufs=2, space="PSUM") as psTp,
                    tc.tile_pool(name="ps21", bufs=2, space="PSUM") as ps2p,
                ):
                    scat_t = scatp.tile([128, NG1, 64], dt.float32, tag="sc")
                    nc.vector.memset(scat_t[:], 0.0)
                    elu_all = scatp.tile([128, NG1, 128], dt.bfloat16,
                                         tag="eluall")
                    scat_done = [0]

                    def flush_scatter(upto):
                        g0s = scat_done[0]
                        ngk = upto - g0s
                        if ngk <= 0:
                            return
                        nc.gpsimd.dma_scatter_add(
                            h2sh[0:S + 1, :], scat_t[:, g0s:upto, :],
                            scat1_t[:, 8 * g0s:8 * upto],
                            128 * ngk, 128 * ngk, 64,
                            single_packet=False, queue_num=nextq())
                        scat_done[0] = upto

                    for sb in SB1:
                        g0 = sb[0]
                        nb = sum(D1[g] for g in sb)
                        boff = o1[g0]
                        gb_t = gathp.tile([128, NBSB1, 256],
                                          dt.bfloat16, tag="gb")
                        nc.gpsimd.dma_gather(
                            gb_t[:, :nb, :], tab1p,
                            i1_all[:, 8 * boff:8 * (boff + nb)],
                            128 * nb, 128 * nb, 256,
                            single_packet=False, queue_num=nextq())
                        if L1STEP < 2:
                            continue
                        # al_src for both pair-halves: prod + reduce
                        prod_t = alsp.tile([128, NBSB1, 256], dt.bfloat16,
                                           tag="prod")
                        als8_t = alsp.tile([128, NBSB1, 8], dt.float32,
                                           tag="als8")
                        als_t = alsp.tile([128, NBSB1, 4], dt.float32,
                                          tag="als")
                        nc.vector.tensor_tensor(
                            prod_t[:, :nb, :], gb_t[:, :nb, :],
                            a1x_t[:, 0:256].unsqueeze(1).broadcast_to(
                                (128, nb, 256)), Alu.mult)
                        nc.vector.tensor_reduce(
                            als8_t[:, :nb, :],
                            prod_t[:, :nb, :].rearrange(
                                "p b (j c) -> p b j c", j=8),
                            axis=Axis.X, op=Alu.add)
                        # parity-select: als = even*pme + odd*(1-pme)
                        #              = odd - (odd-even)*pme
                        pme_b = pm1_t[:, boff:boff + nb].unsqueeze(
                            2).broadcast_to((128, nb, 4))
                        t1_t = alsp.tile([128, NBSB1, 4], dt.float32,
                                         tag="t1")
                        nc.vector.tensor_tensor(
                            t1_t[:, :nb, :], als8_t[:, :nb, 4:8],
                            als8_t[:, :nb, 0:4], Alu.subtract)
                        nc.vector.tensor_tensor(
                            t1_t[:, :nb, :], t1_t[:, :nb, :], pme_b,
                            Alu.mult)
                        nc.vector.tensor_tensor(
                            als_t[:, :nb, :], als8_t[:, :nb, 4:8],
                            t1_t[:, :nb, :], Alu.subtract)
                        off = 0
                        for gi, g in enumerate(sb):
                            D = D1[g]
                            if L1STEP < 3:
                                off += D
                                continue
                            logit_t = smallp.tile([128, Dmax1, 4], dt.float32,
                                                  tag="lg")
                            exb_t = smallp.tile([128, Dmax1, 4], dt.bfloat16,
                                                tag="exb")
                            exe_t = smallp.tile([128, Dmax1, 4], dt.bfloat16,
                                                tag="exe")
                            exo_t = smallp.tile([128, Dmax1, 4], dt.bfloat16,
                                                tag="exo")
                            den_t = smallp.tile([128, 4], dt.float32,
                                                tag="dn")
                            rec_t = smallp.tile([128, 4], dt.float32,
                                                tag="rc")
                            ald_ap = ald1_t[:, g, :]
                            nc.vector.scalar_tensor_tensor(
                                logit_t[:, :D, :], als_t[:, off:off + D, :],
                                0.0,
                                ald_ap.unsqueeze(1).broadcast_to(
                                    (128, D, 4)), Alu.add, Alu.add)
                            nc.vector.scalar_tensor_tensor(
                                logit_t[:, :D, :], logit_t[:, :D, :], NEG,
                                logit_t[:, :D, :], Alu.mult, Alu.max)
                            nc.scalar.activation(exb_t[:, :D, :],
                                                 logit_t[:, :D, :], Act.Exp)
                            nc.vector.tensor_reduce(
                                den_t[:], exb_t[:, :D, :].transpose([0, 2, 1]),
                                axis=Axis.X, op=Alu.add)
                            nc.vector.reciprocal(rec_t[:], den_t[:])
                            pmg = pm1_t[:, boff + off:boff + off + D]
                            pmg_b = pmg.unsqueeze(2).broadcast_to((128, D, 4))
                            nc.vector.tensor_tensor(
                                exe_t[:, :D, :], exb_t[:, :D, :], pmg_b,
                                Alu.mult)
                            nc.vector.tensor_tensor(
                                exo_t[:, :D, :], exb_t[:, :D, :],
                                exe_t[:, :D, :], Alu.subtract)
                            if L1STEP < 4:
                                off += D
                                continue
                            h_e = gb_t[:, off:off + D, 0:128].rearrange(
                                "p b (h c) -> p b h c", h=4)
                            nc.vector.tensor_tensor(
                                h_e, h_e,
                                exe_t[:, :D, :].unsqueeze(3).broadcast_to(
                                    (128, D, 4, HID)), Alu.mult)
                            h_o = gb_t[:, off:off + D, 128:256].rearrange(
                                "p b (h c) -> p b h c", h=4)
                            nc.vector.tensor_tensor(
                                h_o, h_o,
                                exo_t[:, :D, :].unsqueeze(3).broadcast_to(
                                    (128, D, 4, HID)), Alu.mult)
                            if L1STEP < 5:
                                off += D
                                continue
                            agg = aggp.tile([128, 128], dt.float32, tag="agg")
                            for bi in range(2 * D):
                                rhs = gb_t[:, off + bi // 2,
                                           (bi % 2) * 128:(bi % 2 + 1) * 128]
                                nc.tensor.matmul(agg[:], id_t[:], rhs,
                                                 start=(bi == 0),
                                                 stop=(bi == 2 * D - 1))
                            scaled_t = epip.tile([128, 128], dt.float32,
                                                 tag="sd")
                            nc.vector.tensor_tensor(
                                scaled_t[:].rearrange("p (h c) -> p h c", h=4),
                                agg[:].rearrange("p (h c) -> p h c", h=4),
                                rec_t[:].unsqueeze(2).broadcast_to(
                                    (128, 4, HID)), Alu.mult)
                            if HASB1:
                                nc.vector.tensor_tensor(
                                    scaled_t[:], scaled_t[:], b1_t[:],
                                    Alu.add)
                            tmp_t = epip.tile([128, 128], dt.float32,
                                              tag="tm")
                            nc.scalar.activation(tmp_t[:], scaled_t[:],
                                                 Act.Relu, scale=-1.0)
                            nc.scalar.activation(tmp_t[:], tmp_t[:], Act.Exp,
                                                 scale=-1.0)
                            nc.vector.scalar_tensor_tensor(
                                elu_all[:, g, :], tmp_t[:], -1.0, scaled_t[:],
                                Alu.add, Alu.max)
                            off += D
                        # ---- pass 2 for this superblock's groups
                        if L1STEP >= 5:
                            for g in sb:
                                psT = psTp.tile([128, 128], dt.bfloat16,
                                                tag="pt")
                                nc.tensor.transpose(psT[:], elu_all[:, g, :],
                                                    id_t[:])
                                eluT_t = epip.tile([128, 128], dt.bfloat16,
                                                   tag="et")
                                nc.scalar.activation(eluT_t[:], psT[:],
                                                     Act.Copy)
                                ps2 = ps2p.tile([128, 34], dt.float32,
                                                tag="p2")
                                nc.tensor.matmul(ps2[:], eluT_t[:], w2_t[:],
                                                 start=True, stop=True)
                                if g % 2 == 0:
                                    nc.scalar.activation(scat_t[:, g, 0:34],
                                                         ps2[:], Act.Copy)
                                else:
                                    nc.vector.tensor_copy(scat_t[:, g, 0:34],
                                                          ps2[:])
                            # overlap the h2 scatter with the remaining
                            # superblocks (~12-group chunks)
                            if sb[-1] + 1 - scat_done[0] >= 12:
                                flush_scatter(sb[-1] + 1)
                    if L1STEP >= 6:
                        flush_scatter(NG1)
                        # pad slots scatter garbage into the trash row;
                        # overwrite with the al_src=-100 pad row before the
                        # L2 extraction reads it
                        nc.sync.dma_start(h2sh[S:S + 1, :], p2_t[0:1, :])
            if PHASES >= 3:
                # ---------------- exchange ----------------
                # pack own rows to blocked bf16 (also yields the ald column),
                # then AllGather the compact table
                with tc.tile_pool(name="aldtmp2", bufs=1) as atp:
                    atmp = atp.tile([128, NG2, 64], dt.float32)
                    nc.sync.dma_start(
                        atmp[:],
                        h2sh[0:128 * NG2, :].rearrange("(b p) e -> p b e",
                                                       p=128))
                    nc.vector.tensor_copy(ald2_t[:], atmp[:, :, 33:34])
                    atmpb = atp.tile([128, NG2, 64], dt.bfloat16)
                    nc.scalar.activation(atmpb[:], atmp[:], Act.Copy)
                    nc.sync.dma_start(
                        h2b[:, :], atmpb[:].rearrange("p b e -> p (b e)"))
                nc.gpsimd.collective_compute(
                    "AllGather", mybir.AluOpType.bypass, replica_groups=rg,
                    ins=[h2b[:, :]],
                    outs=[table2b[:].rearrange("(r x) -> r x",
                                               r=128 * NCORES)])

            if PHASES >= 4:
                # ---------------- phase L2: edges + pool ----------------
                # preloads below overlap the AllGather
                nc.sync.dma_start(i2_all[:], t_i2[:])
                nc.sync.dma_start(pm2_t[:], t_pm2[:])
                tab2p = table2b[:].rearrange("(y c) -> y c", c=128)
                Dmax2 = max(D2)
                NBSB2 = max(sum(D2[g] for g in sb) for sb in SB2)
                with (
                    tc.tile_pool(name="gath2", bufs=6) as gathp,
                    tc.tile_pool(name="small2", bufs=3) as smallp,
                    tc.tile_pool(name="epi2", bufs=3) as epip,
                    tc.tile_pool(name="agg2", bufs=2, space="PSUM") as aggp,
                    tc.tile_pool(name="poolps", bufs=1,
                                 space="PSUM") as poolpp,
                    tc.tile_pool(name="mp2", bufs=3) as mpp,
                ):
                    poolps = poolpp.tile([HID, GPOOL], dt.float32)
                    h2p_all = mpp.tile([128, NG2, HID], dt.bfloat16,
                                       tag="h2pall", bufs=1)
                    for sb in SB2:
                        g0 = sb[0]
                        nb = sum(D2[g] for g in sb)
                        boff = o2[g0]
                        gb_t = gathp.tile([128, NBSB2, 128], dt.bfloat16,
                                          tag="gb")
                        nc.gpsimd.dma_gather(
                            gb_t[:, :nb, :], tab2p,
                            i2_all[:, 8 * boff:8 * (boff + nb)],
                            128 * nb, 128 * nb, 128, single_packet=False,
                            queue_num=nextq())
                        off = 0
                        for gi, g in enumerate(sb):
                            D = D2[g]
                            logit_t = smallp.tile([128, Dmax2, 1], dt.float32,
                                                  tag="lg")
                            t2_t = smallp.tile([128, Dmax2, 1], dt.float32,
                                               tag="t2")
                            exf_t = smallp.tile([128, Dmax2, 1], dt.float32,
                                                tag="exf")
                            exe_t = smallp.tile([128, Dmax2, 1], dt.float32,
                                                tag="exe")
                            exo_t = smallp.tile([128, Dmax2, 1], dt.float32,
                                                tag="exo")
                            den_t = smallp.tile([128, 1], dt.float32,
                                                tag="dn")
                            rec_t = smallp.tile([128, 1], dt.float32,
                                                tag="rc")
                            ald_ap = ald2_t[:, g, :]
                            pmg = pm2_t[:, boff + off:boff + off + D]
                            pmg_b = pmg.unsqueeze(2)
                            # als = odd - (odd-even)*pme
                            nc.vector.tensor_tensor(
                                t2_t[:, :D, :],
                                gb_t[:, off:off + D, 96:97],
                                gb_t[:, off:off + D, 32:33], Alu.subtract)
                            nc.vector.tensor_tensor(
                                t2_t[:, :D, :], t2_t[:, :D, :], pmg_b,
                                Alu.mult)
                            nc.vector.tensor_tensor(
                                logit_t[:, :D, :],
                                gb_t[:, off:off + D, 96:97],
                                t2_t[:, :D, :], Alu.subtract)
                            nc.vector.tensor_scalar(
                                logit_t[:, :D, :], logit_t[:, :D, :],
                                ald_ap, None, Alu.add)
                            nc.vector.scalar_tensor_tensor(
                                logit_t[:, :D, :], logit_t[:, :D, :], NEG,
                                logit_t[:, :D, :], Alu.mult, Alu.max)
                            nc.scalar.activation(exf_t[:, :D, :],
                                                 logit_t[:, :D, :], Act.Exp)
                            nc.vector.tensor_reduce(
                                den_t[:], exf_t[:, :D, :].transpose([0, 2, 1]),
                                axis=Axis.X, op=Alu.add)
                            nc.vector.reciprocal(rec_t[:], den_t[:])
                            nc.vector.tensor_tensor(
                                exe_t[:, :D, :], exf_t[:, :D, :], pmg_b,
                                Alu.mult)
                            nc.vector.tensor_tensor(
                                exo_t[:, :D, :], exf_t[:, :D, :],
                                exe_t[:, :D, :], Alu.subtract)
                            exh_t = smallp.tile([128, Dmax2, 2, HID],
                                                dt.bfloat16, tag="exh")
                            nc.vector.tensor_tensor(
                                exh_t[:, :D, 0, :],
                                gb_t[:, off:off + D, 0:HID],
                                exe_t[:, :D, :].broadcast_to(
                                    (128, D, HID)), Alu.mult)
                            nc.vector.tensor_tensor(
                                exh_t[:, :D, 1, :],
                                gb_t[:, off:off + D, 64:64 + HID],
                                exo_t[:, :D, :].broadcast_to(
                                    (128, D, HID)), Alu.mult)
                            agg = aggp.tile([128, HID], dt.float32, tag="agg")
                            for bi in range(2 * D):
                                nc.tensor.matmul(
                                    agg[:], id_t[:],
                                    exh_t[:, bi // 2, bi % 2, :],
                                    start=(bi == 0),
                                    stop=(bi == 2 * D - 1))
                            scaled_t = epip.tile([128, HID], dt.float32,
                                                 tag="sd")
                            nc.vector.tensor_scalar(scaled_t[:], agg[:],
                                                    rec_t[:], None, Alu.mult)
                            if HASB2:
                                nc.vector.tensor_tensor(
                                    scaled_t[:], scaled_t[:], b2_t[:],
                                    Alu.add)
                            tmp_t = epip.tile([128, HID], dt.float32,
                                              tag="tm")
                            nc.scalar.activation(tmp_t[:], scaled_t[:],
                                                 Act.Relu, scale=-1.0)
                            nc.scalar.activation(tmp_t[:], tmp_t[:], Act.Exp,
                                                 scale=-1.0)
                            nc.vector.scalar_tensor_tensor(
                                h2p_all[:, g, :], tmp_t[:], -1.0, scaled_t[:],
                                Alu.add, Alu.max)
                            mp_t = mpp.tile([128, GPOOL], dt.bfloat16,
                                            tag="mp")
                            nc.sync.dma_start(
                                mp_t[:], t_mpool[g * 128:(g + 1) * 128, :])
                            nc.tensor.matmul(poolps[:], h2p_all[:, g, :],
                                             mp_t[:], start=(g == 0),
                                             stop=(g == NG2 - 1))
                            off += D
                    # ------------- pool + final linear -------------
                    with tc.tile_pool(name="fin", bufs=1) as finp, \
                            tc.tile_pool(name="finps", bufs=1,
                                         space="PSUM") as fpp:
                        poolsb = finp.tile([HID, GPOOL], dt.float32)
                        nc.vector.tensor_copy(poolsb[:], poolps[:])
                        nc.sync.dma_start(cc_in[:, :], poolsb[:])
                        nc.gpsimd.collective_compute(
                            "AllReduce", Alu.add, replica_groups=rg,
                            ins=[cc_in[:, :]], outs=[cc_out[:, :]])
                        psum_t = finp.tile([HID, GPOOL], dt.float32)
                        nc.sync.dma_start(psum_t[:], cc_out[:, :])
                        mean_t = finp.tile([HID, GPOOL], dt.float32)
                        nc.vector.tensor_tensor(
                            mean_t[:], psum_t[:],
                            rc_t[:], Alu.mult)
                        psO = fpp.tile([GPOOL, OUT], dt.float32)
                        nc.tensor.matmul(psO[:], mean_t[:], wl_t[:],
                                         start=True, stop=True)
                        out_t = finp.tile([GPOOL, OUT], dt.float32)
                        if HASBL:
                            nc.vector.tensor_tensor(out_t[:], psO[:], bl_t[:],
                                                    Alu.add)
                        else:
                            nc.vector.tensor_copy(out_t[:], psO[:])
                        nc.sync.dma_start(t_out[:, :], out_t[:])
            if PHASES < 4:
                with tc.tile_pool(name='dummy', bufs=1) as dp:
                    d = dp.tile([GPOOL, OUT], dt.float32)
                    nc.vector.memset(d[:], 0.0)
                    nc.sync.dma_start(t_out[:, :], d[:])

    nc.compile()
    return nc


def core_inputs(prep, c):
    cd = prep["cores"][c]
    sc_D1, sc_D2 = prep["D1"], prep["D2"]
    NB1, NB2 = sum(sc_D1), sum(sc_D2)
    n1 = max(8 * NB1, 8)
    n2 = max(8 * NB2, 8)

    def padcols(a, cols, dtype):
        if a.shape[1] == cols:
            return np.ascontiguousarray(a)
        out = np.zeros((a.shape[0], cols), dtype)
        out[:, :a.shape[1]] = a
        return out

    return dict(
        xT=np.ascontiguousarray(cd["xT"]),
        w1=prep["W1"], w2ext=prep["W2ext"], wl=prep["Wl"],
        a1x=prep["a1x"],
        b1=prep["b1"], b2=prep["b2"], bl=prep["bl"], rcnt=prep["rcnt"],
        patch1=prep["patch1"], patch2=prep["patch2"], ident=prep["ident"],
        mpool=np.ascontiguousarray(cd["mpool"]),
        idx1=padcols(cd["w_idx1"], n1, np.int16),
        idx2=padcols(cd["w_idx2"], n2, np.int16),
        pme1=padcols(cd["pme1"], max(NB1, 1), np.float32).astype(bf16),
        pme2=padcols(cd["pme2"], max(NB2, 1), np.float32),
        scat1=cd["w_scat1"],
    )


_CACHE = {}


def kernel(**inputs):
    from concourse.bass_utils import run_bass_kernel_spmd

    inputs = {k: np.asarray(v) for k, v in inputs.items()}
    prep = host_prep(**inputs)
    sc = make_sched(prep)
    sc["D1"] = prep["D1"]
    sc["D2"] = prep["D2"]
    key = str(sc)
    if key not in _CACHE:
        _CACHE[key] = build_bass(sc)
    nc = _CACHE[key]
    in_maps = [core_inputs(prep, c) for c in range(NCORES)]
    res = run_bass_kernel_spmd(nc, in_maps, list(range(NCORES)))
    return np.asarray(res.results[0]["out"], np.float32)
